# revision 26
# baseline (speedup 1.0000x reference)
"""Trainium2 Bass kernel for nn_PointWiseMLP (ball query + gather + MLP + pool).

Self-contained: kernel(**inputs) shards across 8 NeuronCores (data-parallel
over batch x query-range), runs the Bass/Tile kernel via run_bass_kernel_spmd,
and gathers the full [2, 128, 8192] output.

v2: support points compacted by support_mask on host (order-preserving, so
"first 32 by index" is unchanged) and padded to N2C=4608; gathers batched in
groups of 4 query tiles to amortize the gpsimd table-walk cost; gv4 built with
8 parallel DMAs; software-pipelined group order (selection of group g+1 is
emitted before the MLP of group g).
"""
import sys
for _p in ("/opt/trn_rl_repo", "/root/.axon_site/_ro/trn_rl_repo"):
    if _p not in sys.path:
        sys.path.append(_p)


import numpy as np
from contextlib import ExitStack

import concourse.bass as bass
import concourse.tile as tile
from concourse import mybir
from concourse._compat import with_exitstack

F32 = mybir.dt.float32
F16 = mybir.dt.float16
BF16 = mybir.dt.bfloat16
I16 = mybir.dt.int16

RADIUS = 0.1
NSAMPLE = 32
EPS = 1e-5
N2C = 4608         # compacted+padded support count
PAIRS = N2C // 16  # 288
WORDS = N2C // 8   # 576
NQ = 2048          # queries per core
NQT = 16           # query tiles per core
GT = 4             # query tiles per gather group
NGRP = NQT // GT   # 4
BIG = 1024.0   # exactly representable in fp16
R2 = float(np.float32(0.01))  # threshold as f32
DROWS = 15     # fp16 d2 decomposition rows
CHUNKS = [(0, 1024), (1024, 1024), (2048, 1024), (3072, 1024), (4096, 512)]

ALU = mybir.AluOpType
ACTF = mybir.ActivationFunctionType


# --------------------------------------------------------------------------
# host-side preparation
# --------------------------------------------------------------------------

def _split_hilo(x, grid=1024.0):
    """Grid split: x = hi + lo with hi on 1/grid grid (exact in fp16 for the
    value ranges used here)."""
    x = x.astype(np.float32)
    hi = np.floor(x.astype(np.float64) * grid) / grid
    hi = hi.astype(np.float32)
    lo = (x - hi).astype(np.float32)
    return hi, lo


def host_prep(inputs):
    B = 2
    qx = np.asarray(inputs['query_xyz'], np.float32)
    sx = np.asarray(inputs['support_xyz'], np.float32)
    qm = np.asarray(inputs['query_mask'], np.int32)
    sm = np.asarray(inputs['support_mask'], np.int32)
    sf = np.asarray(inputs['support_features'], np.float32)

    W0 = np.asarray(inputs['W0'], np.float64)
    W1 = np.asarray(inputs['W1'], np.float64)
    W2 = np.asarray(inputs['W2'], np.float64)

    def fold(Wl, g, b, rm, rv):
        s = np.asarray(g, np.float64) / np.sqrt(np.asarray(rv, np.float64) + EPS)
        return Wl * s[:, None], np.asarray(b, np.float64) - np.asarray(rm, np.float64) * s

    W0p, t0 = fold(W0, inputs['g0'], inputs['b0'], inputs['rm0'], inputs['rv0'])
    W1p, t1 = fold(W1, inputs['g1'], inputs['b1'], inputs['rm1'], inputs['rv1'])
    W2p, t2 = fold(W2, inputs['g2'], inputs['b2'], inputs['rm2'], inputs['rv2'])

    P0 = W0p[:, 0:3] / RADIUS
    C0 = W0p[:, 3:67]
    D0 = W0p[:, 67:131]

    w1t4 = np.tile(W1p.T.astype(np.float32), (4, 1))       # [128, 32]
    w2t = W2p.T.astype(np.float32)                         # [32, 128]

    t1v = t1.astype(np.float32).reshape(32, 1)
    t2v = t2.astype(np.float32).reshape(128, 1)

    # permutation matmul weights for the wrapped gather index layout:
    # idxw[p, 2r+h] = idxg[32*(p//32) + 16h + p%16, r]
    Mh = np.zeros((2, 128, 128), np.float32)
    for h in range(2):
        for p in range(128):
            Mh[h, 32 * (p // 32) + 16 * h + p % 16, p] = 1.0
    ident = np.eye(128, dtype=np.float32)

    pow8 = np.tile((2.0 ** (np.arange(1024) % 8)).astype(np.float32)[None, :], (128, 1))
    iotag = np.tile(((np.arange(GT * PAIRS, dtype=np.int16) % PAIRS) + 1)[None, :],
                    (128, 1))
    shv = np.tile(np.arange(8, dtype=np.int16)[None, :], (128, 1))
    tpat = np.tile((np.tile(np.arange(16, dtype=np.float32), 34) - 15.0)[None, :],
                   (128, 1))
    onesk1 = np.ones((1, 128), np.float32)

    batch_sup = []
    for b in range(B):
        # order-preserving compaction by support_mask; original point 0 is
        # always table entry 0 (selection-masked if its mask is 0) so the
        # zero-neighbor fill gathers the same point the reference does.
        valid = sm[b] > 0
        keep = np.nonzero(valid)[0]
        sel0 = True
        if not valid[0]:
            keep = np.concatenate([[0], keep])
            sel0 = False
        nv = len(keep)
        assert nv <= N2C, (nv, N2C)
        s = np.zeros((N2C, 3), np.float32)
        s[:nv] = sx[b][keep]
        fts = np.zeros((64, N2C), np.float32)
        fts[:, :nv] = sf[b][:, keep]
        selmask = np.zeros(N2C, np.float32)
        selmask[:nv] = 1.0
        if not sel0:
            selmask[0] = 0.0

        # fp16 d2 decomposition: all "hi" rows exact in fp16; residual rows
        # are O(2^-7) so fp16 rounding perturbs d2 by only ~1e-6.
        sh, sl = _split_hilo(s)
        s64, sh64 = s.astype(np.float64), sh.astype(np.float64)
        Ls = (np.sum(s64 * s64, 1) - np.sum(sh64 * sh64, 1)).astype(np.float32)
        sh2 = np.sum(sh64 * sh64, 1).astype(np.float32)
        hi_s, lo_s = _split_hilo(sh2, 512.0)
        rhsd2 = np.zeros((DROWS, N2C), np.float32)
        rhsd2[0:3] = sh.T
        rhsd2[3:6] = -2.0 * sh.T
        rhsd2[6:9] = -2.0 * sl.T
        rhsd2[9] = 1.0
        rhsd2[10] = 1.0
        rhsd2[11] = 1.0
        rhsd2[12] = hi_s
        rhsd2[13] = lo_s + Ls
        rhsd2[14] = BIG * (1.0 - selmask)
        # (G,V) pair table, interleaved + replicated x4 on host:
        # gv4[p, 2j+0] = G_{p%32}(j), gv4[p, 2j+1] = V_{p%32}(j)
        G = D0 @ fts.astype(np.float64) + (P0 @ s.T.astype(np.float64))  # [32,N2C]
        V = (C0 - D0) @ fts.astype(np.float64)                           # [32,N2C]
        gvpair = np.empty((32, 2 * N2C), np.float64)
        gvpair[:, 0::2] = G
        gvpair[:, 1::2] = V
        gv4 = np.tile(gvpair, (4, 1)).astype(np.float32)                 # [128,2N2C]
        batch_sup.append((rhsd2, gv4))

    import ml_dtypes
    npdt = {F32: np.float32, F16: np.float16, BF16: ml_dtypes.bfloat16,
            I16: np.int16}
    in_maps = []
    for c in range(8):
        b = c // 4
        q0 = (c % 4) * NQ
        q = qx[b, q0:q0 + NQ]
        qmk = qm[b, q0:q0 + NQ].astype(np.float32)
        qh, ql = _split_hilo(q)
        q64, qh64 = q.astype(np.float64), qh.astype(np.float64)
        Lq = (np.sum(q64 * q64, 1) - np.sum(qh64 * qh64, 1)).astype(np.float32)
        qh2 = np.sum(qh64 * qh64, 1).astype(np.float32)
        hi_q, lo_q = _split_hilo(qh2, 512.0)
        lhsq = np.zeros((DROWS, NQ), np.float32)
        lhsq[0:3] = -2.0 * qh.T
        lhsq[3:6] = ql.T
        lhsq[6:9] = q.T
        lhsq[9] = hi_q
        lhsq[10] = lo_q + Lq
        lhsq[11] = BIG * (1 - qmk)
        lhsq[12] = 1.0
        lhsq[13] = 1.0
        lhsq[14] = 1.0

        # qdB[32g + u, i*32 + q'] = t0[u] - P0 @ q(i*128 + 32g + q')
        P0q = (P0 @ q.T.astype(np.float64)).reshape(32, NQT, 4, 32)
        qdB = np.zeros((128, 512), np.float64)
        for g in range(4):
            qdB[32 * g:32 * g + 32, :] = (
                t0[:, None] - P0q[:, :, g, :].reshape(32, NQT * 32))

        rhsd2, gv4 = batch_sup[b]
        im = dict(
            lhsq=lhsq, rhsd2=rhsd2, gv4=gv4, qdB=qdB,
            t1v=t1v, t2v=t2v,
            w1t4=w1t4, w2t=w2t,
            mh0=Mh[0], mh1=Mh[1], ident=ident,
            pow8=pow8, iotag=iotag, shv=shv, tpat=tpat,
            qfm=qmk.reshape(NQT, 128).T.copy(),
            onesk1=onesk1,
        )
        for k in im:
            shape, dt = IN_SPECS[k]
            arr = np.ascontiguousarray(im[k]).astype(npdt[dt])
            assert arr.shape == shape, (k, arr.shape, shape)
            im[k] = arr
        in_maps.append(im)
    return in_maps


def host_finish(results):
    out = np.zeros((2, 128, 8192), np.float32)
    for c in range(8):
        b = c // 4
        q0 = (c % 4) * NQ
        out[b, :, q0:q0 + NQ] = results[c]['out']
    return out


IN_SPECS = dict(
    lhsq=((DROWS, NQ), F16), rhsd2=((DROWS, N2C), F16),
    gv4=((128, 2 * N2C), BF16), qdB=((128, 512), F32),
    t1v=((32, 1), F32), t2v=((128, 1), F32),
    w1t4=((128, 32), BF16), w2t=((32, 128), BF16),
    mh0=((128, 128), F32), mh1=((128, 128), F32), ident=((128, 128), F32),
    pow8=((128, 1024), BF16), iotag=((128, GT * PAIRS), I16), shv=((128, 8), I16),
    tpat=((128, 544), F32), qfm=((128, NQT), F32), onesk1=((1, 128), F32),
)


# --------------------------------------------------------------------------
# device kernel
# --------------------------------------------------------------------------

@with_exitstack
def build_kernel(ctx: ExitStack, tc: tile.TileContext, out_ap: bass.AP, ins: dict):
    nc = tc.nc
    ctx.enter_context(nc.allow_low_precision("bf16 mlp + exact small-int sums"))

    consts = ctx.enter_context(tc.tile_pool(name="consts", bufs=1))
    gvp = ctx.enter_context(tc.tile_pool(name="gv", bufs=1))
    selp = ctx.enter_context(tc.tile_pool(name="sel", bufs=2))
    selp1 = ctx.enter_context(tc.tile_pool(name="sel1", bufs=1))
    selp2 = ctx.enter_context(tc.tile_pool(name="sel2", bufs=1))
    smallp = ctx.enter_context(tc.tile_pool(name="small", bufs=1))
    idxwp = ctx.enter_context(tc.tile_pool(name="idxw", bufs=2))
    mlpp = ctx.enter_context(tc.tile_pool(name="mlp", bufs=2))
    mlpp1 = ctx.enter_context(tc.tile_pool(name="mlp1", bufs=1))
    outp = ctx.enter_context(tc.tile_pool(name="outb", bufs=1))
    ps_d2 = ctx.enter_context(tc.tile_pool(name="psd2", bufs=2, space="PSUM"))
    ps_l2 = ctx.enter_context(tc.tile_pool(name="psl2", bufs=1, space="PSUM"))
    ps_l3 = ctx.enter_context(tc.tile_pool(name="psl3", bufs=1, space="PSUM"))

    ct = {}
    for name, (shape, dt) in IN_SPECS.items():
        t = consts.tile(list(shape), dt, tag=f"c_{name}")
        nc.sync.dma_start(out=t[:], in_=ins[name])
        ct[name] = t
    gv4 = ct['gv4']
    qdB = ct['qdB']

    c33 = consts.tile([128, 544], F32, tag="c33")
    nc.vector.memset(c33[:], 33.0)
    ones34 = consts.tile([128, 34], I16, tag="ones34")
    nc.vector.memset(ones34[:], 1)

    # persistent per-core state
    idxall = gvp.tile([128, 512], F32, tag="idxall")   # final idx per qtile (f32)
    ceffall = gvp.tile([128, NQT], F32, tag="ceffall")
    outbuf = outp.tile([128, NQ], F32, tag="outbuf")

    # ---- phase A for a whole group of GT query tiles: per-tile d2 matmuls +
    # mask words, then batched selection post-processing ----
    GP = GT * PAIRS   # 1152
    GS = GT * 544     # 2176

    def emit_group_A(g):
        w8g = selp1.tile([128, GT * WORDS], BF16, tag="w8g")
        for t in range(GT):
            i = g * GT + t
            for (off, csz) in CHUNKS:
                pd2 = ps_d2.tile([128, 1024], F32, tag="ps_d2")
                for n in range(csz // 512):
                    nc.tensor.matmul(
                        pd2[:, bass.ts(n, 512)],
                        ct['lhsq'][:, bass.ts(i, 128)],
                        ct['rhsd2'][:, off + 512 * n:off + 512 * (n + 1)],
                        start=True, stop=True)
                vw8c = selp.tile([128, 1024], BF16, tag="vw8c")
                nc.vector.scalar_tensor_tensor(
                    vw8c[:, 0:csz], pd2[:, 0:csz], R2, ct['pow8'][:, 0:csz],
                    op0=ALU.is_lt, op1=ALU.mult)
                nc.vector.tensor_reduce(
                    w8g[:, t * WORDS + off // 8:t * WORDS + (off + csz) // 8],
                    vw8c[:, 0:csz].rearrange("p (w t) -> p w t", t=8),
                    mybir.AxisListType.X, ALU.add)

        w8v = w8g[:].rearrange("p (c two) -> p c two", two=2)   # c = GP
        w8e = selp2.tile([128, GP], I16, tag="w8e")
        w8o = selp2.tile([128, GP], I16, tag="w8o")
        nc.scalar.activation(w8e[:], w8v[:, :, 0], ACTF.Copy)
        nc.scalar.activation(w8o[:], w8v[:, :, 1], ACTF.Copy)

        s16 = smallp.tile([128, GP], F32, tag="s16")
        nc.vector.tensor_tensor(s16[:], w8v[:, :, 0], w8v[:, :, 1], ALU.add)
        nz = smallp.tile([128, GP], F32, tag="nz")
        nc.scalar.activation(nz[:], s16[:], ACTF.Sign)
        crank = smallp.tile([128, GP], F32, tag="crank")
        for t in range(GT):
            nc.vector.tensor_tensor_scan(
                crank[:, t * PAIRS:(t + 1) * PAIRS],
                nz[:, t * PAIRS:(t + 1) * PAIRS], c33[:, 0:PAIRS], 0.0,
                ALU.add, ALU.min)
        u = smallp.tile([128, GP], F32, tag="s16")
        nc.vector.tensor_tensor(u[:], crank[:], nz[:], ALU.mult)
        v = smallp.tile([128, GP], F32, tag="nz")
        nc.vector.scalar_tensor_tensor(v[:], u[:], 32.5, u[:], op0=ALU.is_le,
                                       op1=ALU.mult)
        # si = v - 1 + 34t (selected) / -1 (unselected): gate the tile offset
        # by (v > 0) so spills stay negative and are ignored by the scatter
        vsel = smallp.tile([128, GP], F32, tag="crank")
        nc.vector.tensor_scalar(vsel[:], v[:], 0.0, None, ALU.is_gt)
        vo = smallp.tile([128, GP], F32, tag="s16")
        for t in range(GT):
            sl_ = slice(t * PAIRS, (t + 1) * PAIRS)
            nc.vector.scalar_tensor_tensor(vo[:, sl_], v[:, sl_], 34.0 * t,
                                           vsel[:, sl_], op0=ALU.add,
                                           op1=ALU.mult)
        si16 = selp2.tile([128, GP], I16, tag="si16")
        nc.vector.tensor_scalar(si16[:], vo[:], -1.0, None, ALU.add)

        dstID = selp2.tile([128, GT * 34], I16, tag="dstID")
        dstWe = selp2.tile([128, GT * 34], I16, tag="dstWe")
        dstWo = selp2.tile([128, GT * 34], I16, tag="dstWo")
        nc.gpsimd.local_scatter(dstID[:], ct['iotag'][:], si16[:], 128,
                                GT * 34, GP)
        nc.gpsimd.local_scatter(dstWe[:], w8e[:], si16[:], 128, GT * 34, GP)
        nc.gpsimd.local_scatter(dstWo[:], w8o[:], si16[:], 128, GT * 34, GP)

        esel16 = selp2.tile([128, GS], I16, tag="esel16")
        # per tile: esel col s*16 + b*8 + u  <-  bit u of dstW(b) col s
        evb = esel16[:].rearrange("p (T s b u) -> p b u T s", s=34, b=2, u=8)
        onesT34 = ones34[:].unsqueeze(1).broadcast_to((128, GT, 34))
        for bidx, dstWx in ((0, dstWe), (1, dstWo)):
            dwv = dstWx[:].rearrange("p (T s) -> p T s", s=34)
            for t in range(8):
                nc.vector.scalar_tensor_tensor(evb[:, bidx, t], dwv,
                                               ct['shv'][:, t:t + 1], onesT34,
                                               op0=ALU.logical_shift_right,
                                               op1=ALU.bitwise_and)
        esel = smallp.tile([128, GS], F32, tag="esel")
        nc.scalar.activation(esel[:], esel16[:], ACTF.Copy)
        idf = smallp.tile([128, GT * 34], F32, tag="idf")
        nc.scalar.activation(idf[:], dstID[:], ACTF.Copy)
        cjp1 = selp2.tile([128, GS], I16, tag="cjp1")
        nc.vector.scalar_tensor_tensor(
            cjp1[:].rearrange("p (T s u) -> p T s u", s=34, u=16),
            idf[:].rearrange("p (T s) -> p T s", s=34)
                .unsqueeze(3).broadcast_to((128, GT, 34, 16)), 16.0,
            ct['tpat'][:].rearrange("p (s u) -> p s u", u=16)
                .unsqueeze(1).broadcast_to((128, GT, 34, 16)),
            op0=ALU.mult, op1=ALU.add)

        crank2 = smallp.tile([128, GS], F32, tag="crank2")
        for t in range(GT):
            nc.vector.tensor_tensor_scan(
                crank2[:, t * 544:(t + 1) * 544],
                esel[:, t * 544:(t + 1) * 544], c33[:], 0.0, ALU.add, ALU.min)
        # effective count (with query-mask fallback to 32)
        cnt0 = smallp.tile([128, GT], F32, tag="cnt0")
        nc.vector.tensor_scalar(
            cnt0[:], crank2[:].rearrange("p (T x) -> p T x", x=544)[:, :, 543],
            32.0, None, ALU.min)
        qfc = smallp.tile([128, GT], F32, tag="qfc")
        nc.vector.tensor_scalar(qfc[:], ct['qfm'][:, g * GT:(g + 1) * GT],
                                -32.0, 32.0, ALU.mult, ALU.add)
        nc.vector.tensor_tensor(ceffall[:, g * GT:(g + 1) * GT], cnt0[:],
                                qfc[:], ALU.max)
        u2 = smallp.tile([128, GS], F32, tag="u2")
        nc.vector.tensor_tensor(u2[:], crank2[:], esel[:], ALU.mult)
        v2 = smallp.tile([128, GS], F32, tag="esel")
        nc.vector.scalar_tensor_tensor(v2[:], u2[:], 32.5, u2[:], op0=ALU.is_le,
                                       op1=ALU.mult)
        v2sel = smallp.tile([128, GS], F32, tag="crank2")
        nc.vector.tensor_scalar(v2sel[:], v2[:], 0.0, None, ALU.is_gt)
        vo2 = smallp.tile([128, GS], F32, tag="u2")
        for t in range(GT):
            sl_ = slice(t * 544, (t + 1) * 544)
            nc.vector.scalar_tensor_tensor(vo2[:, sl_], v2[:, sl_], 34.0 * t,
                                           v2sel[:, sl_], op0=ALU.add,
                                           op1=ALU.mult)
        si2 = selp2.tile([128, GS], I16, tag="si2")
        nc.vector.tensor_scalar(si2[:], vo2[:], -1.0, None, ALU.add)
        idxp1 = selp2.tile([128, GT * 34], I16, tag="idxp1")
        nc.gpsimd.local_scatter(idxp1[:], cjp1[:], si2[:], 128, GT * 34, GS)

        # fill + final gather indices (kept in f32 for the phase-B matmul)
        ii = smallp.tile([128, GT * 32], F32, tag="ii")
        nc.scalar.activation(
            ii[:].rearrange("p (T r) -> p T r", r=32),
            idxp1[:].rearrange("p (T s) -> p T s", s=34)[:, :, 0:32], ACTF.Copy)
        iv = ii[:].rearrange("p (T r) -> p T r", r=32)
        flp1 = smallp.tile([128, GT], F32, tag="flp1")
        nc.vector.tensor_scalar(flp1[:], iv[:, :, 0], 1.0, None, ALU.max)
        flb = flp1[:].unsqueeze(2).broadcast_to((128, GT, 32))
        m = smallp.tile([128, GT * 32], F32, tag="m")
        nc.vector.tensor_scalar(m[:], ii[:], 0.0, None, ALU.is_gt)
        bb = smallp.tile([128, GT * 32], F32, tag="bb")
        nc.vector.tensor_tensor(bb[:], ii[:], m[:], ALU.mult)
        aa = smallp.tile([128, GT * 32], F32, tag="aa")
        nc.vector.tensor_tensor(aa[:].rearrange("p (T r) -> p T r", r=32),
                                m[:].rearrange("p (T r) -> p T r", r=32), flb,
                                ALU.mult)
        cc = smallp.tile([128, GT * 32], F32, tag="m")
        nc.vector.tensor_tensor(cc[:], bb[:], aa[:], ALU.subtract)
        dd0 = smallp.tile([128, GT * 32], F32, tag="bb")
        nc.vector.tensor_tensor(dd0[:].rearrange("p (T r) -> p T r", r=32),
                                cc[:].rearrange("p (T r) -> p T r", r=32), flb,
                                ALU.add)
        nc.vector.tensor_scalar(idxall[:, g * GT * 32:(g + 1) * GT * 32],
                                dd0[:], -1.0, None, ALU.add)

        # wrapped gather index slots via two group-wide permutation matmuls
        psWg = ps_d2.tile([128, 256], F32, tag="ps_d2")
        nc.tensor.matmul(psWg[:, 0:128], ct['mh0'][:],
                         idxall[:, g * 128:(g + 1) * 128], start=True, stop=True)
        nc.tensor.matmul(psWg[:, 128:256], ct['mh1'][:],
                         idxall[:, g * 128:(g + 1) * 128], start=True, stop=True)
        idxwg = idxwp.tile([128, GT * 64], I16, tag="idxwg")
        ivw = idxwg[:].rearrange("p (T k) -> p T k", k=64)
        nc.scalar.activation(ivw[:, :, 0::2],
                             psWg[:, 0:128].rearrange("p (T r) -> p T r", r=32),
                             ACTF.Copy)
        nc.scalar.activation(ivw[:, :, 1::2],
                             psWg[:, 128:256].rearrange("p (T r) -> p T r", r=32),
                             ACTF.Copy)
        return idxwg

    # ---- phase B: gathered-MLP + pooling for query tile i ----
    def emit_mlp(i, gout):
        gv_g = gout.rearrange("p (r q u) -> p r q u", r=32, u=2)[:, :, :, 0]
        gv_v0 = gout.rearrange("p (k u) -> p k u", u=2)[:, 0:32, 1]

        # d = V(center) + qdelta; h1 = relu(G + d)
        dd = mlpp.tile([128, 32], F32, tag="dd")
        nc.vector.tensor_tensor(dd[:], gv_v0, qdB[:, bass.ts(i, 32)], ALU.add)
        h1t = mlpp1.tile([128, 1024], F32, tag="h1t")
        nc.vector.tensor_tensor(
            h1t[:].rearrange("p (r q) -> p r q", q=32), gv_g,
            dd[:].unsqueeze(1).broadcast_to((128, 32, 32)), ALU.add)
        h1 = mlpp.tile([128, 1024], BF16, tag="h1")
        nc.scalar.activation(h1[:], h1t[:], ACTF.Relu)

        # layer 2: per unit uu (K=32 at partition 32*uu)
        h2 = mlpp1.tile([32, 4096], BF16, tag="h2")
        for uu in range(4):
            psL2 = ps_l2.tile([32, 1024], F32, tag="ps_a")
            for n in range(2):
                nc.tensor.matmul(
                    psL2[:, bass.ts(n, 512)],
                    ct['w1t4'][32 * uu:32 * uu + 32, :],
                    h1[32 * uu:32 * uu + 32, bass.ts(n, 512)],
                    start=True, stop=True,
                    tile_position=(32 * uu, 0))
            nc.scalar.activation(h2[:, bass.ts(uu, 1024)], psL2[:], ACTF.Relu,
                                 bias=ct['t1v'][:])

        # layer 3
        h3 = mlpp1.tile([128, 4096], BF16, tag="h3")
        for n2 in range(4):
            psL3 = ps_l3.tile([128, 1024], F32, tag="ps_b3")
            for n in range(2):
                nc.tensor.matmul(psL3[:, bass.ts(n, 512)], ct['w2t'][:],
                                 h2[:, bass.ts(2 * n2 + n, 512)],
                                 start=True, stop=True)
            nc.scalar.activation(h3[:, bass.ts(n2, 1024)], psL3[:], ACTF.Relu,
                                 bias=ct['t2v'][:])

        # pooling
        S = smallp.tile([128, 128], F32, tag="S")
        h30 = smallp.tile([128, 128], F32, tag="h30")
        h3v = h3[:].rearrange("p (a r q) -> p a r q", a=4, r=32)
        for a in range(4):
            nc.vector.tensor_reduce(
                S[:, bass.ts(a, 32)], h3v[:, a, :, :].transpose([0, 2, 1]),
                mybir.AxisListType.X, ALU.add)
            nc.scalar.activation(h30[:, bass.ts(a, 32)], h3v[:, a, 0, :], ACTF.Copy)

        # beta/gamma rows via PE transpose + broadcast
        ceff = ceffall[:, i:i + 1]
        beta = smallp.tile([128, 1], F32, tag="beta")
        nc.vector.reciprocal(beta[:], ceff)
        gm0 = smallp.tile([128, 1], F32, tag="gm0")
        nc.vector.tensor_scalar(gm0[:], ceff, -1.0, 32.0, ALU.mult, ALU.add)
        gamma = smallp.tile([128, 1], F32, tag="gamma")
        nc.vector.tensor_tensor(gamma[:], gm0[:], beta[:], ALU.mult)
        psBG = ps_d2.tile([1, 256], F32, tag="ps_d2")
        nc.tensor.matmul(psBG[:, 0:128], beta[:], ct['ident'][:],
                         start=True, stop=True)
        nc.tensor.matmul(psBG[:, 128:256], gamma[:], ct['ident'][:],
                         start=True, stop=True)
        bgrow = smallp.tile([1, 256], F32, tag="bgrow")
        nc.vector.tensor_copy(bgrow[:], psBG[:])
        psB = ps_d2.tile([128, 256], F32, tag="ps_d2")
        nc.tensor.matmul(psB[:], ct['onesk1'][:], bgrow[:], start=True, stop=True)

        e1 = smallp.tile([128, 128], F32, tag="e1")
        nc.vector.tensor_tensor(e1[:], S[:], psB[:, 0:128], ALU.mult)
        e2 = smallp.tile([128, 128], F32, tag="e2")
        nc.vector.tensor_tensor(e2[:], h30[:], psB[:, 128:256], ALU.mult)
        nc.vector.tensor_tensor(outbuf[:, bass.ts(i, 128)], e1[:], e2[:],
                                ALU.subtract)

    # ==== software-pipelined groups: gather(g) | mlp(g) | select(g+1) ====
    idxwg = emit_group_A(0)
    for g in range(NGRP):
        goutg = mlpp.tile([128, GT * 2048], BF16, tag="goutg")
        nc.gpsimd.ap_gather(goutg[:].rearrange("p (k u) -> p k u", u=2),
                            gv4[:].rearrange("p (j u) -> p j u", u=2),
                            idxwg[:], 128, N2C, 2, GT * 1024)
        for t in range(GT):
            emit_mlp(g * GT + t, goutg[:, t * 2048:(t + 1) * 2048])
        if g + 1 < NGRP:
            idxwg = emit_group_A(g + 1)
        nc.sync.dma_start(out=out_ap[:, g * GT * 128:(g + 1) * GT * 128],
                          in_=outbuf[:, g * GT * 128:(g + 1) * GT * 128])


# ==========================================================================
# harness entry point: kernel(**inputs) -> full output [2, 128, 8192]
# ==========================================================================

_CACHE = {}


def _build_nc():
    import concourse.bacc as bacc
    import concourse.tile as tile_mod
    nc = bacc.Bacc("TRN2", target_bir_lowering=False, debug=False, num_devices=8)
    in_tiles = {}
    for name, (shape, dt) in IN_SPECS.items():
        in_tiles[name] = nc.dram_tensor(
            name, list(shape), dt, kind="ExternalInput").ap()
    out_tile = nc.dram_tensor("out", (128, NQ), F32, kind="ExternalOutput").ap()
    with tile_mod.TileContext(nc) as t:
        build_kernel(t, out_tile, in_tiles)
    nc.compile()
    return nc


def kernel(**inputs):
    from concourse.bass_utils import run_bass_kernel_spmd
    in_maps = host_prep(inputs)
    if "nc" not in _CACHE:
        _CACHE["nc"] = _build_nc()
    res = run_bass_kernel_spmd(_CACHE["nc"], in_maps, list(range(8)))
    return host_finish(res.results)


# revision 32
# speedup vs baseline: 1.0529x; 1.0529x over previous
"""Trainium2 Bass kernel for nn_PointWiseMLP (ball query + gather + MLP + pool).

Self-contained: kernel(**inputs) shards across 8 NeuronCores (data-parallel
over batch x query-range), runs the Bass/Tile kernel via run_bass_kernel_spmd,
and gathers the full [2, 128, 8192] output.

v2: support points compacted by support_mask on host (order-preserving, so
"first 32 by index" is unchanged) and padded to N2C=4608; gathers batched in
groups of 4 query tiles to amortize the gpsimd table-walk cost; gv4 built with
8 parallel DMAs; software-pipelined group order (selection of group g+1 is
emitted before the MLP of group g).
"""
import sys
for _p in ("/opt/trn_rl_repo", "/root/.axon_site/_ro/trn_rl_repo"):
    if _p not in sys.path:
        sys.path.append(_p)


import numpy as np
from contextlib import ExitStack

import concourse.bass as bass
import concourse.tile as tile
from concourse import mybir
from concourse._compat import with_exitstack

F32 = mybir.dt.float32
F16 = mybir.dt.float16
BF16 = mybir.dt.bfloat16
I16 = mybir.dt.int16

RADIUS = 0.1
NSAMPLE = 32
EPS = 1e-5
N2C = 4608         # compacted+padded support count
PAIRS = N2C // 16  # 288
WORDS = N2C // 8   # 576
NQ = 2048          # queries per core
NQT = 16           # query tiles per core
GT = 4             # query tiles per gather group
NGRP = NQT // GT   # 4
BIG = 1024.0   # exactly representable in fp16
R2 = float(np.float32(0.01))  # threshold as f32
DROWS = 15     # fp16 d2 decomposition rows
CHUNKS = [(0, 1024), (1024, 1024), (2048, 1024), (3072, 1024), (4096, 512)]

ALU = mybir.AluOpType
ACTF = mybir.ActivationFunctionType


# --------------------------------------------------------------------------
# host-side preparation
# --------------------------------------------------------------------------

def _split_hilo(x, grid=1024.0):
    """Grid split: x = hi + lo with hi on 1/grid grid (exact in fp16 for the
    value ranges used here)."""
    x = x.astype(np.float32)
    hi = np.floor(x.astype(np.float64) * grid) / grid
    hi = hi.astype(np.float32)
    lo = (x - hi).astype(np.float32)
    return hi, lo


def host_prep(inputs):
    B = 2
    qx = np.asarray(inputs['query_xyz'], np.float32)
    sx = np.asarray(inputs['support_xyz'], np.float32)
    qm = np.asarray(inputs['query_mask'], np.int32)
    sm = np.asarray(inputs['support_mask'], np.int32)
    sf = np.asarray(inputs['support_features'], np.float32)

    W0 = np.asarray(inputs['W0'], np.float64)
    W1 = np.asarray(inputs['W1'], np.float64)
    W2 = np.asarray(inputs['W2'], np.float64)

    def fold(Wl, g, b, rm, rv):
        s = np.asarray(g, np.float64) / np.sqrt(np.asarray(rv, np.float64) + EPS)
        return Wl * s[:, None], np.asarray(b, np.float64) - np.asarray(rm, np.float64) * s

    W0p, t0 = fold(W0, inputs['g0'], inputs['b0'], inputs['rm0'], inputs['rv0'])
    W1p, t1 = fold(W1, inputs['g1'], inputs['b1'], inputs['rm1'], inputs['rv1'])
    W2p, t2 = fold(W2, inputs['g2'], inputs['b2'], inputs['rm2'], inputs['rv2'])

    P0 = W0p[:, 0:3] / RADIUS
    C0 = W0p[:, 3:67]
    D0 = W0p[:, 67:131]

    w1t4 = np.tile(W1p.T.astype(np.float32), (4, 1))       # [128, 32]
    w2t = W2p.T.astype(np.float32)                         # [32, 128]

    t1v = t1.astype(np.float32).reshape(32, 1)
    t2v = t2.astype(np.float32).reshape(128, 1)

    # permutation matmul weights for the wrapped gather index layout:
    # idxw[p, 2r+h] = idxg[32*(p//32) + 16h + p%16, r]
    Mh = np.zeros((2, 128, 128), np.float32)
    for h in range(2):
        for p in range(128):
            Mh[h, 32 * (p // 32) + 16 * h + p % 16, p] = 1.0
    ident = np.eye(128, dtype=np.float32)

    pow8 = np.tile((2.0 ** (np.arange(1024) % 8)).astype(np.float32)[None, :], (128, 1))
    # scatter id source, pre-scaled by 16 so cjp1 = dstID + tpat16 directly
    iotag = np.tile((((np.arange(GT * PAIRS, dtype=np.int16) % PAIRS) + 1) * 16)[None, :],
                    (128, 1))
    shv = np.tile(np.arange(8, dtype=np.int16)[None, :], (128, 1))
    tpat16 = np.tile((np.tile(np.arange(16, dtype=np.int16), 34) - 15)[None, :],
                     (128, 1))
    onesk1 = np.ones((1, 128), np.float32)

    batch_sup = []
    for b in range(B):
        # order-preserving compaction by support_mask; original point 0 is
        # always table entry 0 (selection-masked if its mask is 0) so the
        # zero-neighbor fill gathers the same point the reference does.
        valid = sm[b] > 0
        keep = np.nonzero(valid)[0]
        sel0 = True
        if not valid[0]:
            keep = np.concatenate([[0], keep])
            sel0 = False
        nv = len(keep)
        assert nv <= N2C, (nv, N2C)
        s = np.zeros((N2C, 3), np.float32)
        s[:nv] = sx[b][keep]
        fts = np.zeros((64, N2C), np.float32)
        fts[:, :nv] = sf[b][:, keep]
        selmask = np.zeros(N2C, np.float32)
        selmask[:nv] = 1.0
        if not sel0:
            selmask[0] = 0.0

        # fp16 d2 decomposition: all "hi" rows exact in fp16; residual rows
        # are O(2^-7) so fp16 rounding perturbs d2 by only ~1e-6.
        sh, sl = _split_hilo(s)
        s64, sh64 = s.astype(np.float64), sh.astype(np.float64)
        Ls = (np.sum(s64 * s64, 1) - np.sum(sh64 * sh64, 1)).astype(np.float32)
        sh2 = np.sum(sh64 * sh64, 1).astype(np.float32)
        hi_s, lo_s = _split_hilo(sh2, 512.0)
        rhsd2 = np.zeros((DROWS, N2C), np.float32)
        rhsd2[0:3] = sh.T
        rhsd2[3:6] = -2.0 * sh.T
        rhsd2[6:9] = -2.0 * sl.T
        rhsd2[9] = 1.0
        rhsd2[10] = 1.0
        rhsd2[11] = 1.0
        rhsd2[12] = hi_s
        rhsd2[13] = lo_s + Ls
        rhsd2[14] = BIG * (1.0 - selmask)
        # (G,V) pair table, interleaved + replicated x4 on host:
        # gv4[p, 2j+0] = G_{p%32}(j), gv4[p, 2j+1] = V_{p%32}(j)
        G = D0 @ fts.astype(np.float64) + (P0 @ s.T.astype(np.float64))  # [32,N2C]
        V = (C0 - D0) @ fts.astype(np.float64)                           # [32,N2C]
        gvpair = np.empty((32, 2 * N2C), np.float64)
        gvpair[:, 0::2] = G
        gvpair[:, 1::2] = V
        gv4 = np.tile(gvpair, (4, 1)).astype(np.float32)                 # [128,2N2C]
        batch_sup.append((rhsd2, gv4))

    import ml_dtypes
    npdt = {F32: np.float32, F16: np.float16, BF16: ml_dtypes.bfloat16,
            I16: np.int16}
    in_maps = []
    for c in range(8):
        b = c // 4
        q0 = (c % 4) * NQ
        q = qx[b, q0:q0 + NQ]
        qmk = qm[b, q0:q0 + NQ].astype(np.float32)
        qh, ql = _split_hilo(q)
        q64, qh64 = q.astype(np.float64), qh.astype(np.float64)
        Lq = (np.sum(q64 * q64, 1) - np.sum(qh64 * qh64, 1)).astype(np.float32)
        qh2 = np.sum(qh64 * qh64, 1).astype(np.float32)
        hi_q, lo_q = _split_hilo(qh2, 512.0)
        lhsq = np.zeros((DROWS, NQ), np.float32)
        lhsq[0:3] = -2.0 * qh.T
        lhsq[3:6] = ql.T
        lhsq[6:9] = q.T
        lhsq[9] = hi_q
        lhsq[10] = lo_q + Lq
        lhsq[11] = BIG * (1 - qmk)
        lhsq[12] = 1.0
        lhsq[13] = 1.0
        lhsq[14] = 1.0

        # qdB[32g + u, i*32 + q'] = t0[u] - P0 @ q(i*128 + 32g + q')
        P0q = (P0 @ q.T.astype(np.float64)).reshape(32, NQT, 4, 32)
        qdB = np.zeros((128, 512), np.float64)
        for g in range(4):
            qdB[32 * g:32 * g + 32, :] = (
                t0[:, None] - P0q[:, :, g, :].reshape(32, NQT * 32))

        rhsd2, gv4 = batch_sup[b]
        im = dict(
            lhsq=lhsq, rhsd2=rhsd2, gv4=gv4, qdB=qdB,
            t1v=t1v, t2v=t2v,
            w1t4=w1t4, w2t=w2t,
            mh0=Mh[0], mh1=Mh[1], ident=ident,
            pow8=pow8, iotag=iotag, shv=shv, tpat16=tpat16,
            qfm=qmk.reshape(NQT, 128).T.copy(),
            onesk1=onesk1,
        )
        for k in im:
            shape, dt = IN_SPECS[k]
            arr = np.ascontiguousarray(im[k]).astype(npdt[dt])
            assert arr.shape == shape, (k, arr.shape, shape)
            im[k] = arr
        in_maps.append(im)
    return in_maps


def host_finish(results):
    out = np.zeros((2, 128, 8192), np.float32)
    for c in range(8):
        b = c // 4
        q0 = (c % 4) * NQ
        out[b, :, q0:q0 + NQ] = results[c]['out']
    return out


IN_SPECS = dict(
    lhsq=((DROWS, NQ), F16), rhsd2=((DROWS, N2C), F16),
    gv4=((128, 2 * N2C), BF16), qdB=((128, 512), F32),
    t1v=((32, 1), F32), t2v=((128, 1), F32),
    w1t4=((128, 32), BF16), w2t=((32, 128), BF16),
    mh0=((128, 128), F32), mh1=((128, 128), F32), ident=((128, 128), F32),
    pow8=((128, 1024), BF16), iotag=((128, GT * PAIRS), I16), shv=((128, 8), I16),
    tpat16=((128, 544), I16), qfm=((128, NQT), F32), onesk1=((1, 128), F32),
)


# --------------------------------------------------------------------------
# device kernel
# --------------------------------------------------------------------------

@with_exitstack
def build_kernel(ctx: ExitStack, tc: tile.TileContext, out_ap: bass.AP, ins: dict):
    nc = tc.nc
    ctx.enter_context(nc.allow_low_precision("bf16 mlp + exact small-int sums"))

    consts = ctx.enter_context(tc.tile_pool(name="consts", bufs=1))
    gvp = ctx.enter_context(tc.tile_pool(name="gv", bufs=1))
    selp = ctx.enter_context(tc.tile_pool(name="sel", bufs=2))
    selp1 = ctx.enter_context(tc.tile_pool(name="sel1", bufs=1))
    selp2 = ctx.enter_context(tc.tile_pool(name="sel2", bufs=1))
    smallp = ctx.enter_context(tc.tile_pool(name="small", bufs=1))
    idxwp = ctx.enter_context(tc.tile_pool(name="idxw", bufs=2))
    mlpp = ctx.enter_context(tc.tile_pool(name="mlp", bufs=2))
    mlpp1 = ctx.enter_context(tc.tile_pool(name="mlp1", bufs=1))
    outp = ctx.enter_context(tc.tile_pool(name="outb", bufs=1))
    ps_d2 = ctx.enter_context(tc.tile_pool(name="psd2", bufs=2, space="PSUM"))
    ps_l2 = ctx.enter_context(tc.tile_pool(name="psl2", bufs=1, space="PSUM"))
    ps_l3 = ctx.enter_context(tc.tile_pool(name="psl3", bufs=1, space="PSUM"))

    ct = {}
    for name, (shape, dt) in IN_SPECS.items():
        t = consts.tile(list(shape), dt, tag=f"c_{name}")
        nc.sync.dma_start(out=t[:], in_=ins[name])
        ct[name] = t
    gv4 = ct['gv4']
    qdB = ct['qdB']

    c33 = consts.tile([128, 544], BF16, tag="c33")
    nc.vector.memset(c33[:], 33.0)
    ones34 = consts.tile([128, 34], I16, tag="ones34")
    nc.vector.memset(ones34[:], 1)

    # persistent per-core state
    idxall = gvp.tile([128, 512], F32, tag="idxall")   # final idx per qtile (f32)
    ceffall = gvp.tile([128, NQT], F32, tag="ceffall")
    outbuf = outp.tile([128, NQ], F32, tag="outbuf")

    # ---- phase A for a whole group of GT query tiles: per-tile d2 matmuls +
    # mask words, then batched selection post-processing ----
    GP = GT * PAIRS   # 1152
    GS = GT * 544     # 2176

    def emit_group_A(g):
        w8g = selp1.tile([128, GT * WORDS], BF16, tag="w8g")
        for t in range(GT):
            i = g * GT + t
            for (off, csz) in CHUNKS:
                pd2 = ps_d2.tile([128, 1024], F32, tag="ps_d2")
                for n in range(csz // 512):
                    nc.tensor.matmul(
                        pd2[:, bass.ts(n, 512)],
                        ct['lhsq'][:, bass.ts(i, 128)],
                        ct['rhsd2'][:, off + 512 * n:off + 512 * (n + 1)],
                        start=True, stop=True)
                vw8c = selp.tile([128, 1024], BF16, tag="vw8c")
                nc.vector.scalar_tensor_tensor(
                    vw8c[:, 0:csz], pd2[:, 0:csz], R2, ct['pow8'][:, 0:csz],
                    op0=ALU.is_lt, op1=ALU.mult)
                nc.vector.tensor_reduce(
                    w8g[:, t * WORDS + off // 8:t * WORDS + (off + csz) // 8],
                    vw8c[:, 0:csz].rearrange("p (w t) -> p w t", t=8),
                    mybir.AxisListType.X, ALU.add)

        w8v = w8g[:].rearrange("p (c two) -> p c two", two=2)   # c = GP
        # scatter sources converted on the Pool engine (same queue as the
        # scatters that consume them -- no cross-engine hop, Pool is idle)
        w8e = selp2.tile([128, GP], I16, tag="w8e")
        w8o = selp2.tile([128, GP], I16, tag="w8o")
        nc.gpsimd.tensor_scalar(w8e[:], w8v[:, :, 0], 0.0, None, ALU.add)
        nc.gpsimd.tensor_scalar(w8o[:], w8v[:, :, 1], 0.0, None, ALU.add)

        nz = smallp.tile([128, GP], BF16, tag="nz")
        nc.vector.tensor_tensor(nz[:], w8v[:, :, 0], w8v[:, :, 1],
                                ALU.logical_or)
        crank = smallp.tile([128, GP], BF16, tag="crank")
        for t in range(GT):
            nc.vector.tensor_tensor_scan(
                crank[:, t * PAIRS:(t + 1) * PAIRS],
                nz[:, t * PAIRS:(t + 1) * PAIRS], c33[:, 0:PAIRS], 0.0,
                ALU.add, ALU.min)
        u = smallp.tile([128, GP], BF16, tag="u")
        nc.vector.tensor_tensor(u[:], crank[:], nz[:], ALU.mult)
        v = smallp.tile([128, GP], BF16, tag="nz")
        nc.vector.scalar_tensor_tensor(v[:], u[:], 32.5, u[:], op0=ALU.is_le,
                                       op1=ALU.mult)
        si16 = selp2.tile([128, GP], I16, tag="si16")
        nc.vector.tensor_scalar(si16[:], v[:], -1.0, None, ALU.add)

        dstID = selp2.tile([128, GT * 34], I16, tag="dstID")
        dstWe = selp2.tile([128, GT * 34], I16, tag="dstWe")
        dstWo = selp2.tile([128, GT * 34], I16, tag="dstWo")
        for t in range(GT):
            sl_ = slice(t * PAIRS, (t + 1) * PAIRS)
            ds_ = slice(t * 34, (t + 1) * 34)
            nc.gpsimd.local_scatter(dstID[:, ds_], ct['iotag'][:, sl_],
                                    si16[:, sl_], 128, 34, PAIRS)
            nc.gpsimd.local_scatter(dstWe[:, ds_], w8e[:, sl_], si16[:, sl_],
                                    128, 34, PAIRS)
            nc.gpsimd.local_scatter(dstWo[:, ds_], w8o[:, sl_], si16[:, sl_],
                                    128, 34, PAIRS)

        esel16 = selp2.tile([128, GS], I16, tag="esel16")
        # per tile: esel col s*16 + b*8 + u  <-  bit u of dstW(b) col s
        evb = esel16[:].rearrange("p (T s b u) -> p b u T s", s=34, b=2, u=8)
        onesT34 = ones34[:].unsqueeze(1).broadcast_to((128, GT, 34))
        for bidx, dstWx in ((0, dstWe), (1, dstWo)):
            dwv = dstWx[:].rearrange("p (T s) -> p T s", s=34)
            for t in range(8):
                nc.vector.scalar_tensor_tensor(evb[:, bidx, t], dwv,
                                               ct['shv'][:, t:t + 1], onesT34,
                                               op0=ALU.logical_shift_right,
                                               op1=ALU.bitwise_and)
        esel = smallp.tile([128, GS], BF16, tag="esel")
        nc.gpsimd.tensor_scalar(esel[:], esel16[:], 0.0, None, ALU.add)
        cjp1 = selp2.tile([128, GS], I16, tag="cjp1")
        nc.vector.tensor_tensor(
            cjp1[:].rearrange("p (T s u) -> p T s u", s=34, u=16),
            dstID[:].rearrange("p (T s) -> p T s", s=34)
                .unsqueeze(3).broadcast_to((128, GT, 34, 16)),
            ct['tpat16'][:].rearrange("p (s u) -> p s u", u=16)
                .unsqueeze(1).broadcast_to((128, GT, 34, 16)),
            ALU.add)

        crank2 = smallp.tile([128, GS], BF16, tag="crank2")
        for t in range(GT):
            nc.vector.tensor_tensor_scan(
                crank2[:, t * 544:(t + 1) * 544],
                esel[:, t * 544:(t + 1) * 544], c33[:], 0.0, ALU.add, ALU.min)
        # effective count (with query-mask fallback to 32)
        cnt0 = smallp.tile([128, GT], F32, tag="cnt0")
        nc.vector.tensor_scalar(
            cnt0[:], crank2[:].rearrange("p (T x) -> p T x", x=544)[:, :, 543],
            32.0, None, ALU.min)
        qfc = smallp.tile([128, GT], F32, tag="qfc")
        nc.vector.tensor_scalar(qfc[:], ct['qfm'][:, g * GT:(g + 1) * GT],
                                -32.0, 32.0, ALU.mult, ALU.add)
        nc.vector.tensor_tensor(ceffall[:, g * GT:(g + 1) * GT], cnt0[:],
                                qfc[:], ALU.max)
        u2 = smallp.tile([128, GS], BF16, tag="u2")
        nc.vector.tensor_tensor(u2[:], crank2[:], esel[:], ALU.mult)
        v2 = smallp.tile([128, GS], BF16, tag="esel")
        nc.vector.scalar_tensor_tensor(v2[:], u2[:], 32.5, u2[:], op0=ALU.is_le,
                                       op1=ALU.mult)
        si2 = selp2.tile([128, GS], I16, tag="si2")
        nc.vector.tensor_scalar(si2[:], v2[:], -1.0, None, ALU.add)
        idxp1 = selp2.tile([128, GT * 34], I16, tag="idxp1")
        for t in range(GT):
            nc.gpsimd.local_scatter(idxp1[:, t * 34:(t + 1) * 34],
                                    cjp1[:, t * 544:(t + 1) * 544],
                                    si2[:, t * 544:(t + 1) * 544], 128, 34, 544)

        # fill + final gather indices (kept in f32 for the phase-B matmul)
        ii = smallp.tile([128, GT * 32], F32, tag="ii")
        nc.gpsimd.tensor_scalar(
            ii[:].rearrange("p (T r) -> p T r", r=32),
            idxp1[:].rearrange("p (T s) -> p T s", s=34)[:, :, 0:32],
            0.0, None, ALU.add)
        iv = ii[:].rearrange("p (T r) -> p T r", r=32)
        flp1 = smallp.tile([128, GT], F32, tag="flp1")
        nc.vector.tensor_scalar(flp1[:], iv[:, :, 0], 1.0, None, ALU.max)
        flb = flp1[:].unsqueeze(2).broadcast_to((128, GT, 32))
        m = smallp.tile([128, GT * 32], F32, tag="m")
        nc.vector.tensor_scalar(m[:], ii[:], 0.0, None, ALU.is_gt)
        bb = smallp.tile([128, GT * 32], F32, tag="bb")
        nc.vector.tensor_tensor(bb[:], ii[:], m[:], ALU.mult)
        aa = smallp.tile([128, GT * 32], F32, tag="aa")
        nc.vector.tensor_tensor(aa[:].rearrange("p (T r) -> p T r", r=32),
                                m[:].rearrange("p (T r) -> p T r", r=32), flb,
                                ALU.mult)
        cc = smallp.tile([128, GT * 32], F32, tag="m")
        nc.vector.tensor_tensor(cc[:], bb[:], aa[:], ALU.subtract)
        dd0 = smallp.tile([128, GT * 32], F32, tag="bb")
        nc.vector.tensor_tensor(dd0[:].rearrange("p (T r) -> p T r", r=32),
                                cc[:].rearrange("p (T r) -> p T r", r=32), flb,
                                ALU.add)
        nc.vector.tensor_scalar(idxall[:, g * GT * 32:(g + 1) * GT * 32],
                                dd0[:], -1.0, None, ALU.add)

        # wrapped gather index slots via two group-wide permutation matmuls
        psWg = ps_d2.tile([128, 256], F32, tag="ps_d2")
        nc.tensor.matmul(psWg[:, 0:128], ct['mh0'][:],
                         idxall[:, g * 128:(g + 1) * 128], start=True, stop=True)
        nc.tensor.matmul(psWg[:, 128:256], ct['mh1'][:],
                         idxall[:, g * 128:(g + 1) * 128], start=True, stop=True)
        idxwg = idxwp.tile([128, GT * 64], I16, tag="idxwg")
        ivw = idxwg[:].rearrange("p (T k) -> p T k", k=64)
        nc.scalar.activation(ivw[:, :, 0::2],
                             psWg[:, 0:128].rearrange("p (T r) -> p T r", r=32),
                             ACTF.Copy)
        nc.scalar.activation(ivw[:, :, 1::2],
                             psWg[:, 128:256].rearrange("p (T r) -> p T r", r=32),
                             ACTF.Copy)
        return idxwg

    # ---- phase B: gathered-MLP + pooling for query tile i ----
    def emit_mlp(i, gout):
        gv_g = gout.rearrange("p (r q u) -> p r q u", r=32, u=2)[:, :, :, 0]
        gv_v0 = gout.rearrange("p (k u) -> p k u", u=2)[:, 0:32, 1]

        # d = V(center) + qdelta; h1 = relu(G + d)
        dd = mlpp.tile([128, 32], F32, tag="dd")
        nc.vector.tensor_tensor(dd[:], gv_v0, qdB[:, bass.ts(i, 32)], ALU.add)
        h1t = mlpp1.tile([128, 1024], F32, tag="h1t")
        nc.vector.tensor_tensor(
            h1t[:].rearrange("p (r q) -> p r q", q=32), gv_g,
            dd[:].unsqueeze(1).broadcast_to((128, 32, 32)), ALU.add)
        h1 = mlpp.tile([128, 1024], BF16, tag="h1")
        nc.scalar.activation(h1[:], h1t[:], ACTF.Relu)

        # layer 2: per unit uu (K=32 at partition 32*uu)
        h2 = mlpp1.tile([32, 4096], BF16, tag="h2")
        for uu in range(4):
            psL2 = ps_l2.tile([32, 1024], F32, tag="ps_a")
            for n in range(2):
                nc.tensor.matmul(
                    psL2[:, bass.ts(n, 512)],
                    ct['w1t4'][32 * uu:32 * uu + 32, :],
                    h1[32 * uu:32 * uu + 32, bass.ts(n, 512)],
                    start=True, stop=True,
                    tile_position=(32 * uu, 0))
            nc.scalar.activation(h2[:, bass.ts(uu, 1024)], psL2[:], ACTF.Relu,
                                 bias=ct['t1v'][:])

        # layer 3
        h3 = mlpp1.tile([128, 4096], BF16, tag="h3")
        for n2 in range(4):
            psL3 = ps_l3.tile([128, 1024], F32, tag="ps_b3")
            for n in range(2):
                nc.tensor.matmul(psL3[:, bass.ts(n, 512)], ct['w2t'][:],
                                 h2[:, bass.ts(2 * n2 + n, 512)],
                                 start=True, stop=True)
            nc.scalar.activation(h3[:, bass.ts(n2, 1024)], psL3[:], ACTF.Relu,
                                 bias=ct['t2v'][:])

        # pooling
        S = smallp.tile([128, 128], F32, tag="S")
        h30 = smallp.tile([128, 128], F32, tag="h30")
        h3v = h3[:].rearrange("p (a r q) -> p a r q", a=4, r=32)
        for a in range(4):
            nc.vector.tensor_reduce(
                S[:, bass.ts(a, 32)], h3v[:, a, :, :].transpose([0, 2, 1]),
                mybir.AxisListType.X, ALU.add)
            nc.scalar.activation(h30[:, bass.ts(a, 32)], h3v[:, a, 0, :], ACTF.Copy)

        # beta/gamma rows via PE transpose + broadcast
        ceff = ceffall[:, i:i + 1]
        beta = smallp.tile([128, 1], F32, tag="beta")
        nc.vector.reciprocal(beta[:], ceff)
        gm0 = smallp.tile([128, 1], F32, tag="gm0")
        nc.vector.tensor_scalar(gm0[:], ceff, -1.0, 32.0, ALU.mult, ALU.add)
        gamma = smallp.tile([128, 1], F32, tag="gamma")
        nc.vector.tensor_tensor(gamma[:], gm0[:], beta[:], ALU.mult)
        psBG = ps_d2.tile([1, 256], F32, tag="ps_d2")
        nc.tensor.matmul(psBG[:, 0:128], beta[:], ct['ident'][:],
                         start=True, stop=True)
        nc.tensor.matmul(psBG[:, 128:256], gamma[:], ct['ident'][:],
                         start=True, stop=True)
        bgrow = smallp.tile([1, 256], F32, tag="bgrow")
        nc.vector.tensor_copy(bgrow[:], psBG[:])
        psB = ps_d2.tile([128, 256], F32, tag="ps_d2")
        nc.tensor.matmul(psB[:], ct['onesk1'][:], bgrow[:], start=True, stop=True)

        e1 = smallp.tile([128, 128], F32, tag="e1")
        nc.vector.tensor_tensor(e1[:], S[:], psB[:, 0:128], ALU.mult)
        e2 = smallp.tile([128, 128], F32, tag="e2")
        nc.vector.tensor_tensor(e2[:], h30[:], psB[:, 128:256], ALU.mult)
        nc.vector.tensor_tensor(outbuf[:, bass.ts(i, 128)], e1[:], e2[:],
                                ALU.subtract)

    # ==== software-pipelined groups: gather(g) | mlp(g) | select(g+1) ====
    idxwg = emit_group_A(0)
    for g in range(NGRP):
        goutg = mlpp.tile([128, GT * 2048], BF16, tag="goutg")
        nc.gpsimd.ap_gather(goutg[:].rearrange("p (k u) -> p k u", u=2),
                            gv4[:].rearrange("p (j u) -> p j u", u=2),
                            idxwg[:], 128, N2C, 2, GT * 1024)
        for t in range(GT):
            emit_mlp(g * GT + t, goutg[:, t * 2048:(t + 1) * 2048])
        if g + 1 < NGRP:
            idxwg = emit_group_A(g + 1)
        nc.sync.dma_start(out=out_ap[:, g * GT * 128:(g + 1) * GT * 128],
                          in_=outbuf[:, g * GT * 128:(g + 1) * GT * 128])


# ==========================================================================
# harness entry point: kernel(**inputs) -> full output [2, 128, 8192]
# ==========================================================================

_CACHE = {}


def _build_nc():
    import concourse.bacc as bacc
    import concourse.tile as tile_mod
    nc = bacc.Bacc("TRN2", target_bir_lowering=False, debug=False, num_devices=8)
    in_tiles = {}
    for name, (shape, dt) in IN_SPECS.items():
        in_tiles[name] = nc.dram_tensor(
            name, list(shape), dt, kind="ExternalInput").ap()
    out_tile = nc.dram_tensor("out", (128, NQ), F32, kind="ExternalOutput").ap()
    with tile_mod.TileContext(nc) as t:
        build_kernel(t, out_tile, in_tiles)
    nc.compile()
    return nc


def kernel(**inputs):
    from concourse.bass_utils import run_bass_kernel_spmd
    in_maps = host_prep(inputs)
    if "nc" not in _CACHE:
        _CACHE["nc"] = _build_nc()
    res = run_bass_kernel_spmd(_CACHE["nc"], in_maps, list(range(8)))
    return host_finish(res.results)


# revision 37
# speedup vs baseline: 1.3077x; 1.2420x over previous
"""Trainium2 Bass kernel for nn_PointWiseMLP (ball query + gather + MLP + pool).

Self-contained: kernel(**inputs) shards across 8 NeuronCores (data-parallel
over batch x query-range), runs the Bass/Tile kernel via run_bass_kernel_spmd,
and gathers the full [2, 128, 8192] output.

v3: - support compacted by support_mask on host (order-preserving) and
      k-d-tree query reordering so each 128-query tile is spatially compact;
      each tile only tests the support points inside its radius-expanded bbox
      (<= SUB=1536 of 8192), cutting ball-query work ~5x.
    - per-tile (G,V) pair tables are built on host in tile-local index space
      and DMA'd per group, so the selection pipeline's local indices feed the
      gather directly (no index translation anywhere).
    - d2 decomposed into 15 fp16 rows (4x faster on PE than f32, ~1e-6 exact).
    - selection post-processing batched per 4-tile group, scan values in bf16,
      scatter-source conversions on the (idle) gpsimd engine.
"""
import sys
for _p in ("/opt/trn_rl_repo", "/root/.axon_site/_ro/trn_rl_repo"):
    if _p not in sys.path:
        sys.path.append(_p)


import numpy as np
from contextlib import ExitStack

import concourse.bass as bass
import concourse.tile as tile
from concourse import mybir
from concourse._compat import with_exitstack

F32 = mybir.dt.float32
F16 = mybir.dt.float16
BF16 = mybir.dt.bfloat16
I16 = mybir.dt.int16

RADIUS = 0.1
NSAMPLE = 32
EPS = 1e-5
N2C = 4608         # compacted+padded support count (host-side bound)
SUB = 1536         # per-tile support subset budget
WORDS = SUB // 8   # 192
PAIRS = SUB // 16  # 96
NQ = 2048          # queries per core
NQT = 16           # query tiles per core
GT = 4             # query tiles per gather group
NGRP = NQT // GT   # 4
BIG = 1024.0       # exactly representable in fp16
R2 = float(np.float32(0.01))  # threshold as f32
DROWS = 15         # fp16 d2 decomposition rows
CHUNKS = [(0, 1024), (1024, 512)]

ALU = mybir.AluOpType
ACTF = mybir.ActivationFunctionType


# --------------------------------------------------------------------------
# host-side preparation
# --------------------------------------------------------------------------

def _split_hilo(x, grid=1024.0):
    """Grid split: x = hi + lo with hi on 1/grid grid (exact in fp16 for the
    value ranges used here)."""
    x = x.astype(np.float32)
    hi = np.floor(x.astype(np.float64) * grid) / grid
    hi = hi.astype(np.float32)
    lo = (x - hi).astype(np.float32)
    return hi, lo


def _kd_leaves(pts, idx, depth):
    if depth == 0:
        return [idx]
    ext = pts[idx].max(0) - pts[idx].min(0)
    ax = int(np.argmax(ext))
    order = idx[np.argsort(pts[idx, ax], kind="stable")]
    h = len(order) // 2
    return (_kd_leaves(pts, order[:h], depth - 1)
            + _kd_leaves(pts, order[h:], depth - 1))


_PERMS = {}


def host_prep(inputs):
    B = 2
    qx = np.asarray(inputs['query_xyz'], np.float32)
    sx = np.asarray(inputs['support_xyz'], np.float32)
    qm = np.asarray(inputs['query_mask'], np.int32)
    sm = np.asarray(inputs['support_mask'], np.int32)
    sf = np.asarray(inputs['support_features'], np.float32)

    W0 = np.asarray(inputs['W0'], np.float64)
    W1 = np.asarray(inputs['W1'], np.float64)
    W2 = np.asarray(inputs['W2'], np.float64)

    def fold(Wl, g, b, rm, rv):
        s = np.asarray(g, np.float64) / np.sqrt(np.asarray(rv, np.float64) + EPS)
        return Wl * s[:, None], np.asarray(b, np.float64) - np.asarray(rm, np.float64) * s

    W0p, t0 = fold(W0, inputs['g0'], inputs['b0'], inputs['rm0'], inputs['rv0'])
    W1p, t1 = fold(W1, inputs['g1'], inputs['b1'], inputs['rm1'], inputs['rv1'])
    W2p, t2 = fold(W2, inputs['g2'], inputs['b2'], inputs['rm2'], inputs['rv2'])

    P0 = W0p[:, 0:3] / RADIUS
    C0 = W0p[:, 3:67]
    D0 = W0p[:, 67:131]

    w1t4 = np.tile(W1p.T.astype(np.float32), (4, 1))       # [128, 32]
    w2t = W2p.T.astype(np.float32)                         # [32, 128]
    t1v = t1.astype(np.float32).reshape(32, 1)
    t2v = t2.astype(np.float32).reshape(128, 1)

    # permutation matmul weights for the wrapped gather index layout:
    # idxw[p, 2r+h] = idxg[32*(p//32) + 16h + p%16, r]
    Mh = np.zeros((2, 128, 128), np.float32)
    for h in range(2):
        for p in range(128):
            Mh[h, 32 * (p // 32) + 16 * h + p % 16, p] = 1.0
    ident = np.eye(128, dtype=np.float32)

    pow8 = np.tile((2.0 ** (np.arange(1024) % 8)).astype(np.float32)[None, :], (128, 1))
    # scatter id source, pre-scaled by 16 so cjp1 = dstID + tpat16 yields
    # (slot*SUB + local_idx + 1) directly
    iotag = np.tile(((np.arange(GT * PAIRS, dtype=np.int16) + 1) * 16)[None, :],
                    (128, 1))
    shv = np.tile(np.arange(8, dtype=np.int16)[None, :], (128, 1))
    tpat16 = np.tile((np.tile(np.arange(16, dtype=np.int16), 34) - 15)[None, :],
                     (128, 1))
    onesk1 = np.ones((1, 128), np.float32)

    batch_sup = []
    for b in range(B):
        # order-preserving compaction by support_mask; original point 0 is
        # always table entry 0 (selection-masked if its mask is 0) so the
        # zero-neighbor fill gathers the same point the reference does.
        valid = sm[b] > 0
        keep = np.nonzero(valid)[0]
        sel0 = True
        if not valid[0]:
            keep = np.concatenate([[0], keep])
            sel0 = False
        nv = len(keep)
        assert nv <= N2C, (nv, N2C)
        s = np.zeros((N2C, 3), np.float32)
        s[:nv] = sx[b][keep]
        fts = np.zeros((64, N2C), np.float32)
        fts[:, :nv] = sf[b][:, keep]
        selmask = np.zeros(N2C, np.float32)
        selmask[:nv] = 1.0
        if not sel0:
            selmask[0] = 0.0

        # fp16 d2 decomposition (support side), global-compacted columns
        sh, sl = _split_hilo(s)
        s64, sh64 = s.astype(np.float64), sh.astype(np.float64)
        Ls = (np.sum(s64 * s64, 1) - np.sum(sh64 * sh64, 1)).astype(np.float32)
        sh2 = np.sum(sh64 * sh64, 1).astype(np.float32)
        hi_s, lo_s = _split_hilo(sh2, 512.0)
        rhsg = np.zeros((DROWS, N2C), np.float32)
        rhsg[0:3] = sh.T
        rhsg[3:6] = -2.0 * sh.T
        rhsg[6:9] = -2.0 * sl.T
        rhsg[9] = 1.0
        rhsg[10] = 1.0
        rhsg[11] = 1.0
        rhsg[12] = hi_s
        rhsg[13] = lo_s + Ls
        rhsg[14] = BIG * (1.0 - selmask)

        # (G,V) pair table in global-compacted index space
        G = D0 @ fts.astype(np.float64) + (P0 @ s.T.astype(np.float64))
        V = (C0 - D0) @ fts.astype(np.float64)
        gvpair = np.empty((32, 2 * N2C), np.float32)
        gvpair[:, 0::2] = G.astype(np.float32)
        gvpair[:, 1::2] = V.astype(np.float32)
        batch_sup.append((rhsg, gvpair, s[:nv], nv))

    import ml_dtypes
    npdt = {F32: np.float32, F16: np.float16, BF16: ml_dtypes.bfloat16,
            I16: np.int16}
    in_maps = []
    for c in range(8):
        b = c // 4
        q0 = (c % 4) * NQ
        rhsg, gvpair, s_c, nv = batch_sup[b]

        # k-d reorder queries so each 128-tile is spatially compact
        qraw = qx[b, q0:q0 + NQ]
        perm = np.concatenate(_kd_leaves(qraw, np.arange(NQ), 4))
        _PERMS[c] = perm
        q = qraw[perm]
        qmk = qm[b, q0:q0 + NQ].astype(np.float32)[perm]

        qh, ql = _split_hilo(q)
        q64, qh64 = q.astype(np.float64), qh.astype(np.float64)
        Lq = (np.sum(q64 * q64, 1) - np.sum(qh64 * qh64, 1)).astype(np.float32)
        qh2 = np.sum(qh64 * qh64, 1).astype(np.float32)
        hi_q, lo_q = _split_hilo(qh2, 512.0)
        lhsq = np.zeros((DROWS, NQ), np.float32)
        lhsq[0:3] = -2.0 * qh.T
        lhsq[3:6] = ql.T
        lhsq[6:9] = q.T
        lhsq[9] = hi_q
        lhsq[10] = lo_q + Lq
        lhsq[11] = BIG * (1 - qmk)
        lhsq[12] = 1.0
        lhsq[13] = 1.0
        lhsq[14] = 1.0

        # per-tile support subsets (within bbox + RADIUS), tile-local tables
        rhsd2t = np.zeros((DROWS, NQT * SUB), np.float32)
        rhsd2t[14] = BIG
        gvt = np.zeros((4, 32, 2 * GT * SUB), np.float32)   # per group
        for i in range(NQT):
            qt = q[i * 128:(i + 1) * 128]
            lo = qt.min(0) - RADIUS
            hi = qt.max(0) + RADIUS
            subs = np.nonzero(np.all((s_c >= lo) & (s_c <= hi), axis=1))[0]
            if len(subs) == 0 or subs[0] != 0:
                subs = np.concatenate([[0], subs])   # fill fallback -> entry 0
            ns = len(subs)
            assert ns <= SUB, (ns, SUB)
            rhsd2t[:, i * SUB:i * SUB + ns] = rhsg[:, subs]
            g_, t_ = i // GT, i % GT
            pcols = (2 * subs[:, None] + np.arange(2)[None, :]).ravel()
            gvt[g_, :, 2 * t_ * SUB:2 * t_ * SUB + 2 * ns] = gvpair[:, pcols]

        # qdB[32g + u, i*32 + q'] = t0[u] - P0 @ q(i*128 + 32g + q')
        P0q = (P0 @ q.T.astype(np.float64)).reshape(32, NQT, 4, 32)
        qdB = np.zeros((128, 512), np.float64)
        for g in range(4):
            qdB[32 * g:32 * g + 32, :] = (
                t0[:, None] - P0q[:, :, g, :].reshape(32, NQT * 32))

        im = dict(
            lhsq=lhsq, rhsd2t=rhsd2t, qdB=qdB,
            gvt0=np.tile(gvt[0], (4, 1)), gvt1=np.tile(gvt[1], (4, 1)),
            gvt2=np.tile(gvt[2], (4, 1)), gvt3=np.tile(gvt[3], (4, 1)),
            t1v=t1v, t2v=t2v,
            w1t4=w1t4, w2t=w2t,
            mh0=Mh[0], mh1=Mh[1], ident=ident,
            pow8=pow8, iotag=iotag, shv=shv, tpat16=tpat16,
            qfm=qmk.reshape(NQT, 128).T.copy(),
            onesk1=onesk1,
        )
        for k in im:
            shape, dt = IN_SPECS[k]
            arr = np.ascontiguousarray(im[k]).astype(npdt[dt])
            assert arr.shape == shape, (k, arr.shape, shape)
            im[k] = arr
        in_maps.append(im)
    return in_maps


def host_finish(results):
    out = np.zeros((2, 128, 8192), np.float32)
    for c in range(8):
        b = c // 4
        q0 = (c % 4) * NQ
        out[b][:, q0 + _PERMS[c]] = results[c]['out']
    return out


IN_SPECS = dict(
    lhsq=((DROWS, NQ), F16), rhsd2t=((DROWS, NQT * SUB), F16),
    qdB=((128, 512), F32),
    gvt0=((128, 2 * GT * SUB), BF16), gvt1=((128, 2 * GT * SUB), BF16),
    gvt2=((128, 2 * GT * SUB), BF16), gvt3=((128, 2 * GT * SUB), BF16),
    t1v=((32, 1), F32), t2v=((128, 1), F32),
    w1t4=((128, 32), BF16), w2t=((32, 128), BF16),
    mh0=((128, 128), F32), mh1=((128, 128), F32), ident=((128, 128), F32),
    pow8=((128, 1024), BF16), iotag=((128, GT * PAIRS), I16), shv=((128, 8), I16),
    tpat16=((128, 544), I16), qfm=((128, NQT), F32), onesk1=((1, 128), F32),
)


# --------------------------------------------------------------------------
# device kernel
# --------------------------------------------------------------------------

@with_exitstack
def build_kernel(ctx: ExitStack, tc: tile.TileContext, out_ap: bass.AP, ins: dict):
    nc = tc.nc
    ctx.enter_context(nc.allow_low_precision("bf16 mlp + exact small-int sums"))

    consts = ctx.enter_context(tc.tile_pool(name="consts", bufs=1))
    gvp = ctx.enter_context(tc.tile_pool(name="gv", bufs=1))
    selp = ctx.enter_context(tc.tile_pool(name="sel", bufs=2))
    selp1 = ctx.enter_context(tc.tile_pool(name="sel1", bufs=1))
    selp2 = ctx.enter_context(tc.tile_pool(name="sel2", bufs=1))
    smallp = ctx.enter_context(tc.tile_pool(name="small", bufs=1))
    idxwp = ctx.enter_context(tc.tile_pool(name="idxw", bufs=2))
    gvtp = ctx.enter_context(tc.tile_pool(name="gvt", bufs=2))
    mlpp = ctx.enter_context(tc.tile_pool(name="mlp", bufs=2))
    mlpp1 = ctx.enter_context(tc.tile_pool(name="mlp1", bufs=1))
    outp = ctx.enter_context(tc.tile_pool(name="outb", bufs=1))
    ps_d2 = ctx.enter_context(tc.tile_pool(name="psd2", bufs=2, space="PSUM"))
    ps_l2 = ctx.enter_context(tc.tile_pool(name="psl2", bufs=1, space="PSUM"))
    ps_l3 = ctx.enter_context(tc.tile_pool(name="psl3", bufs=1, space="PSUM"))

    GVT_NAMES = ("gvt0", "gvt1", "gvt2", "gvt3")
    ct = {}
    for name, (shape, dt) in IN_SPECS.items():
        if name in GVT_NAMES or name == "rhsd2t":
            continue
        t = consts.tile(list(shape), dt, tag=f"c_{name}")
        nc.sync.dma_start(out=t[:], in_=ins[name])
        ct[name] = t
    qdB = ct['qdB']

    c33 = consts.tile([128, 544], BF16, tag="c33")
    nc.vector.memset(c33[:], 33.0)
    ones34 = consts.tile([128, 34], I16, tag="ones34")
    nc.vector.memset(ones34[:], 1)

    # persistent per-core state
    idxall = gvp.tile([128, 512], F32, tag="idxall")   # final idx per qtile (f32)
    ceffall = gvp.tile([128, NQT], F32, tag="ceffall")
    outbuf = outp.tile([128, NQ], F32, tag="outbuf")

    def load_gvt(g):
        gvtg = gvtp.tile([128, 2 * GT * SUB], BF16, tag="gvtg")
        nc.sync.dma_start(out=gvtg[:], in_=ins[GVT_NAMES[g]])
        return gvtg

    # ---- phase A for a whole group of GT query tiles: per-tile d2 matmuls +
    # mask words, then batched selection post-processing ----
    GP = GT * PAIRS   # 384
    GS = GT * 544     # 2176

    def emit_group_A(g):
        rhsg2 = gvtp.tile([DROWS, GT * SUB], F16, tag="rhsg2")
        nc.sync.dma_start(out=rhsg2[:],
                          in_=ins['rhsd2t'][:, g * GT * SUB:(g + 1) * GT * SUB])
        w8g = selp1.tile([128, GT * WORDS], BF16, tag="w8g")
        for t in range(GT):
            i = g * GT + t
            for (off, csz) in CHUNKS:
                pd2 = ps_d2.tile([128, 1024], F32, tag="ps_d2")
                for n in range(csz // 512):
                    nc.tensor.matmul(
                        pd2[:, bass.ts(n, 512)],
                        ct['lhsq'][:, bass.ts(i, 128)],
                        rhsg2[:, t * SUB + off + 512 * n:
                              t * SUB + off + 512 * (n + 1)],
                        start=True, stop=True)
                vw8c = selp.tile([128, 1024], BF16, tag="vw8c")
                nc.vector.scalar_tensor_tensor(
                    vw8c[:, 0:csz], pd2[:, 0:csz], R2, ct['pow8'][:, 0:csz],
                    op0=ALU.is_lt, op1=ALU.mult)
                nc.vector.tensor_reduce(
                    w8g[:, t * WORDS + off // 8:t * WORDS + (off + csz) // 8],
                    vw8c[:, 0:csz].rearrange("p (w t) -> p w t", t=8),
                    mybir.AxisListType.X, ALU.add)

        w8v = w8g[:].rearrange("p (c two) -> p c two", two=2)   # c = GP
        # scatter sources converted on the Pool engine (same queue as the
        # scatters that consume them -- no cross-engine hop, Pool is idle)
        w8e = selp2.tile([128, GP], I16, tag="w8e")
        w8o = selp2.tile([128, GP], I16, tag="w8o")
        nc.gpsimd.tensor_scalar(w8e[:], w8v[:, :, 0], 0.0, None, ALU.add)
        nc.gpsimd.tensor_scalar(w8o[:], w8v[:, :, 1], 0.0, None, ALU.add)

        nz = smallp.tile([128, GP], BF16, tag="nz")
        nc.vector.tensor_tensor(nz[:], w8v[:, :, 0], w8v[:, :, 1],
                                ALU.logical_or)
        crank = smallp.tile([128, GP], BF16, tag="crank")
        for t in range(GT):
            nc.vector.tensor_tensor_scan(
                crank[:, t * PAIRS:(t + 1) * PAIRS],
                nz[:, t * PAIRS:(t + 1) * PAIRS], c33[:, 0:PAIRS], 0.0,
                ALU.add, ALU.min)
        u = smallp.tile([128, GP], BF16, tag="u")
        nc.vector.tensor_tensor(u[:], crank[:], nz[:], ALU.mult)
        v = smallp.tile([128, GP], BF16, tag="nz")
        nc.vector.scalar_tensor_tensor(v[:], u[:], 32.5, u[:], op0=ALU.is_le,
                                       op1=ALU.mult)
        si16 = selp2.tile([128, GP], I16, tag="si16")
        nc.vector.tensor_scalar(si16[:], v[:], -1.0, None, ALU.add)

        dstID = selp2.tile([128, GT * 34], I16, tag="dstID")
        dstWe = selp2.tile([128, GT * 34], I16, tag="dstWe")
        dstWo = selp2.tile([128, GT * 34], I16, tag="dstWo")
        for t in range(GT):
            sl_ = slice(t * PAIRS, (t + 1) * PAIRS)
            ds_ = slice(t * 34, (t + 1) * 34)
            nc.gpsimd.local_scatter(dstID[:, ds_], ct['iotag'][:, sl_],
                                    si16[:, sl_], 128, 34, PAIRS)
            nc.gpsimd.local_scatter(dstWe[:, ds_], w8e[:, sl_], si16[:, sl_],
                                    128, 34, PAIRS)
            nc.gpsimd.local_scatter(dstWo[:, ds_], w8o[:, sl_], si16[:, sl_],
                                    128, 34, PAIRS)

        esel16 = selp2.tile([128, GS], I16, tag="esel16")
        # per tile: esel col s*16 + b*8 + u  <-  bit u of dstW(b) col s
        evb = esel16[:].rearrange("p (T s b u) -> p b u T s", s=34, b=2, u=8)
        onesT34 = ones34[:].unsqueeze(1).broadcast_to((128, GT, 34))
        for bidx, dstWx in ((0, dstWe), (1, dstWo)):
            dwv = dstWx[:].rearrange("p (T s) -> p T s", s=34)
            for t in range(8):
                nc.vector.scalar_tensor_tensor(evb[:, bidx, t], dwv,
                                               ct['shv'][:, t:t + 1], onesT34,
                                               op0=ALU.logical_shift_right,
                                               op1=ALU.bitwise_and)
        esel = smallp.tile([128, GS], BF16, tag="esel")
        nc.gpsimd.tensor_scalar(esel[:], esel16[:], 0.0, None, ALU.add)
        cjp1 = selp2.tile([128, GS], I16, tag="cjp1")
        nc.vector.tensor_tensor(
            cjp1[:].rearrange("p (T s u) -> p T s u", s=34, u=16),
            dstID[:].rearrange("p (T s) -> p T s", s=34)
                .unsqueeze(3).broadcast_to((128, GT, 34, 16)),
            ct['tpat16'][:].rearrange("p (s u) -> p s u", u=16)
                .unsqueeze(1).broadcast_to((128, GT, 34, 16)),
            ALU.add)

        crank2 = smallp.tile([128, GS], BF16, tag="crank2")
        for t in range(GT):
            nc.vector.tensor_tensor_scan(
                crank2[:, t * 544:(t + 1) * 544],
                esel[:, t * 544:(t + 1) * 544], c33[:], 0.0, ALU.add, ALU.min)
        # effective count (with query-mask fallback to 32)
        cnt0 = smallp.tile([128, GT], F32, tag="cnt0")
        nc.vector.tensor_scalar(
            cnt0[:], crank2[:].rearrange("p (T x) -> p T x", x=544)[:, :, 543],
            32.0, None, ALU.min)
        qfc = smallp.tile([128, GT], F32, tag="qfc")
        nc.vector.tensor_scalar(qfc[:], ct['qfm'][:, g * GT:(g + 1) * GT],
                                -32.0, 32.0, ALU.mult, ALU.add)
        nc.vector.tensor_tensor(ceffall[:, g * GT:(g + 1) * GT], cnt0[:],
                                qfc[:], ALU.max)
        u2 = smallp.tile([128, GS], BF16, tag="u2")
        nc.vector.tensor_tensor(u2[:], crank2[:], esel[:], ALU.mult)
        v2 = smallp.tile([128, GS], BF16, tag="esel")
        nc.vector.scalar_tensor_tensor(v2[:], u2[:], 32.5, u2[:], op0=ALU.is_le,
                                       op1=ALU.mult)
        si2 = selp2.tile([128, GS], I16, tag="si2")
        nc.vector.tensor_scalar(si2[:], v2[:], -1.0, None, ALU.add)
        idxp1 = selp2.tile([128, GT * 34], I16, tag="idxp1")
        for t in range(GT):
            nc.gpsimd.local_scatter(idxp1[:, t * 34:(t + 1) * 34],
                                    cjp1[:, t * 544:(t + 1) * 544],
                                    si2[:, t * 544:(t + 1) * 544], 128, 34, 544)

        # fill + final gather indices (kept in f32 for the phase-B matmul);
        # idx values are slot*SUB + local; local fallback 0 = global point 0
        ii = smallp.tile([128, GT * 32], F32, tag="ii")
        nc.gpsimd.tensor_scalar(
            ii[:].rearrange("p (T r) -> p T r", r=32),
            idxp1[:].rearrange("p (T s) -> p T s", s=34)[:, :, 0:32],
            0.0, None, ALU.add)
        iv = ii[:].rearrange("p (T r) -> p T r", r=32)
        flp1 = smallp.tile([128, GT], F32, tag="flp1")
        nc.vector.tensor_scalar(flp1[:], iv[:, :, 0], 1.0, None, ALU.max)
        flb = flp1[:].unsqueeze(2).broadcast_to((128, GT, 32))
        m = smallp.tile([128, GT * 32], F32, tag="m")
        nc.vector.tensor_scalar(m[:], ii[:], 0.0, None, ALU.is_gt)
        bb = smallp.tile([128, GT * 32], F32, tag="bb")
        nc.vector.tensor_tensor(bb[:], ii[:], m[:], ALU.mult)
        aa = smallp.tile([128, GT * 32], F32, tag="aa")
        nc.vector.tensor_tensor(aa[:].rearrange("p (T r) -> p T r", r=32),
                                m[:].rearrange("p (T r) -> p T r", r=32), flb,
                                ALU.mult)
        cc = smallp.tile([128, GT * 32], F32, tag="m")
        nc.vector.tensor_tensor(cc[:], bb[:], aa[:], ALU.subtract)
        dd0 = smallp.tile([128, GT * 32], F32, tag="bb")
        nc.vector.tensor_tensor(dd0[:].rearrange("p (T r) -> p T r", r=32),
                                cc[:].rearrange("p (T r) -> p T r", r=32), flb,
                                ALU.add)
        nc.vector.tensor_scalar(idxall[:, g * GT * 32:(g + 1) * GT * 32],
                                dd0[:], -1.0, None, ALU.add)

        # wrapped gather index slots via two group-wide permutation matmuls
        psWg = ps_d2.tile([128, 256], F32, tag="ps_d2")
        nc.tensor.matmul(psWg[:, 0:128], ct['mh0'][:],
                         idxall[:, g * 128:(g + 1) * 128], start=True, stop=True)
        nc.tensor.matmul(psWg[:, 128:256], ct['mh1'][:],
                         idxall[:, g * 128:(g + 1) * 128], start=True, stop=True)
        idxwg = idxwp.tile([128, GT * 64], I16, tag="idxwg")
        ivw = idxwg[:].rearrange("p (T k) -> p T k", k=64)
        nc.scalar.activation(ivw[:, :, 0::2],
                             psWg[:, 0:128].rearrange("p (T r) -> p T r", r=32),
                             ACTF.Copy)
        nc.scalar.activation(ivw[:, :, 1::2],
                             psWg[:, 128:256].rearrange("p (T r) -> p T r", r=32),
                             ACTF.Copy)
        return idxwg

    # ---- phase B: gathered-MLP + pooling for query tile i ----
    def emit_mlp(i, gout):
        gv_g = gout.rearrange("p (r q u) -> p r q u", r=32, u=2)[:, :, :, 0]
        gv_v0 = gout.rearrange("p (k u) -> p k u", u=2)[:, 0:32, 1]

        # d = V(center) + qdelta; h1 = relu(G + d)
        dd = mlpp.tile([128, 32], F32, tag="dd")
        nc.vector.tensor_tensor(dd[:], gv_v0, qdB[:, bass.ts(i, 32)], ALU.add)
        h1t = mlpp1.tile([128, 1024], F32, tag="h1t")
        nc.vector.tensor_tensor(
            h1t[:].rearrange("p (r q) -> p r q", q=32), gv_g,
            dd[:].unsqueeze(1).broadcast_to((128, 32, 32)), ALU.add)
        h1 = mlpp.tile([128, 1024], BF16, tag="h1")
        nc.scalar.activation(h1[:], h1t[:], ACTF.Relu)

        # layer 2: per unit uu (K=32 at partition 32*uu)
        h2 = mlpp1.tile([32, 4096], BF16, tag="h2")
        for uu in range(4):
            psL2 = ps_l2.tile([32, 1024], F32, tag="ps_a")
            for n in range(2):
                nc.tensor.matmul(
                    psL2[:, bass.ts(n, 512)],
                    ct['w1t4'][32 * uu:32 * uu + 32, :],
                    h1[32 * uu:32 * uu + 32, bass.ts(n, 512)],
                    start=True, stop=True,
                    tile_position=(32 * uu, 0))
            nc.scalar.activation(h2[:, bass.ts(uu, 1024)], psL2[:], ACTF.Relu,
                                 bias=ct['t1v'][:])

        # layer 3
        h3 = mlpp1.tile([128, 4096], BF16, tag="h3")
        for n2 in range(4):
            psL3 = ps_l3.tile([128, 1024], F32, tag="ps_b3")
            for n in range(2):
                nc.tensor.matmul(psL3[:, bass.ts(n, 512)], ct['w2t'][:],
                                 h2[:, bass.ts(2 * n2 + n, 512)],
                                 start=True, stop=True)
            nc.scalar.activation(h3[:, bass.ts(n2, 1024)], psL3[:], ACTF.Relu,
                                 bias=ct['t2v'][:])

        # pooling
        S = smallp.tile([128, 128], F32, tag="S")
        h30 = smallp.tile([128, 128], F32, tag="h30")
        h3v = h3[:].rearrange("p (a r q) -> p a r q", a=4, r=32)
        for a in range(4):
            nc.vector.tensor_reduce(
                S[:, bass.ts(a, 32)], h3v[:, a, :, :].transpose([0, 2, 1]),
                mybir.AxisListType.X, ALU.add)
            nc.scalar.activation(h30[:, bass.ts(a, 32)], h3v[:, a, 0, :], ACTF.Copy)

        # beta/gamma rows via PE transpose + broadcast
        ceff = ceffall[:, i:i + 1]
        beta = smallp.tile([128, 1], F32, tag="beta")
        nc.vector.reciprocal(beta[:], ceff)
        gm0 = smallp.tile([128, 1], F32, tag="gm0")
        nc.vector.tensor_scalar(gm0[:], ceff, -1.0, 32.0, ALU.mult, ALU.add)
        gamma = smallp.tile([128, 1], F32, tag="gamma")
        nc.vector.tensor_tensor(gamma[:], gm0[:], beta[:], ALU.mult)
        psBG = ps_d2.tile([1, 256], F32, tag="ps_d2")
        nc.tensor.matmul(psBG[:, 0:128], beta[:], ct['ident'][:],
                         start=True, stop=True)
        nc.tensor.matmul(psBG[:, 128:256], gamma[:], ct['ident'][:],
                         start=True, stop=True)
        bgrow = smallp.tile([1, 256], F32, tag="bgrow")
        nc.vector.tensor_copy(bgrow[:], psBG[:])
        psB = ps_d2.tile([128, 256], F32, tag="ps_d2")
        nc.tensor.matmul(psB[:], ct['onesk1'][:], bgrow[:], start=True, stop=True)

        e1 = smallp.tile([128, 128], F32, tag="e1")
        nc.vector.tensor_tensor(e1[:], S[:], psB[:, 0:128], ALU.mult)
        e2 = smallp.tile([128, 128], F32, tag="e2")
        nc.vector.tensor_tensor(e2[:], h30[:], psB[:, 128:256], ALU.mult)
        nc.vector.tensor_tensor(outbuf[:, bass.ts(i, 128)], e1[:], e2[:],
                                ALU.subtract)

    # ==== software-pipelined groups: gather(g) | mlp(g) | select(g+1) ====
    gvtg = load_gvt(0)
    idxwg = emit_group_A(0)
    for g in range(NGRP):
        goutg = mlpp.tile([128, GT * 2048], BF16, tag="goutg")
        nc.gpsimd.ap_gather(goutg[:].rearrange("p (k u) -> p k u", u=2),
                            gvtg[:].rearrange("p (j u) -> p j u", u=2),
                            idxwg[:], 128, GT * SUB, 2, GT * 1024)
        if g + 1 < NGRP:
            gvtg = load_gvt(g + 1)
        for t in range(GT):
            emit_mlp(g * GT + t, goutg[:, t * 2048:(t + 1) * 2048])
        if g + 1 < NGRP:
            idxwg = emit_group_A(g + 1)
        nc.sync.dma_start(out=out_ap[:, g * GT * 128:(g + 1) * GT * 128],
                          in_=outbuf[:, g * GT * 128:(g + 1) * GT * 128])


# ==========================================================================
# harness entry point: kernel(**inputs) -> full output [2, 128, 8192]
# ==========================================================================

_CACHE = {}


def _build_nc():
    import concourse.bacc as bacc
    import concourse.tile as tile_mod
    nc = bacc.Bacc("TRN2", target_bir_lowering=False, debug=False, num_devices=8)
    in_tiles = {}
    for name, (shape, dt) in IN_SPECS.items():
        in_tiles[name] = nc.dram_tensor(
            name, list(shape), dt, kind="ExternalInput").ap()
    out_tile = nc.dram_tensor("out", (128, NQ), F32, kind="ExternalOutput").ap()
    with tile_mod.TileContext(nc) as t:
        build_kernel(t, out_tile, in_tiles)
    nc.compile()
    return nc


def kernel(**inputs):
    from concourse.bass_utils import run_bass_kernel_spmd
    in_maps = host_prep(inputs)
    if "nc" not in _CACHE:
        _CACHE["nc"] = _build_nc()
    res = run_bass_kernel_spmd(_CACHE["nc"], in_maps, list(range(8)))
    return host_finish(res.results)


# revision 43
# speedup vs baseline: 1.3160x; 1.0064x over previous
"""Trainium2 Bass kernel for nn_PointWiseMLP (ball query + gather + MLP + pool).

Self-contained: kernel(**inputs) shards across 8 NeuronCores (data-parallel
over batch x query-range), runs the Bass/Tile kernel via run_bass_kernel_spmd,
and gathers the full [2, 128, 8192] output.

v3: - support compacted by support_mask on host (order-preserving) and
      k-d-tree query reordering so each 128-query tile is spatially compact;
      each tile only tests the support points inside its radius-expanded bbox
      (<= SUB=1536 of 8192), cutting ball-query work ~5x.
    - per-tile (G,V) pair tables are built on host in tile-local index space
      and DMA'd per group, so the selection pipeline's local indices feed the
      gather directly (no index translation anywhere).
    - d2 decomposed into 15 fp16 rows (4x faster on PE than f32, ~1e-6 exact).
    - selection post-processing batched per 4-tile group, scan values in bf16,
      scatter-source conversions on the (idle) gpsimd engine.
"""
import sys
for _p in ("/opt/trn_rl_repo", "/root/.axon_site/_ro/trn_rl_repo"):
    if _p not in sys.path:
        sys.path.append(_p)


import numpy as np
from contextlib import ExitStack

import concourse.bass as bass
import concourse.tile as tile
from concourse import mybir
from concourse._compat import with_exitstack

F32 = mybir.dt.float32
F16 = mybir.dt.float16
BF16 = mybir.dt.bfloat16
I16 = mybir.dt.int16

RADIUS = 0.1
NSAMPLE = 32
EPS = 1e-5
N2C = 4608         # compacted+padded support count (host-side bound)
SUB = 1536         # per-tile support subset budget
WORDS = SUB // 8   # 192
PAIRS = SUB // 16  # 96
NQ = 2048          # queries per core
NQT = 16           # query tiles per core
GT = 4             # query tiles per gather group
NGRP = NQT // GT   # 4
BIG = 1024.0       # exactly representable in fp16
R2 = float(np.float32(0.01))  # threshold as f32
DROWS = 15         # fp16 d2 decomposition rows
CHUNKS = [(0, 512), (512, 512), (1024, 512)]

ALU = mybir.AluOpType
ACTF = mybir.ActivationFunctionType


# --------------------------------------------------------------------------
# host-side preparation
# --------------------------------------------------------------------------

def _split_hilo(x, grid=1024.0):
    """Grid split: x = hi + lo with hi on 1/grid grid (exact in fp16 for the
    value ranges used here)."""
    x = x.astype(np.float32)
    hi = np.floor(x.astype(np.float64) * grid) / grid
    hi = hi.astype(np.float32)
    lo = (x - hi).astype(np.float32)
    return hi, lo


def _kd_leaves(pts, idx, depth):
    if depth == 0:
        return [idx]
    ext = pts[idx].max(0) - pts[idx].min(0)
    ax = int(np.argmax(ext))
    order = idx[np.argsort(pts[idx, ax], kind="stable")]
    h = len(order) // 2
    return (_kd_leaves(pts, order[:h], depth - 1)
            + _kd_leaves(pts, order[h:], depth - 1))


_PERMS = {}


def host_prep(inputs):
    B = 2
    qx = np.asarray(inputs['query_xyz'], np.float32)
    sx = np.asarray(inputs['support_xyz'], np.float32)
    qm = np.asarray(inputs['query_mask'], np.int32)
    sm = np.asarray(inputs['support_mask'], np.int32)
    sf = np.asarray(inputs['support_features'], np.float32)

    W0 = np.asarray(inputs['W0'], np.float64)
    W1 = np.asarray(inputs['W1'], np.float64)
    W2 = np.asarray(inputs['W2'], np.float64)

    def fold(Wl, g, b, rm, rv):
        s = np.asarray(g, np.float64) / np.sqrt(np.asarray(rv, np.float64) + EPS)
        return Wl * s[:, None], np.asarray(b, np.float64) - np.asarray(rm, np.float64) * s

    W0p, t0 = fold(W0, inputs['g0'], inputs['b0'], inputs['rm0'], inputs['rv0'])
    W1p, t1 = fold(W1, inputs['g1'], inputs['b1'], inputs['rm1'], inputs['rv1'])
    W2p, t2 = fold(W2, inputs['g2'], inputs['b2'], inputs['rm2'], inputs['rv2'])

    P0 = W0p[:, 0:3] / RADIUS
    C0 = W0p[:, 3:67]
    D0 = W0p[:, 67:131]

    w1t4 = np.tile(W1p.T.astype(np.float32), (4, 1))       # [128, 32]
    w2t = W2p.T.astype(np.float32)                         # [32, 128]
    t1v = t1.astype(np.float32).reshape(32, 1)
    t2v = t2.astype(np.float32).reshape(128, 1)

    # permutation matmul weights for the wrapped gather index layout:
    # idxw[p, 2r+h] = idxg[32*(p//32) + 16h + p%16, r]
    Mh = np.zeros((2, 128, 128), np.float32)
    for h in range(2):
        for p in range(128):
            Mh[h, 32 * (p // 32) + 16 * h + p % 16, p] = 1.0
    ident = np.eye(128, dtype=np.float32)

    pow8 = np.tile((2.0 ** (np.arange(1024) % 8)).astype(np.float32)[None, :], (128, 1))
    # scatter id source, pre-scaled by 16 so cjp1 = dstID + tpat16 yields
    # (slot*SUB + local_idx + 1) directly
    iotag = np.tile(((np.arange(GT * PAIRS, dtype=np.int16) + 1) * 16)[None, :],
                    (128, 1))
    shv = np.tile(np.arange(8, dtype=np.int16)[None, :], (128, 1))
    tpat16 = np.tile((np.tile(np.arange(16, dtype=np.int16), 34) - 15)[None, :],
                     (128, 1))
    onesk1 = np.ones((1, 128), np.float32)

    batch_sup = []
    for b in range(B):
        # order-preserving compaction by support_mask; original point 0 is
        # always table entry 0 (selection-masked if its mask is 0) so the
        # zero-neighbor fill gathers the same point the reference does.
        valid = sm[b] > 0
        keep = np.nonzero(valid)[0]
        sel0 = True
        if not valid[0]:
            keep = np.concatenate([[0], keep])
            sel0 = False
        nv = len(keep)
        assert nv <= N2C, (nv, N2C)
        s = np.zeros((N2C, 3), np.float32)
        s[:nv] = sx[b][keep]
        fts = np.zeros((64, N2C), np.float32)
        fts[:, :nv] = sf[b][:, keep]
        selmask = np.zeros(N2C, np.float32)
        selmask[:nv] = 1.0
        if not sel0:
            selmask[0] = 0.0

        # fp16 d2 decomposition (support side), global-compacted columns
        sh, sl = _split_hilo(s)
        s64, sh64 = s.astype(np.float64), sh.astype(np.float64)
        Ls = (np.sum(s64 * s64, 1) - np.sum(sh64 * sh64, 1)).astype(np.float32)
        sh2 = np.sum(sh64 * sh64, 1).astype(np.float32)
        hi_s, lo_s = _split_hilo(sh2, 512.0)
        rhsg = np.zeros((DROWS, N2C), np.float32)
        rhsg[0:3] = sh.T
        rhsg[3:6] = -2.0 * sh.T
        rhsg[6:9] = -2.0 * sl.T
        rhsg[9] = 1.0
        rhsg[10] = 1.0
        rhsg[11] = 1.0
        rhsg[12] = hi_s
        rhsg[13] = lo_s + Ls
        rhsg[14] = BIG * (1.0 - selmask)

        # (G,V) pair table in global-compacted index space
        G = D0 @ fts.astype(np.float64) + (P0 @ s.T.astype(np.float64))
        V = (C0 - D0) @ fts.astype(np.float64)
        gvpair = np.empty((32, 2 * N2C), np.float32)
        gvpair[:, 0::2] = G.astype(np.float32)
        gvpair[:, 1::2] = V.astype(np.float32)
        batch_sup.append((rhsg, gvpair, s[:nv], nv))

    import ml_dtypes
    npdt = {F32: np.float32, F16: np.float16, BF16: ml_dtypes.bfloat16,
            I16: np.int16}
    in_maps = []
    for c in range(8):
        b = c // 4
        q0 = (c % 4) * NQ
        rhsg, gvpair, s_c, nv = batch_sup[b]

        # k-d reorder queries so each 128-tile is spatially compact
        qraw = qx[b, q0:q0 + NQ]
        perm = np.concatenate(_kd_leaves(qraw, np.arange(NQ), 4))
        _PERMS[c] = perm
        q = qraw[perm]
        qmk = qm[b, q0:q0 + NQ].astype(np.float32)[perm]

        qh, ql = _split_hilo(q)
        q64, qh64 = q.astype(np.float64), qh.astype(np.float64)
        Lq = (np.sum(q64 * q64, 1) - np.sum(qh64 * qh64, 1)).astype(np.float32)
        qh2 = np.sum(qh64 * qh64, 1).astype(np.float32)
        hi_q, lo_q = _split_hilo(qh2, 512.0)
        lhsq = np.zeros((DROWS, NQ), np.float32)
        lhsq[0:3] = -2.0 * qh.T
        lhsq[3:6] = ql.T
        lhsq[6:9] = q.T
        lhsq[9] = hi_q
        lhsq[10] = lo_q + Lq
        lhsq[11] = BIG * (1 - qmk)
        lhsq[12] = 1.0
        lhsq[13] = 1.0
        lhsq[14] = 1.0

        # per-tile support subsets (within bbox + RADIUS), tile-local tables
        rhsd2t = np.zeros((DROWS, NQT * SUB), np.float32)
        rhsd2t[14] = BIG
        gvt = np.zeros((4, 32, 2 * GT * SUB), np.float32)   # per group
        for i in range(NQT):
            qt = q[i * 128:(i + 1) * 128]
            lo = qt.min(0) - RADIUS
            hi = qt.max(0) + RADIUS
            subs = np.nonzero(np.all((s_c >= lo) & (s_c <= hi), axis=1))[0]
            if len(subs) == 0 or subs[0] != 0:
                subs = np.concatenate([[0], subs])   # fill fallback -> entry 0
            ns = len(subs)
            assert ns <= SUB, (ns, SUB)
            rhsd2t[:, i * SUB:i * SUB + ns] = rhsg[:, subs]
            g_, t_ = i // GT, i % GT
            pcols = (2 * subs[:, None] + np.arange(2)[None, :]).ravel()
            gvt[g_, :, 2 * t_ * SUB:2 * t_ * SUB + 2 * ns] = gvpair[:, pcols]

        # qdB[32g + u, i*32 + q'] = t0[u] - P0 @ q(i*128 + 32g + q')
        P0q = (P0 @ q.T.astype(np.float64)).reshape(32, NQT, 4, 32)
        qdB = np.zeros((128, 512), np.float64)
        for g in range(4):
            qdB[32 * g:32 * g + 32, :] = (
                t0[:, None] - P0q[:, :, g, :].reshape(32, NQT * 32))

        im = dict(
            lhsq=lhsq, rhsd2t=rhsd2t, qdB=qdB,
            gvt0=np.tile(gvt[0], (4, 1)), gvt1=np.tile(gvt[1], (4, 1)),
            gvt2=np.tile(gvt[2], (4, 1)), gvt3=np.tile(gvt[3], (4, 1)),
            t1v=t1v, t2v=t2v,
            w1t4=w1t4, w2t=w2t,
            mh0=Mh[0], mh1=Mh[1], ident=ident,
            pow8=pow8, iotag=iotag, shv=shv, tpat16=tpat16,
            qfm=qmk.reshape(NQT, 128).T.copy(),
            onesk1=onesk1,
        )
        for k in im:
            shape, dt = IN_SPECS[k]
            arr = np.ascontiguousarray(im[k]).astype(npdt[dt])
            assert arr.shape == shape, (k, arr.shape, shape)
            im[k] = arr
        in_maps.append(im)
    return in_maps


def host_finish(results):
    out = np.zeros((2, 128, 8192), np.float32)
    for c in range(8):
        b = c // 4
        q0 = (c % 4) * NQ
        out[b][:, q0 + _PERMS[c]] = results[c]['out']
    return out


IN_SPECS = dict(
    lhsq=((DROWS, NQ), F16), rhsd2t=((DROWS, NQT * SUB), F16),
    qdB=((128, 512), F32),
    gvt0=((128, 2 * GT * SUB), BF16), gvt1=((128, 2 * GT * SUB), BF16),
    gvt2=((128, 2 * GT * SUB), BF16), gvt3=((128, 2 * GT * SUB), BF16),
    t1v=((32, 1), F32), t2v=((128, 1), F32),
    w1t4=((128, 32), BF16), w2t=((32, 128), BF16),
    mh0=((128, 128), F32), mh1=((128, 128), F32), ident=((128, 128), F32),
    pow8=((128, 1024), BF16), iotag=((128, GT * PAIRS), I16), shv=((128, 8), I16),
    tpat16=((128, 544), I16), qfm=((128, NQT), F32), onesk1=((1, 128), F32),
)


# --------------------------------------------------------------------------
# device kernel
# --------------------------------------------------------------------------

@with_exitstack
def build_kernel(ctx: ExitStack, tc: tile.TileContext, out_ap: bass.AP, ins: dict):
    nc = tc.nc
    ctx.enter_context(nc.allow_low_precision("bf16 mlp + exact small-int sums"))

    consts = ctx.enter_context(tc.tile_pool(name="consts", bufs=1))
    gvp = ctx.enter_context(tc.tile_pool(name="gv", bufs=1))
    selp = ctx.enter_context(tc.tile_pool(name="sel", bufs=2))
    selp1 = ctx.enter_context(tc.tile_pool(name="sel1", bufs=1))
    selp2 = ctx.enter_context(tc.tile_pool(name="sel2", bufs=1))
    smallp = ctx.enter_context(tc.tile_pool(name="small", bufs=1))
    idxwp = ctx.enter_context(tc.tile_pool(name="idxw", bufs=2))
    gvtp = ctx.enter_context(tc.tile_pool(name="gvt", bufs=2))
    mlpp = ctx.enter_context(tc.tile_pool(name="mlp", bufs=2))
    mlpp1 = ctx.enter_context(tc.tile_pool(name="mlp1", bufs=1))
    mlph3 = ctx.enter_context(tc.tile_pool(name="mlph3", bufs=2))
    outp = ctx.enter_context(tc.tile_pool(name="outb", bufs=1))
    ps_d2 = ctx.enter_context(tc.tile_pool(name="psd2", bufs=2, space="PSUM"))
    ps_l2 = ctx.enter_context(tc.tile_pool(name="psl2", bufs=2, space="PSUM"))
    ps_l3 = ctx.enter_context(tc.tile_pool(name="psl3", bufs=2, space="PSUM"))

    GVT_NAMES = ("gvt0", "gvt1", "gvt2", "gvt3")
    ct = {}
    for name, (shape, dt) in IN_SPECS.items():
        if name in GVT_NAMES or name == "rhsd2t":
            continue
        t = consts.tile(list(shape), dt, tag=f"c_{name}")
        nc.sync.dma_start(out=t[:], in_=ins[name])
        ct[name] = t
    qdB = ct['qdB']

    c33 = consts.tile([128, 544], BF16, tag="c33")
    nc.vector.memset(c33[:], 33.0)
    ones34 = consts.tile([128, 34], I16, tag="ones34")
    nc.vector.memset(ones34[:], 1)

    # persistent per-core state
    idxall = gvp.tile([128, 512], F32, tag="idxall")   # final idx per qtile (f32)
    ceffall = gvp.tile([128, NQT], F32, tag="ceffall")
    outbuf = outp.tile([128, NQ], F32, tag="outbuf")

    def load_gvt(g):
        gvtg = gvtp.tile([128, 2 * GT * SUB], BF16, tag="gvtg")
        nc.sync.dma_start(out=gvtg[:], in_=ins[GVT_NAMES[g]])
        return gvtg

    # ---- phase A for a whole group of GT query tiles: per-tile d2 matmuls +
    # mask words, then batched selection post-processing ----
    GP = GT * PAIRS   # 384
    GS = GT * 544     # 2176

    def emit_group_A(g):
        rhsg2 = gvtp.tile([DROWS, GT * SUB], F16, tag="rhsg2")
        nc.sync.dma_start(out=rhsg2[:],
                          in_=ins['rhsd2t'][:, g * GT * SUB:(g + 1) * GT * SUB])
        w8g = selp1.tile([128, GT * WORDS], BF16, tag="w8g")
        for t in range(GT):
            i = g * GT + t
            for (off, csz) in CHUNKS:
                pd2 = ps_d2.tile([128, 512], F32, tag="ps_d2")
                nc.tensor.matmul(
                    pd2[:], ct['lhsq'][:, bass.ts(i, 128)],
                    rhsg2[:, t * SUB + off:t * SUB + off + csz],
                    start=True, stop=True)
                vw8c = selp.tile([128, 512], BF16, tag="vw8c")
                nc.vector.scalar_tensor_tensor(
                    vw8c[:], pd2[:], R2, ct['pow8'][:, 0:512],
                    op0=ALU.is_lt, op1=ALU.mult)
                nc.vector.tensor_reduce(
                    w8g[:, t * WORDS + off // 8:t * WORDS + (off + csz) // 8],
                    vw8c[:].rearrange("p (w t) -> p w t", t=8),
                    mybir.AxisListType.X, ALU.add)

        w8v = w8g[:].rearrange("p (c two) -> p c two", two=2)   # c = GP
        # scatter sources converted on the Pool engine (same queue as the
        # scatters that consume them -- no cross-engine hop, Pool is idle)
        w8e = selp2.tile([128, GP], I16, tag="w8e")
        w8o = selp2.tile([128, GP], I16, tag="w8o")
        nc.gpsimd.tensor_scalar(w8e[:], w8v[:, :, 0], 0.0, None, ALU.add)
        nc.gpsimd.tensor_scalar(w8o[:], w8v[:, :, 1], 0.0, None, ALU.add)

        nz = smallp.tile([128, GP], BF16, tag="nz")
        nc.vector.tensor_tensor(nz[:], w8v[:, :, 0], w8v[:, :, 1],
                                ALU.logical_or)
        crank = smallp.tile([128, GP], BF16, tag="crank")
        for t in range(GT):
            nc.vector.tensor_tensor_scan(
                crank[:, t * PAIRS:(t + 1) * PAIRS],
                nz[:, t * PAIRS:(t + 1) * PAIRS], c33[:, 0:PAIRS], 0.0,
                ALU.add, ALU.min)
        u = smallp.tile([128, GP], BF16, tag="u")
        nc.vector.tensor_tensor(u[:], crank[:], nz[:], ALU.mult)
        v = smallp.tile([128, GP], BF16, tag="nz")
        nc.vector.scalar_tensor_tensor(v[:], u[:], 32.5, u[:], op0=ALU.is_le,
                                       op1=ALU.mult)
        si16 = selp2.tile([128, GP], I16, tag="si16")
        nc.vector.tensor_scalar(si16[:], v[:], -1.0, None, ALU.add)

        dstID = selp2.tile([128, GT * 34], I16, tag="dstID")
        dstWe = selp2.tile([128, GT * 34], I16, tag="dstWe")
        dstWo = selp2.tile([128, GT * 34], I16, tag="dstWo")
        for t in range(GT):
            sl_ = slice(t * PAIRS, (t + 1) * PAIRS)
            ds_ = slice(t * 34, (t + 1) * 34)
            nc.gpsimd.local_scatter(dstID[:, ds_], ct['iotag'][:, sl_],
                                    si16[:, sl_], 128, 34, PAIRS)
            nc.gpsimd.local_scatter(dstWe[:, ds_], w8e[:, sl_], si16[:, sl_],
                                    128, 34, PAIRS)
            nc.gpsimd.local_scatter(dstWo[:, ds_], w8o[:, sl_], si16[:, sl_],
                                    128, 34, PAIRS)

        esel16 = selp2.tile([128, GS], I16, tag="esel16")
        # per tile: esel col s*16 + b*8 + u  <-  bit u of dstW(b) col s
        evb = esel16[:].rearrange("p (T s b u) -> p b u T s", s=34, b=2, u=8)
        onesT34 = ones34[:].unsqueeze(1).broadcast_to((128, GT, 34))
        for bidx, dstWx in ((0, dstWe), (1, dstWo)):
            dwv = dstWx[:].rearrange("p (T s) -> p T s", s=34)
            for t in range(8):
                nc.vector.scalar_tensor_tensor(evb[:, bidx, t], dwv,
                                               ct['shv'][:, t:t + 1], onesT34,
                                               op0=ALU.logical_shift_right,
                                               op1=ALU.bitwise_and)
        esel = smallp.tile([128, GS], BF16, tag="esel")
        nc.gpsimd.tensor_scalar(esel[:], esel16[:], 0.0, None, ALU.add)
        cjp1 = selp2.tile([128, GS], I16, tag="cjp1")
        nc.vector.tensor_tensor(
            cjp1[:].rearrange("p (T s u) -> p T s u", s=34, u=16),
            dstID[:].rearrange("p (T s) -> p T s", s=34)
                .unsqueeze(3).broadcast_to((128, GT, 34, 16)),
            ct['tpat16'][:].rearrange("p (s u) -> p s u", u=16)
                .unsqueeze(1).broadcast_to((128, GT, 34, 16)),
            ALU.add)

        crank2 = smallp.tile([128, GS], BF16, tag="crank2")
        for t in range(GT):
            nc.vector.tensor_tensor_scan(
                crank2[:, t * 544:(t + 1) * 544],
                esel[:, t * 544:(t + 1) * 544], c33[:], 0.0, ALU.add, ALU.min)
        # effective count (with query-mask fallback to 32)
        cnt0 = smallp.tile([128, GT], F32, tag="cnt0")
        nc.vector.tensor_scalar(
            cnt0[:], crank2[:].rearrange("p (T x) -> p T x", x=544)[:, :, 543],
            32.0, None, ALU.min)
        qfc = smallp.tile([128, GT], F32, tag="qfc")
        nc.vector.tensor_scalar(qfc[:], ct['qfm'][:, g * GT:(g + 1) * GT],
                                -32.0, 32.0, ALU.mult, ALU.add)
        nc.vector.tensor_tensor(ceffall[:, g * GT:(g + 1) * GT], cnt0[:],
                                qfc[:], ALU.max)
        u2 = smallp.tile([128, GS], BF16, tag="u2")
        nc.vector.tensor_tensor(u2[:], crank2[:], esel[:], ALU.mult)
        v2 = smallp.tile([128, GS], BF16, tag="esel")
        nc.vector.scalar_tensor_tensor(v2[:], u2[:], 32.5, u2[:], op0=ALU.is_le,
                                       op1=ALU.mult)
        si2 = selp2.tile([128, GS], I16, tag="si2")
        nc.vector.tensor_scalar(si2[:], v2[:], -1.0, None, ALU.add)
        idxp1 = selp2.tile([128, GT * 34], I16, tag="idxp1")
        for t in range(GT):
            nc.gpsimd.local_scatter(idxp1[:, t * 34:(t + 1) * 34],
                                    cjp1[:, t * 544:(t + 1) * 544],
                                    si2[:, t * 544:(t + 1) * 544], 128, 34, 544)

        # fill + final gather indices (kept in f32 for the phase-B matmul);
        # idx values are slot*SUB + local; local fallback 0 = global point 0
        ii = smallp.tile([128, GT * 32], F32, tag="ii")
        nc.gpsimd.tensor_scalar(
            ii[:].rearrange("p (T r) -> p T r", r=32),
            idxp1[:].rearrange("p (T s) -> p T s", s=34)[:, :, 0:32],
            0.0, None, ALU.add)
        iv = ii[:].rearrange("p (T r) -> p T r", r=32)
        flp1 = smallp.tile([128, GT], F32, tag="flp1")
        nc.vector.tensor_scalar(flp1[:], iv[:, :, 0], 1.0, None, ALU.max)
        flb = flp1[:].unsqueeze(2).broadcast_to((128, GT, 32))
        m = smallp.tile([128, GT * 32], F32, tag="m")
        nc.vector.tensor_scalar(m[:], ii[:], 0.0, None, ALU.is_gt)
        bb = smallp.tile([128, GT * 32], F32, tag="bb")
        nc.vector.tensor_tensor(bb[:], ii[:], m[:], ALU.mult)
        aa = smallp.tile([128, GT * 32], F32, tag="aa")
        nc.vector.tensor_tensor(aa[:].rearrange("p (T r) -> p T r", r=32),
                                m[:].rearrange("p (T r) -> p T r", r=32), flb,
                                ALU.mult)
        cc = smallp.tile([128, GT * 32], F32, tag="m")
        nc.vector.tensor_tensor(cc[:], bb[:], aa[:], ALU.subtract)
        dd0 = smallp.tile([128, GT * 32], F32, tag="bb")
        nc.vector.tensor_tensor(dd0[:].rearrange("p (T r) -> p T r", r=32),
                                cc[:].rearrange("p (T r) -> p T r", r=32), flb,
                                ALU.add)
        nc.vector.tensor_scalar(idxall[:, g * GT * 32:(g + 1) * GT * 32],
                                dd0[:], -1.0, None, ALU.add)

        # wrapped gather index slots via two group-wide permutation matmuls
        psWg = ps_d2.tile([128, 256], F32, tag="ps_d2")
        nc.tensor.matmul(psWg[:, 0:128], ct['mh0'][:],
                         idxall[:, g * 128:(g + 1) * 128], start=True, stop=True)
        nc.tensor.matmul(psWg[:, 128:256], ct['mh1'][:],
                         idxall[:, g * 128:(g + 1) * 128], start=True, stop=True)
        idxwg = idxwp.tile([128, GT * 64], I16, tag="idxwg")
        ivw = idxwg[:].rearrange("p (T k) -> p T k", k=64)
        nc.scalar.activation(ivw[:, :, 0::2],
                             psWg[:, 0:128].rearrange("p (T r) -> p T r", r=32),
                             ACTF.Copy)
        nc.scalar.activation(ivw[:, :, 1::2],
                             psWg[:, 128:256].rearrange("p (T r) -> p T r", r=32),
                             ACTF.Copy)
        return idxwg

    # ---- phase B: gathered-MLP + pooling for query tile i ----
    def emit_mlp(i, gout):
        gv_g = gout.rearrange("p (r q u) -> p r q u", r=32, u=2)[:, :, :, 0]
        gv_v0 = gout.rearrange("p (k u) -> p k u", u=2)[:, 0:32, 1]

        # d = V(center) + qdelta; h1 = relu(G + d)
        dd = mlpp.tile([128, 32], F32, tag="dd")
        nc.vector.tensor_tensor(dd[:], gv_v0, qdB[:, bass.ts(i, 32)], ALU.add)
        h1t = mlpp1.tile([128, 1024], F32, tag="h1t")
        nc.vector.tensor_tensor(
            h1t[:].rearrange("p (r q) -> p r q", q=32), gv_g,
            dd[:].unsqueeze(1).broadcast_to((128, 32, 32)), ALU.add)
        h1 = mlpp.tile([128, 1024], BF16, tag="h1")
        nc.scalar.activation(h1[:], h1t[:], ACTF.Relu)

        # layer 2: per unit uu (K=32 at partition 32*uu)
        h2 = mlpp1.tile([32, 4096], BF16, tag="h2")
        for uu in range(4):
            psL2 = ps_l2.tile([32, 1024], F32, tag="ps_a")
            for n in range(2):
                nc.tensor.matmul(
                    psL2[:, bass.ts(n, 512)],
                    ct['w1t4'][32 * uu:32 * uu + 32, :],
                    h1[32 * uu:32 * uu + 32, bass.ts(n, 512)],
                    start=True, stop=True,
                    tile_position=(32 * uu, 0))
            nc.scalar.activation(h2[:, bass.ts(uu, 1024)], psL2[:], ACTF.Relu,
                                 bias=ct['t1v'][:])

        # layer 3
        h3 = mlph3.tile([128, 4096], BF16, tag="h3")
        for n3 in range(8):
            psL3 = ps_l3.tile([128, 512], F32, tag="ps_b3")
            nc.tensor.matmul(psL3[:], ct['w2t'][:], h2[:, bass.ts(n3, 512)],
                             start=True, stop=True)
            nc.scalar.activation(h3[:, bass.ts(n3, 512)], psL3[:], ACTF.Relu,
                                 bias=ct['t2v'][:])

        # pooling
        S = smallp.tile([128, 128], F32, tag="S")
        h3v = h3[:].rearrange("p (a r q) -> p a r q", a=4, r=32)
        for a in range(4):
            nc.vector.tensor_reduce(
                S[:, bass.ts(a, 32)], h3v[:, a, :, :].transpose([0, 2, 1]),
                mybir.AxisListType.X, ALU.add)

        # beta/gamma rows via PE transpose + broadcast
        ceff = ceffall[:, i:i + 1]
        beta = smallp.tile([128, 1], F32, tag="beta")
        nc.vector.reciprocal(beta[:], ceff)
        gm0 = smallp.tile([128, 1], F32, tag="gm0")
        nc.vector.tensor_scalar(gm0[:], ceff, -1.0, 32.0, ALU.mult, ALU.add)
        gamma = smallp.tile([128, 1], F32, tag="gamma")
        nc.vector.tensor_tensor(gamma[:], gm0[:], beta[:], ALU.mult)
        psBG = ps_d2.tile([1, 256], F32, tag="ps_d2")
        nc.tensor.matmul(psBG[:, 0:128], beta[:], ct['ident'][:],
                         start=True, stop=True)
        nc.tensor.matmul(psBG[:, 128:256], gamma[:], ct['ident'][:],
                         start=True, stop=True)
        bgrow = smallp.tile([1, 256], F32, tag="bgrow")
        nc.vector.tensor_copy(bgrow[:], psBG[:])
        psB = ps_d2.tile([128, 256], F32, tag="ps_d2")
        nc.tensor.matmul(psB[:], ct['onesk1'][:], bgrow[:], start=True, stop=True)

        e1 = smallp.tile([128, 128], F32, tag="e1")
        nc.vector.tensor_tensor(e1[:], S[:], psB[:, 0:128], ALU.mult)
        e2 = smallp.tile([128, 128], F32, tag="e2")
        nc.vector.tensor_tensor(
            e2[:].rearrange("p (a q) -> p a q", a=4), h3v[:, :, 0, :],
            psB[:, 128:256].rearrange("p (a q) -> p a q", a=4), ALU.mult)
        nc.vector.tensor_tensor(outbuf[:, bass.ts(i, 128)], e1[:], e2[:],
                                ALU.subtract)

    # ==== software-pipelined groups: gather(g) | mlp(g) | select(g+1) ====
    gvtg = load_gvt(0)
    idxwg = emit_group_A(0)
    for g in range(NGRP):
        goutg = mlpp.tile([128, GT * 2048], BF16, tag="goutg")
        nc.gpsimd.ap_gather(goutg[:].rearrange("p (k u) -> p k u", u=2),
                            gvtg[:].rearrange("p (j u) -> p j u", u=2),
                            idxwg[:], 128, GT * SUB, 2, GT * 1024)
        if g + 1 < NGRP:
            gvtg = load_gvt(g + 1)
        for t in range(GT):
            emit_mlp(g * GT + t, goutg[:, t * 2048:(t + 1) * 2048])
        if g + 1 < NGRP:
            idxwg = emit_group_A(g + 1)
        nc.sync.dma_start(out=out_ap[:, g * GT * 128:(g + 1) * GT * 128],
                          in_=outbuf[:, g * GT * 128:(g + 1) * GT * 128])


# ==========================================================================
# harness entry point: kernel(**inputs) -> full output [2, 128, 8192]
# ==========================================================================

_CACHE = {}


def _build_nc():
    import concourse.bacc as bacc
    import concourse.tile as tile_mod
    nc = bacc.Bacc("TRN2", target_bir_lowering=False, debug=False, num_devices=8)
    in_tiles = {}
    for name, (shape, dt) in IN_SPECS.items():
        in_tiles[name] = nc.dram_tensor(
            name, list(shape), dt, kind="ExternalInput").ap()
    out_tile = nc.dram_tensor("out", (128, NQ), F32, kind="ExternalOutput").ap()
    with tile_mod.TileContext(nc) as t:
        build_kernel(t, out_tile, in_tiles)
    nc.compile()
    return nc


def kernel(**inputs):
    from concourse.bass_utils import run_bass_kernel_spmd
    in_maps = host_prep(inputs)
    if "nc" not in _CACHE:
        _CACHE["nc"] = _build_nc()
    res = run_bass_kernel_spmd(_CACHE["nc"], in_maps, list(range(8)))
    return host_finish(res.results)


# revision 45
# speedup vs baseline: 1.7535x; 1.3324x over previous
"""Trainium2 Bass kernel for nn_PointWiseMLP (ball query + gather + MLP + pool).

Self-contained: kernel(**inputs) shards across 8 NeuronCores (data-parallel
over batch x query-range), runs the Bass/Tile kernel via run_bass_kernel_spmd,
and gathers the full [2, 128, 8192] output.

v3: - support compacted by support_mask on host (order-preserving) and
      k-d-tree query reordering so each 128-query tile is spatially compact;
      each tile only tests the support points inside its radius-expanded bbox
      (<= SUB=1536 of 8192), cutting ball-query work ~5x.
    - per-tile (G,V) pair tables are built on host in tile-local index space
      and DMA'd per group, so the selection pipeline's local indices feed the
      gather directly (no index translation anywhere).
    - d2 decomposed into 15 fp16 rows (4x faster on PE than f32, ~1e-6 exact).
    - selection post-processing batched per 4-tile group, scan values in bf16,
      scatter-source conversions on the (idle) gpsimd engine.
"""
import sys
for _p in ("/opt/trn_rl_repo", "/root/.axon_site/_ro/trn_rl_repo"):
    if _p not in sys.path:
        sys.path.append(_p)


import numpy as np
from contextlib import ExitStack

import concourse.bass as bass
import concourse.tile as tile
from concourse import mybir
from concourse._compat import with_exitstack

F32 = mybir.dt.float32
F16 = mybir.dt.float16
BF16 = mybir.dt.bfloat16
I16 = mybir.dt.int16

RADIUS = 0.1
NSAMPLE = 32
EPS = 1e-5
N2C = 4608         # compacted+padded support count (host-side bound)
SUB = 1536         # per-tile support subset budget
WORDS = SUB // 8   # 192
PAIRS = SUB // 16  # 96
NQ = 2048          # queries per core
NQT = 16           # query tiles per core
GT = 4             # query tiles per gather group
NGRP = NQT // GT   # 4
BIG = 1024.0       # exactly representable in fp16
R2 = float(np.float32(0.01))  # threshold as f32
DROWS = 15         # fp16 d2 decomposition rows
CHUNKS = [(0, 512), (512, 512), (1024, 512)]

ALU = mybir.AluOpType
ACTF = mybir.ActivationFunctionType


# --------------------------------------------------------------------------
# host-side preparation
# --------------------------------------------------------------------------

def _split_hilo(x, grid=1024.0):
    """Grid split: x = hi + lo with hi on 1/grid grid (exact in fp16 for the
    value ranges used here)."""
    x = x.astype(np.float32)
    hi = np.floor(x.astype(np.float64) * grid) / grid
    hi = hi.astype(np.float32)
    lo = (x - hi).astype(np.float32)
    return hi, lo


def _kd_leaves(pts, idx, depth):
    if depth == 0:
        return [idx]
    ext = pts[idx].max(0) - pts[idx].min(0)
    ax = int(np.argmax(ext))
    order = idx[np.argsort(pts[idx, ax], kind="stable")]
    h = len(order) // 2
    return (_kd_leaves(pts, order[:h], depth - 1)
            + _kd_leaves(pts, order[h:], depth - 1))


_PERMS = {}


def host_prep(inputs):
    B = 2
    qx = np.asarray(inputs['query_xyz'], np.float32)
    sx = np.asarray(inputs['support_xyz'], np.float32)
    qm = np.asarray(inputs['query_mask'], np.int32)
    sm = np.asarray(inputs['support_mask'], np.int32)
    sf = np.asarray(inputs['support_features'], np.float32)

    W0 = np.asarray(inputs['W0'], np.float64)
    W1 = np.asarray(inputs['W1'], np.float64)
    W2 = np.asarray(inputs['W2'], np.float64)

    def fold(Wl, g, b, rm, rv):
        s = np.asarray(g, np.float64) / np.sqrt(np.asarray(rv, np.float64) + EPS)
        return Wl * s[:, None], np.asarray(b, np.float64) - np.asarray(rm, np.float64) * s

    W0p, t0 = fold(W0, inputs['g0'], inputs['b0'], inputs['rm0'], inputs['rv0'])
    W1p, t1 = fold(W1, inputs['g1'], inputs['b1'], inputs['rm1'], inputs['rv1'])
    W2p, t2 = fold(W2, inputs['g2'], inputs['b2'], inputs['rm2'], inputs['rv2'])

    P0 = W0p[:, 0:3] / RADIUS
    C0 = W0p[:, 3:67]
    D0 = W0p[:, 67:131]

    w1t4 = np.tile(W1p.T.astype(np.float32), (4, 1))       # [128, 32]
    w2t = W2p.T.astype(np.float32)                         # [32, 128]
    t1v = t1.astype(np.float32).reshape(32, 1)
    t2v = t2.astype(np.float32).reshape(128, 1)

    # permutation matmul weights for the wrapped gather index layout:
    # idxw[p, 2r+h] = idxg[32*(p//32) + 16h + p%16, r]
    Mh = np.zeros((2, 128, 128), np.float32)
    for h in range(2):
        for p in range(128):
            Mh[h, 32 * (p // 32) + 16 * h + p % 16, p] = 1.0
    ident = np.eye(128, dtype=np.float32)

    pow8 = np.tile((2.0 ** (np.arange(1024) % 8)).astype(np.float32)[None, :], (128, 1))
    # scatter id source, pre-scaled by 16 so cjp1 = dstID + tpat16 yields
    # (slot*SUB + local_idx + 1) directly
    iotag = np.tile(((np.arange(GT * PAIRS, dtype=np.int16) + 1) * 16)[None, :],
                    (128, 1))
    shv = np.tile(np.arange(8, dtype=np.int16)[None, :], (128, 1))
    tpat16 = np.tile((np.tile(np.arange(16, dtype=np.int16), 34) - 15)[None, :],
                     (128, 1))
    onesk1 = np.ones((1, 128), np.float32)

    batch_sup = []
    for b in range(B):
        # order-preserving compaction by support_mask; original point 0 is
        # always table entry 0 (selection-masked if its mask is 0) so the
        # zero-neighbor fill gathers the same point the reference does.
        valid = sm[b] > 0
        keep = np.nonzero(valid)[0]
        sel0 = True
        if not valid[0]:
            keep = np.concatenate([[0], keep])
            sel0 = False
        nv = len(keep)
        assert nv <= N2C, (nv, N2C)
        s = np.zeros((N2C, 3), np.float32)
        s[:nv] = sx[b][keep]
        fts = np.zeros((64, N2C), np.float32)
        fts[:, :nv] = sf[b][:, keep]
        selmask = np.zeros(N2C, np.float32)
        selmask[:nv] = 1.0
        if not sel0:
            selmask[0] = 0.0

        # fp16 d2 decomposition (support side), global-compacted columns
        sh, sl = _split_hilo(s)
        s64, sh64 = s.astype(np.float64), sh.astype(np.float64)
        Ls = (np.sum(s64 * s64, 1) - np.sum(sh64 * sh64, 1)).astype(np.float32)
        sh2 = np.sum(sh64 * sh64, 1).astype(np.float32)
        hi_s, lo_s = _split_hilo(sh2, 512.0)
        rhsg = np.zeros((DROWS, N2C), np.float32)
        rhsg[0:3] = sh.T
        rhsg[3:6] = -2.0 * sh.T
        rhsg[6:9] = -2.0 * sl.T
        rhsg[9] = 1.0
        rhsg[10] = 1.0
        rhsg[11] = 1.0
        rhsg[12] = hi_s
        rhsg[13] = lo_s + Ls
        rhsg[14] = BIG * (1.0 - selmask)

        # (G,V) pair table in global-compacted index space
        G = D0 @ fts.astype(np.float64) + (P0 @ s.T.astype(np.float64))
        V = (C0 - D0) @ fts.astype(np.float64)
        gvpair = np.empty((32, 2 * N2C), np.float32)
        gvpair[:, 0::2] = G.astype(np.float32)
        gvpair[:, 1::2] = V.astype(np.float32)
        batch_sup.append((rhsg, gvpair, s[:nv], nv))

    import ml_dtypes
    npdt = {F32: np.float32, F16: np.float16, BF16: ml_dtypes.bfloat16,
            I16: np.int16}
    in_maps = []
    for c in range(8):
        b = c // 4
        q0 = (c % 4) * NQ
        rhsg, gvpair, s_c, nv = batch_sup[b]

        # k-d reorder queries so each 128-tile is spatially compact
        qraw = qx[b, q0:q0 + NQ]
        perm = np.concatenate(_kd_leaves(qraw, np.arange(NQ), 4))
        _PERMS[c] = perm
        q = qraw[perm]
        qmk = qm[b, q0:q0 + NQ].astype(np.float32)[perm]

        qh, ql = _split_hilo(q)
        q64, qh64 = q.astype(np.float64), qh.astype(np.float64)
        Lq = (np.sum(q64 * q64, 1) - np.sum(qh64 * qh64, 1)).astype(np.float32)
        qh2 = np.sum(qh64 * qh64, 1).astype(np.float32)
        hi_q, lo_q = _split_hilo(qh2, 512.0)
        lhsq = np.zeros((DROWS, NQ), np.float32)
        lhsq[0:3] = -2.0 * qh.T
        lhsq[3:6] = ql.T
        lhsq[6:9] = q.T
        lhsq[9] = hi_q
        lhsq[10] = lo_q + Lq
        lhsq[11] = BIG * (1 - qmk)
        lhsq[12] = 1.0
        lhsq[13] = 1.0
        lhsq[14] = 1.0

        # per-tile support subsets (within bbox + RADIUS), tile-local tables
        rhsd2t = np.zeros((DROWS, NQT * SUB), np.float32)
        rhsd2t[14] = BIG
        gvt = np.zeros((4, 32, 2 * GT * SUB), np.float32)   # per group
        for i in range(NQT):
            qt = q[i * 128:(i + 1) * 128]
            lo = qt.min(0) - RADIUS
            hi = qt.max(0) + RADIUS
            subs = np.nonzero(np.all((s_c >= lo) & (s_c <= hi), axis=1))[0]
            if len(subs) == 0 or subs[0] != 0:
                subs = np.concatenate([[0], subs])   # fill fallback -> entry 0
            ns = len(subs)
            assert ns <= SUB, (ns, SUB)
            rhsd2t[:, i * SUB:i * SUB + ns] = rhsg[:, subs]
            g_, t_ = i // GT, i % GT
            pcols = (2 * subs[:, None] + np.arange(2)[None, :]).ravel()
            gvt[g_, :, 2 * t_ * SUB:2 * t_ * SUB + 2 * ns] = gvpair[:, pcols]

        # qdB[32g + u, i*32 + q'] = t0[u] - P0 @ q(i*128 + 32g + q')
        P0q = (P0 @ q.T.astype(np.float64)).reshape(32, NQT, 4, 32)
        qdB = np.zeros((128, 512), np.float64)
        for g in range(4):
            qdB[32 * g:32 * g + 32, :] = (
                t0[:, None] - P0q[:, :, g, :].reshape(32, NQT * 32))

        im = dict(
            lhsq=lhsq, rhsd2t=rhsd2t, qdB=qdB,
            gvt0=np.tile(gvt[0], (4, 1)), gvt1=np.tile(gvt[1], (4, 1)),
            gvt2=np.tile(gvt[2], (4, 1)), gvt3=np.tile(gvt[3], (4, 1)),
            t1v=t1v, t2v=t2v,
            w1t4=w1t4, w2t=w2t,
            mh0=Mh[0], mh1=Mh[1], ident=ident,
            pow8=pow8, iotag=iotag, shv=shv, tpat16=tpat16,
            qfm=qmk.reshape(NQT, 128).T.copy(),
            onesk1=onesk1,
        )
        for k in im:
            shape, dt = IN_SPECS[k]
            arr = np.ascontiguousarray(im[k]).astype(npdt[dt])
            assert arr.shape == shape, (k, arr.shape, shape)
            im[k] = arr
        in_maps.append(im)
    return in_maps


def host_finish(results):
    out = np.zeros((2, 128, 8192), np.float32)
    for c in range(8):
        b = c // 4
        q0 = (c % 4) * NQ
        out[b][:, q0 + _PERMS[c]] = results[c]['out']
    return out


IN_SPECS = dict(
    lhsq=((DROWS, NQ), F16), rhsd2t=((DROWS, NQT * SUB), F16),
    qdB=((128, 512), F32),
    gvt0=((128, 2 * GT * SUB), BF16), gvt1=((128, 2 * GT * SUB), BF16),
    gvt2=((128, 2 * GT * SUB), BF16), gvt3=((128, 2 * GT * SUB), BF16),
    t1v=((32, 1), F32), t2v=((128, 1), F32),
    w1t4=((128, 32), BF16), w2t=((32, 128), BF16),
    mh0=((128, 128), F32), mh1=((128, 128), F32), ident=((128, 128), F32),
    pow8=((128, 1024), BF16), iotag=((128, GT * PAIRS), I16), shv=((128, 8), I16),
    tpat16=((128, 544), I16), qfm=((128, NQT), F32), onesk1=((1, 128), F32),
)


# --------------------------------------------------------------------------
# device kernel
# --------------------------------------------------------------------------

@with_exitstack
def build_kernel(ctx: ExitStack, tc: tile.TileContext, out_ap: bass.AP, ins: dict):
    nc = tc.nc
    ctx.enter_context(nc.allow_low_precision("bf16 mlp + exact small-int sums"))

    consts = ctx.enter_context(tc.tile_pool(name="consts", bufs=1))
    gvp = ctx.enter_context(tc.tile_pool(name="gv", bufs=1))
    selp = ctx.enter_context(tc.tile_pool(name="sel", bufs=2))
    selp1 = ctx.enter_context(tc.tile_pool(name="sel1", bufs=1))
    selp2 = ctx.enter_context(tc.tile_pool(name="sel2", bufs=1))
    smallp = ctx.enter_context(tc.tile_pool(name="small", bufs=1))
    idxwp = ctx.enter_context(tc.tile_pool(name="idxw", bufs=2))
    gvtp = ctx.enter_context(tc.tile_pool(name="gvt", bufs=2))
    mlpp = ctx.enter_context(tc.tile_pool(name="mlp", bufs=2))
    mlpp1 = ctx.enter_context(tc.tile_pool(name="mlp1", bufs=1))
    mlph3 = ctx.enter_context(tc.tile_pool(name="mlph3", bufs=2))
    outp = ctx.enter_context(tc.tile_pool(name="outb", bufs=1))
    ps_d2 = ctx.enter_context(tc.tile_pool(name="psd2", bufs=2, space="PSUM"))
    ps_l2 = ctx.enter_context(tc.tile_pool(name="psl2", bufs=2, space="PSUM"))
    ps_l3 = ctx.enter_context(tc.tile_pool(name="psl3", bufs=2, space="PSUM"))

    GVT_NAMES = ("gvt0", "gvt1", "gvt2", "gvt3")
    ct = {}
    for name, (shape, dt) in IN_SPECS.items():
        if name in GVT_NAMES or name == "rhsd2t":
            continue
        t = consts.tile(list(shape), dt, tag=f"c_{name}")
        nc.sync.dma_start(out=t[:], in_=ins[name])
        ct[name] = t
    qdB = ct['qdB']

    c33 = consts.tile([128, 544], BF16, tag="c33")
    nc.vector.memset(c33[:], 33.0)
    ones34 = consts.tile([128, 34], I16, tag="ones34")
    nc.vector.memset(ones34[:], 1)

    # persistent per-core state
    idxall = gvp.tile([128, 512], F32, tag="idxall")   # final idx per qtile (f32)
    ceffall = gvp.tile([128, NQT], F32, tag="ceffall")
    outbuf = outp.tile([128, NQ], F32, tag="outbuf")

    def load_gvt(g):
        gvtg = gvtp.tile([128, 2 * GT * SUB], BF16, tag="gvtg")
        nc.sync.dma_start(out=gvtg[:], in_=ins[GVT_NAMES[g]])
        return gvtg

    # ---- phase A for a whole group of GT query tiles: per-tile d2 matmuls +
    # mask words, then batched selection post-processing ----
    GP = GT * PAIRS   # 384
    GS = GT * 544     # 2176

    def emit_group_A1(g):
        """d2 matmuls + in-radius mask words for group g (no MLP deps)."""
        rhsg2 = gvtp.tile([DROWS, GT * SUB], F16, tag="rhsg2")
        nc.sync.dma_start(out=rhsg2[:],
                          in_=ins['rhsd2t'][:, g * GT * SUB:(g + 1) * GT * SUB])
        w8g = selp1.tile([128, GT * WORDS], BF16, tag="w8g")
        for t in range(GT):
            i = g * GT + t
            for (off, csz) in CHUNKS:
                pd2 = ps_d2.tile([128, 512], F32, tag="ps_d2")
                nc.tensor.matmul(
                    pd2[:], ct['lhsq'][:, bass.ts(i, 128)],
                    rhsg2[:, t * SUB + off:t * SUB + off + csz],
                    start=True, stop=True)
                vw8c = selp.tile([128, 512], BF16, tag="vw8c")
                nc.vector.scalar_tensor_tensor(
                    vw8c[:], pd2[:], R2, ct['pow8'][:, 0:512],
                    op0=ALU.is_lt, op1=ALU.mult)
                nc.vector.tensor_reduce(
                    w8g[:, t * WORDS + off // 8:t * WORDS + (off + csz) // 8],
                    vw8c[:].rearrange("p (w t) -> p w t", t=8),
                    mybir.AxisListType.X, ALU.add)
        return w8g

    def emit_group_A2(g, w8g):
        """batched selection post-processing -> wrapped gather indices."""
        w8v = w8g[:].rearrange("p (c two) -> p c two", two=2)   # c = GP
        # scatter sources converted on the Pool engine (same queue as the
        # scatters that consume them -- no cross-engine hop, Pool is idle)
        w8e = selp2.tile([128, GP], I16, tag="w8e")
        w8o = selp2.tile([128, GP], I16, tag="w8o")
        nc.gpsimd.tensor_scalar(w8e[:], w8v[:, :, 0], 0.0, None, ALU.add)
        nc.gpsimd.tensor_scalar(w8o[:], w8v[:, :, 1], 0.0, None, ALU.add)

        nz = smallp.tile([128, GP], BF16, tag="nz")
        nc.vector.tensor_tensor(nz[:], w8v[:, :, 0], w8v[:, :, 1],
                                ALU.logical_or)
        crank = smallp.tile([128, GP], BF16, tag="crank")
        for t in range(GT):
            nc.vector.tensor_tensor_scan(
                crank[:, t * PAIRS:(t + 1) * PAIRS],
                nz[:, t * PAIRS:(t + 1) * PAIRS], c33[:, 0:PAIRS], 0.0,
                ALU.add, ALU.min)
        u = smallp.tile([128, GP], BF16, tag="u")
        nc.vector.tensor_tensor(u[:], crank[:], nz[:], ALU.mult)
        v = smallp.tile([128, GP], BF16, tag="nz")
        nc.vector.scalar_tensor_tensor(v[:], u[:], 32.5, u[:], op0=ALU.is_le,
                                       op1=ALU.mult)
        si16 = selp2.tile([128, GP], I16, tag="si16")
        nc.vector.tensor_scalar(si16[:], v[:], -1.0, None, ALU.add)

        dstID = selp2.tile([128, GT * 34], I16, tag="dstID")
        dstWe = selp2.tile([128, GT * 34], I16, tag="dstWe")
        dstWo = selp2.tile([128, GT * 34], I16, tag="dstWo")
        for t in range(GT):
            sl_ = slice(t * PAIRS, (t + 1) * PAIRS)
            ds_ = slice(t * 34, (t + 1) * 34)
            nc.gpsimd.local_scatter(dstID[:, ds_], ct['iotag'][:, sl_],
                                    si16[:, sl_], 128, 34, PAIRS)
            nc.gpsimd.local_scatter(dstWe[:, ds_], w8e[:, sl_], si16[:, sl_],
                                    128, 34, PAIRS)
            nc.gpsimd.local_scatter(dstWo[:, ds_], w8o[:, sl_], si16[:, sl_],
                                    128, 34, PAIRS)

        esel16 = selp2.tile([128, GS], I16, tag="esel16")
        # per tile: esel col s*16 + b*8 + u  <-  bit u of dstW(b) col s
        evb = esel16[:].rearrange("p (T s b u) -> p b u T s", s=34, b=2, u=8)
        onesT34 = ones34[:].unsqueeze(1).broadcast_to((128, GT, 34))
        for bidx, dstWx in ((0, dstWe), (1, dstWo)):
            dwv = dstWx[:].rearrange("p (T s) -> p T s", s=34)
            for t in range(8):
                nc.vector.scalar_tensor_tensor(evb[:, bidx, t], dwv,
                                               ct['shv'][:, t:t + 1], onesT34,
                                               op0=ALU.logical_shift_right,
                                               op1=ALU.bitwise_and)
        esel = smallp.tile([128, GS], BF16, tag="esel")
        nc.gpsimd.tensor_scalar(esel[:], esel16[:], 0.0, None, ALU.add)
        cjp1 = selp2.tile([128, GS], I16, tag="cjp1")
        nc.vector.tensor_tensor(
            cjp1[:].rearrange("p (T s u) -> p T s u", s=34, u=16),
            dstID[:].rearrange("p (T s) -> p T s", s=34)
                .unsqueeze(3).broadcast_to((128, GT, 34, 16)),
            ct['tpat16'][:].rearrange("p (s u) -> p s u", u=16)
                .unsqueeze(1).broadcast_to((128, GT, 34, 16)),
            ALU.add)

        crank2 = smallp.tile([128, GS], BF16, tag="crank2")
        for t in range(GT):
            nc.vector.tensor_tensor_scan(
                crank2[:, t * 544:(t + 1) * 544],
                esel[:, t * 544:(t + 1) * 544], c33[:], 0.0, ALU.add, ALU.min)
        # effective count (with query-mask fallback to 32)
        cnt0 = smallp.tile([128, GT], F32, tag="cnt0")
        nc.vector.tensor_scalar(
            cnt0[:], crank2[:].rearrange("p (T x) -> p T x", x=544)[:, :, 543],
            32.0, None, ALU.min)
        qfc = smallp.tile([128, GT], F32, tag="qfc")
        nc.vector.tensor_scalar(qfc[:], ct['qfm'][:, g * GT:(g + 1) * GT],
                                -32.0, 32.0, ALU.mult, ALU.add)
        nc.vector.tensor_tensor(ceffall[:, g * GT:(g + 1) * GT], cnt0[:],
                                qfc[:], ALU.max)
        u2 = smallp.tile([128, GS], BF16, tag="u2")
        nc.vector.tensor_tensor(u2[:], crank2[:], esel[:], ALU.mult)
        v2 = smallp.tile([128, GS], BF16, tag="esel")
        nc.vector.scalar_tensor_tensor(v2[:], u2[:], 32.5, u2[:], op0=ALU.is_le,
                                       op1=ALU.mult)
        si2 = selp2.tile([128, GS], I16, tag="si2")
        nc.vector.tensor_scalar(si2[:], v2[:], -1.0, None, ALU.add)
        idxp1 = selp2.tile([128, GT * 34], I16, tag="idxp1")
        for t in range(GT):
            nc.gpsimd.local_scatter(idxp1[:, t * 34:(t + 1) * 34],
                                    cjp1[:, t * 544:(t + 1) * 544],
                                    si2[:, t * 544:(t + 1) * 544], 128, 34, 544)

        # fill + final gather indices (kept in f32 for the phase-B matmul);
        # idx values are slot*SUB + local; local fallback 0 = global point 0
        ii = smallp.tile([128, GT * 32], F32, tag="ii")
        nc.gpsimd.tensor_scalar(
            ii[:].rearrange("p (T r) -> p T r", r=32),
            idxp1[:].rearrange("p (T s) -> p T s", s=34)[:, :, 0:32],
            0.0, None, ALU.add)
        iv = ii[:].rearrange("p (T r) -> p T r", r=32)
        flp1 = smallp.tile([128, GT], F32, tag="flp1")
        nc.vector.tensor_scalar(flp1[:], iv[:, :, 0], 1.0, None, ALU.max)
        flb = flp1[:].unsqueeze(2).broadcast_to((128, GT, 32))
        m = smallp.tile([128, GT * 32], F32, tag="m")
        nc.vector.tensor_scalar(m[:], ii[:], 0.0, None, ALU.is_gt)
        bb = smallp.tile([128, GT * 32], F32, tag="bb")
        nc.vector.tensor_tensor(bb[:], ii[:], m[:], ALU.mult)
        aa = smallp.tile([128, GT * 32], F32, tag="aa")
        nc.vector.tensor_tensor(aa[:].rearrange("p (T r) -> p T r", r=32),
                                m[:].rearrange("p (T r) -> p T r", r=32), flb,
                                ALU.mult)
        cc = smallp.tile([128, GT * 32], F32, tag="m")
        nc.vector.tensor_tensor(cc[:], bb[:], aa[:], ALU.subtract)
        dd0 = smallp.tile([128, GT * 32], F32, tag="bb")
        nc.vector.tensor_tensor(dd0[:].rearrange("p (T r) -> p T r", r=32),
                                cc[:].rearrange("p (T r) -> p T r", r=32), flb,
                                ALU.add)
        nc.vector.tensor_scalar(idxall[:, g * GT * 32:(g + 1) * GT * 32],
                                dd0[:], -1.0, None, ALU.add)

        # wrapped gather index slots via two group-wide permutation matmuls
        psWg = ps_d2.tile([128, 256], F32, tag="ps_d2")
        nc.tensor.matmul(psWg[:, 0:128], ct['mh0'][:],
                         idxall[:, g * 128:(g + 1) * 128], start=True, stop=True)
        nc.tensor.matmul(psWg[:, 128:256], ct['mh1'][:],
                         idxall[:, g * 128:(g + 1) * 128], start=True, stop=True)
        idxwg = idxwp.tile([128, GT * 64], I16, tag="idxwg")
        ivw = idxwg[:].rearrange("p (T k) -> p T k", k=64)
        nc.scalar.activation(ivw[:, :, 0::2],
                             psWg[:, 0:128].rearrange("p (T r) -> p T r", r=32),
                             ACTF.Copy)
        nc.scalar.activation(ivw[:, :, 1::2],
                             psWg[:, 128:256].rearrange("p (T r) -> p T r", r=32),
                             ACTF.Copy)
        return idxwg

    # ---- phase B: gathered-MLP + pooling for query tile i ----
    def emit_mlp(i, gout):
        gv_g = gout.rearrange("p (r q u) -> p r q u", r=32, u=2)[:, :, :, 0]
        gv_v0 = gout.rearrange("p (k u) -> p k u", u=2)[:, 0:32, 1]

        # d = V(center) + qdelta; h1 = relu(G + d)
        dd = mlpp.tile([128, 32], F32, tag="dd")
        nc.vector.tensor_tensor(dd[:], gv_v0, qdB[:, bass.ts(i, 32)], ALU.add)
        h1t = mlpp1.tile([128, 1024], F32, tag="h1t")
        nc.vector.tensor_tensor(
            h1t[:].rearrange("p (r q) -> p r q", q=32), gv_g,
            dd[:].unsqueeze(1).broadcast_to((128, 32, 32)), ALU.add)
        h1 = mlpp.tile([128, 1024], BF16, tag="h1")
        nc.scalar.activation(h1[:], h1t[:], ACTF.Relu)

        # layer 2: per unit uu (K=32 at partition 32*uu)
        h2 = mlpp1.tile([32, 4096], BF16, tag="h2")
        for uu in range(4):
            psL2 = ps_l2.tile([32, 1024], F32, tag="ps_a")
            for n in range(2):
                nc.tensor.matmul(
                    psL2[:, bass.ts(n, 512)],
                    ct['w1t4'][32 * uu:32 * uu + 32, :],
                    h1[32 * uu:32 * uu + 32, bass.ts(n, 512)],
                    start=True, stop=True,
                    tile_position=(32 * uu, 0))
            nc.scalar.activation(h2[:, bass.ts(uu, 1024)], psL2[:], ACTF.Relu,
                                 bias=ct['t1v'][:])

        # layer 3
        h3 = mlph3.tile([128, 4096], BF16, tag="h3")
        for n3 in range(8):
            psL3 = ps_l3.tile([128, 512], F32, tag="ps_b3")
            nc.tensor.matmul(psL3[:], ct['w2t'][:], h2[:, bass.ts(n3, 512)],
                             start=True, stop=True)
            nc.scalar.activation(h3[:, bass.ts(n3, 512)], psL3[:], ACTF.Relu,
                                 bias=ct['t2v'][:])

        # pooling
        S = smallp.tile([128, 128], F32, tag="S")
        h3v = h3[:].rearrange("p (a r q) -> p a r q", a=4, r=32)
        for a in range(4):
            nc.vector.tensor_reduce(
                S[:, bass.ts(a, 32)], h3v[:, a, :, :].transpose([0, 2, 1]),
                mybir.AxisListType.X, ALU.add)

        # beta/gamma rows via PE transpose + broadcast
        ceff = ceffall[:, i:i + 1]
        beta = smallp.tile([128, 1], F32, tag="beta")
        nc.vector.reciprocal(beta[:], ceff)
        gm0 = smallp.tile([128, 1], F32, tag="gm0")
        nc.vector.tensor_scalar(gm0[:], ceff, -1.0, 32.0, ALU.mult, ALU.add)
        gamma = smallp.tile([128, 1], F32, tag="gamma")
        nc.vector.tensor_tensor(gamma[:], gm0[:], beta[:], ALU.mult)
        psBG = ps_d2.tile([1, 256], F32, tag="ps_d2")
        nc.tensor.matmul(psBG[:, 0:128], beta[:], ct['ident'][:],
                         start=True, stop=True)
        nc.tensor.matmul(psBG[:, 128:256], gamma[:], ct['ident'][:],
                         start=True, stop=True)
        bgrow = smallp.tile([1, 256], F32, tag="bgrow")
        nc.vector.tensor_copy(bgrow[:], psBG[:])
        psB = ps_d2.tile([128, 256], F32, tag="ps_d2")
        nc.tensor.matmul(psB[:], ct['onesk1'][:], bgrow[:], start=True, stop=True)

        e1 = smallp.tile([128, 128], F32, tag="e1")
        nc.vector.tensor_tensor(e1[:], S[:], psB[:, 0:128], ALU.mult)
        e2 = smallp.tile([128, 128], F32, tag="e2")
        nc.vector.tensor_tensor(
            e2[:].rearrange("p (a q) -> p a q", a=4), h3v[:, :, 0, :],
            psB[:, 128:256].rearrange("p (a q) -> p a q", a=4), ALU.mult)
        nc.vector.tensor_tensor(outbuf[:, bass.ts(i, 128)], e1[:], e2[:],
                                ALU.subtract)

    # ==== software-pipelined groups ====
    # engine-queue order per group g: gather(g) runs while DVE chews the
    # independent A1(g+1) mask work; mlp(g) then overlaps the A2(g+1)
    # selection chain.
    gvtg = load_gvt(0)
    idxwg = emit_group_A2(0, emit_group_A1(0))
    for g in range(NGRP):
        goutg = mlpp.tile([128, GT * 2048], BF16, tag="goutg")
        nc.gpsimd.ap_gather(goutg[:].rearrange("p (k u) -> p k u", u=2),
                            gvtg[:].rearrange("p (j u) -> p j u", u=2),
                            idxwg[:], 128, GT * SUB, 2, GT * 1024)
        if g + 1 < NGRP:
            gvtg = load_gvt(g + 1)
            w8gn = emit_group_A1(g + 1)
        for t in range(GT):
            emit_mlp(g * GT + t, goutg[:, t * 2048:(t + 1) * 2048])
        if g + 1 < NGRP:
            idxwg = emit_group_A2(g + 1, w8gn)
        nc.sync.dma_start(out=out_ap[:, g * GT * 128:(g + 1) * GT * 128],
                          in_=outbuf[:, g * GT * 128:(g + 1) * GT * 128])


# ==========================================================================
# harness entry point: kernel(**inputs) -> full output [2, 128, 8192]
# ==========================================================================

_CACHE = {}


def _build_nc():
    import concourse.bacc as bacc
    import concourse.tile as tile_mod
    nc = bacc.Bacc("TRN2", target_bir_lowering=False, debug=False, num_devices=8)
    in_tiles = {}
    for name, (shape, dt) in IN_SPECS.items():
        in_tiles[name] = nc.dram_tensor(
            name, list(shape), dt, kind="ExternalInput").ap()
    out_tile = nc.dram_tensor("out", (128, NQ), F32, kind="ExternalOutput").ap()
    with tile_mod.TileContext(nc) as t:
        build_kernel(t, out_tile, in_tiles)
    nc.compile()
    return nc


def kernel(**inputs):
    from concourse.bass_utils import run_bass_kernel_spmd
    in_maps = host_prep(inputs)
    if "nc" not in _CACHE:
        _CACHE["nc"] = _build_nc()
    res = run_bass_kernel_spmd(_CACHE["nc"], in_maps, list(range(8)))
    return host_finish(res.results)


# revision 46
# speedup vs baseline: 1.7707x; 1.0098x over previous
"""Trainium2 Bass kernel for nn_PointWiseMLP (ball query + gather + MLP + pool).

Self-contained: kernel(**inputs) shards across 8 NeuronCores (data-parallel
over batch x query-range), runs the Bass/Tile kernel via run_bass_kernel_spmd,
and gathers the full [2, 128, 8192] output.

v3: - support compacted by support_mask on host (order-preserving) and
      k-d-tree query reordering so each 128-query tile is spatially compact;
      each tile only tests the support points inside its radius-expanded bbox
      (<= SUB=1536 of 8192), cutting ball-query work ~5x.
    - per-tile (G,V) pair tables are built on host in tile-local index space
      and DMA'd per group, so the selection pipeline's local indices feed the
      gather directly (no index translation anywhere).
    - d2 decomposed into 15 fp16 rows (4x faster on PE than f32, ~1e-6 exact).
    - selection post-processing batched per 4-tile group, scan values in bf16,
      scatter-source conversions on the (idle) gpsimd engine.
"""
import sys
for _p in ("/opt/trn_rl_repo", "/root/.axon_site/_ro/trn_rl_repo"):
    if _p not in sys.path:
        sys.path.append(_p)


import numpy as np
from contextlib import ExitStack

import concourse.bass as bass
import concourse.tile as tile
from concourse import mybir
from concourse._compat import with_exitstack

F32 = mybir.dt.float32
F16 = mybir.dt.float16
BF16 = mybir.dt.bfloat16
I16 = mybir.dt.int16

RADIUS = 0.1
NSAMPLE = 32
EPS = 1e-5
N2C = 4608         # compacted+padded support count (host-side bound)
SUB = 1536         # per-tile support subset budget
WORDS = SUB // 8   # 192
PAIRS = SUB // 16  # 96
NQ = 2048          # queries per core
NQT = 16           # query tiles per core
GT = 4             # query tiles per gather group
NGRP = NQT // GT   # 4
BIG = 1024.0       # exactly representable in fp16
R2 = float(np.float32(0.01))  # threshold as f32
DROWS = 15         # fp16 d2 decomposition rows
CHUNKS = [(0, 512), (512, 512), (1024, 512)]

ALU = mybir.AluOpType
ACTF = mybir.ActivationFunctionType


# --------------------------------------------------------------------------
# host-side preparation
# --------------------------------------------------------------------------

def _split_hilo(x, grid=1024.0):
    """Grid split: x = hi + lo with hi on 1/grid grid (exact in fp16 for the
    value ranges used here)."""
    x = x.astype(np.float32)
    hi = np.floor(x.astype(np.float64) * grid) / grid
    hi = hi.astype(np.float32)
    lo = (x - hi).astype(np.float32)
    return hi, lo


def _kd_leaves(pts, idx, depth):
    if depth == 0:
        return [idx]
    ext = pts[idx].max(0) - pts[idx].min(0)
    ax = int(np.argmax(ext))
    order = idx[np.argsort(pts[idx, ax], kind="stable")]
    h = len(order) // 2
    return (_kd_leaves(pts, order[:h], depth - 1)
            + _kd_leaves(pts, order[h:], depth - 1))


_PERMS = {}


def host_prep(inputs):
    B = 2
    qx = np.asarray(inputs['query_xyz'], np.float32)
    sx = np.asarray(inputs['support_xyz'], np.float32)
    qm = np.asarray(inputs['query_mask'], np.int32)
    sm = np.asarray(inputs['support_mask'], np.int32)
    sf = np.asarray(inputs['support_features'], np.float32)

    W0 = np.asarray(inputs['W0'], np.float64)
    W1 = np.asarray(inputs['W1'], np.float64)
    W2 = np.asarray(inputs['W2'], np.float64)

    def fold(Wl, g, b, rm, rv):
        s = np.asarray(g, np.float64) / np.sqrt(np.asarray(rv, np.float64) + EPS)
        return Wl * s[:, None], np.asarray(b, np.float64) - np.asarray(rm, np.float64) * s

    W0p, t0 = fold(W0, inputs['g0'], inputs['b0'], inputs['rm0'], inputs['rv0'])
    W1p, t1 = fold(W1, inputs['g1'], inputs['b1'], inputs['rm1'], inputs['rv1'])
    W2p, t2 = fold(W2, inputs['g2'], inputs['b2'], inputs['rm2'], inputs['rv2'])

    P0 = W0p[:, 0:3] / RADIUS
    C0 = W0p[:, 3:67]
    D0 = W0p[:, 67:131]

    w1t4 = np.tile(W1p.T.astype(np.float32), (4, 1))       # [128, 32]
    w2t = W2p.T.astype(np.float32)                         # [32, 128]
    t1v = t1.astype(np.float32).reshape(32, 1)
    t2v = t2.astype(np.float32).reshape(128, 1)

    # permutation matmul weights for the wrapped gather index layout:
    # idxw[p, 2r+h] = idxg[32*(p//32) + 16h + p%16, r]
    Mh = np.zeros((2, 128, 128), np.float32)
    for h in range(2):
        for p in range(128):
            Mh[h, 32 * (p // 32) + 16 * h + p % 16, p] = 1.0
    ident = np.eye(128, dtype=np.float32)

    pow8 = np.tile((2.0 ** (np.arange(1024) % 8)).astype(np.float32)[None, :], (128, 1))
    # scatter id source, pre-scaled by 16 so cjp1 = dstID + tpat16 yields
    # (slot*SUB + local_idx + 1) directly
    iotag = np.tile(((np.arange(GT * PAIRS, dtype=np.int16) + 1) * 16)[None, :],
                    (128, 1))
    shv = np.tile(np.arange(8, dtype=np.int16)[None, :], (128, 1))
    tpat16 = np.tile((np.tile(np.arange(16, dtype=np.int16), 34) - 15)[None, :],
                     (128, 1))
    onesk1 = np.ones((1, 128), np.float32)

    batch_sup = []
    for b in range(B):
        # order-preserving compaction by support_mask; original point 0 is
        # always table entry 0 (selection-masked if its mask is 0) so the
        # zero-neighbor fill gathers the same point the reference does.
        valid = sm[b] > 0
        keep = np.nonzero(valid)[0]
        sel0 = True
        if not valid[0]:
            keep = np.concatenate([[0], keep])
            sel0 = False
        nv = len(keep)
        assert nv <= N2C, (nv, N2C)
        s = np.zeros((N2C, 3), np.float32)
        s[:nv] = sx[b][keep]
        fts = np.zeros((64, N2C), np.float32)
        fts[:, :nv] = sf[b][:, keep]
        selmask = np.zeros(N2C, np.float32)
        selmask[:nv] = 1.0
        if not sel0:
            selmask[0] = 0.0

        # fp16 d2 decomposition (support side), global-compacted columns
        sh, sl = _split_hilo(s)
        s64, sh64 = s.astype(np.float64), sh.astype(np.float64)
        Ls = (np.sum(s64 * s64, 1) - np.sum(sh64 * sh64, 1)).astype(np.float32)
        sh2 = np.sum(sh64 * sh64, 1).astype(np.float32)
        hi_s, lo_s = _split_hilo(sh2, 512.0)
        rhsg = np.zeros((DROWS, N2C), np.float32)
        rhsg[0:3] = sh.T
        rhsg[3:6] = -2.0 * sh.T
        rhsg[6:9] = -2.0 * sl.T
        rhsg[9] = 1.0
        rhsg[10] = 1.0
        rhsg[11] = 1.0
        rhsg[12] = hi_s
        rhsg[13] = lo_s + Ls
        rhsg[14] = BIG * (1.0 - selmask)

        # (G,V) pair table in global-compacted index space
        G = D0 @ fts.astype(np.float64) + (P0 @ s.T.astype(np.float64))
        V = (C0 - D0) @ fts.astype(np.float64)
        gvpair = np.empty((32, 2 * N2C), np.float32)
        gvpair[:, 0::2] = G.astype(np.float32)
        gvpair[:, 1::2] = V.astype(np.float32)
        batch_sup.append((rhsg, gvpair, s[:nv], nv))

    import ml_dtypes
    npdt = {F32: np.float32, F16: np.float16, BF16: ml_dtypes.bfloat16,
            I16: np.int16}
    in_maps = []
    for c in range(8):
        b = c // 4
        q0 = (c % 4) * NQ
        rhsg, gvpair, s_c, nv = batch_sup[b]

        # k-d reorder queries so each 128-tile is spatially compact
        qraw = qx[b, q0:q0 + NQ]
        perm = np.concatenate(_kd_leaves(qraw, np.arange(NQ), 4))
        _PERMS[c] = perm
        q = qraw[perm]
        qmk = qm[b, q0:q0 + NQ].astype(np.float32)[perm]

        qh, ql = _split_hilo(q)
        q64, qh64 = q.astype(np.float64), qh.astype(np.float64)
        Lq = (np.sum(q64 * q64, 1) - np.sum(qh64 * qh64, 1)).astype(np.float32)
        qh2 = np.sum(qh64 * qh64, 1).astype(np.float32)
        hi_q, lo_q = _split_hilo(qh2, 512.0)
        lhsq = np.zeros((DROWS, NQ), np.float32)
        lhsq[0:3] = -2.0 * qh.T
        lhsq[3:6] = ql.T
        lhsq[6:9] = q.T
        lhsq[9] = hi_q
        lhsq[10] = lo_q + Lq
        lhsq[11] = BIG * (1 - qmk)
        lhsq[12] = 1.0
        lhsq[13] = 1.0
        lhsq[14] = 1.0

        # per-tile support subsets (within bbox + RADIUS), tile-local tables
        rhsd2t = np.zeros((DROWS, NQT * SUB), np.float32)
        rhsd2t[14] = BIG
        gvt = np.zeros((4, 32, 2 * GT * SUB), np.float32)   # per group
        for i in range(NQT):
            qt = q[i * 128:(i + 1) * 128]
            lo = qt.min(0) - RADIUS
            hi = qt.max(0) + RADIUS
            subs = np.nonzero(np.all((s_c >= lo) & (s_c <= hi), axis=1))[0]
            if len(subs) == 0 or subs[0] != 0:
                subs = np.concatenate([[0], subs])   # fill fallback -> entry 0
            ns = len(subs)
            assert ns <= SUB, (ns, SUB)
            rhsd2t[:, i * SUB:i * SUB + ns] = rhsg[:, subs]
            g_, t_ = i // GT, i % GT
            pcols = (2 * subs[:, None] + np.arange(2)[None, :]).ravel()
            gvt[g_, :, 2 * t_ * SUB:2 * t_ * SUB + 2 * ns] = gvpair[:, pcols]

        # qdB[32g + u, i*32 + q'] = t0[u] - P0 @ q(i*128 + 32g + q')
        P0q = (P0 @ q.T.astype(np.float64)).reshape(32, NQT, 4, 32)
        qdB = np.zeros((128, 512), np.float64)
        for g in range(4):
            qdB[32 * g:32 * g + 32, :] = (
                t0[:, None] - P0q[:, :, g, :].reshape(32, NQT * 32))

        im = dict(
            lhsq=lhsq, rhsd2t=rhsd2t, qdB=qdB,
            gvt0=np.tile(gvt[0], (4, 1)), gvt1=np.tile(gvt[1], (4, 1)),
            gvt2=np.tile(gvt[2], (4, 1)), gvt3=np.tile(gvt[3], (4, 1)),
            t1v=t1v, t2v=t2v,
            w1t4=w1t4, w2t=w2t,
            mh0=Mh[0], mh1=Mh[1], ident=ident,
            pow8=pow8, iotag=iotag, shv=shv, tpat16=tpat16,
            qfm=qmk.reshape(NQT, 128).T.copy(),
            onesk1=onesk1,
        )
        for k in im:
            shape, dt = IN_SPECS[k]
            arr = np.ascontiguousarray(im[k]).astype(npdt[dt])
            assert arr.shape == shape, (k, arr.shape, shape)
            im[k] = arr
        in_maps.append(im)
    return in_maps


def host_finish(results):
    out = np.zeros((2, 128, 8192), np.float32)
    for c in range(8):
        b = c // 4
        q0 = (c % 4) * NQ
        out[b][:, q0 + _PERMS[c]] = results[c]['out']
    return out


IN_SPECS = dict(
    lhsq=((DROWS, NQ), F16), rhsd2t=((DROWS, NQT * SUB), F16),
    qdB=((128, 512), F32),
    gvt0=((128, 2 * GT * SUB), BF16), gvt1=((128, 2 * GT * SUB), BF16),
    gvt2=((128, 2 * GT * SUB), BF16), gvt3=((128, 2 * GT * SUB), BF16),
    t1v=((32, 1), F32), t2v=((128, 1), F32),
    w1t4=((128, 32), BF16), w2t=((32, 128), BF16),
    mh0=((128, 128), F32), mh1=((128, 128), F32), ident=((128, 128), F32),
    pow8=((128, 1024), BF16), iotag=((128, GT * PAIRS), I16), shv=((128, 8), I16),
    tpat16=((128, 544), I16), qfm=((128, NQT), F32), onesk1=((1, 128), F32),
)


# --------------------------------------------------------------------------
# device kernel
# --------------------------------------------------------------------------

@with_exitstack
def build_kernel(ctx: ExitStack, tc: tile.TileContext, out_ap: bass.AP, ins: dict):
    nc = tc.nc
    ctx.enter_context(nc.allow_low_precision("bf16 mlp + exact small-int sums"))

    consts = ctx.enter_context(tc.tile_pool(name="consts", bufs=1))
    gvp = ctx.enter_context(tc.tile_pool(name="gv", bufs=1))
    selp = ctx.enter_context(tc.tile_pool(name="sel", bufs=2))
    selp1 = ctx.enter_context(tc.tile_pool(name="sel1", bufs=1))
    selp2 = ctx.enter_context(tc.tile_pool(name="sel2", bufs=1))
    smallp = ctx.enter_context(tc.tile_pool(name="small", bufs=1))
    idxwp = ctx.enter_context(tc.tile_pool(name="idxw", bufs=2))
    gvtp = ctx.enter_context(tc.tile_pool(name="gvt", bufs=2))
    mlpp = ctx.enter_context(tc.tile_pool(name="mlp", bufs=2))
    mlpp1 = ctx.enter_context(tc.tile_pool(name="mlp1", bufs=1))
    mlph3 = ctx.enter_context(tc.tile_pool(name="mlph3", bufs=2))
    outp = ctx.enter_context(tc.tile_pool(name="outb", bufs=1))
    ps_d2 = ctx.enter_context(tc.tile_pool(name="psd2", bufs=2, space="PSUM"))
    ps_l2 = ctx.enter_context(tc.tile_pool(name="psl2", bufs=2, space="PSUM"))
    ps_l3 = ctx.enter_context(tc.tile_pool(name="psl3", bufs=2, space="PSUM"))

    GVT_NAMES = ("gvt0", "gvt1", "gvt2", "gvt3")
    ct = {}
    for name, (shape, dt) in IN_SPECS.items():
        if name in GVT_NAMES or name == "rhsd2t":
            continue
        t = consts.tile(list(shape), dt, tag=f"c_{name}")
        nc.sync.dma_start(out=t[:], in_=ins[name])
        ct[name] = t
    qdB = ct['qdB']

    c33 = consts.tile([128, 544], BF16, tag="c33")
    nc.vector.memset(c33[:], 33.0)
    ones34 = consts.tile([128, 34], I16, tag="ones34")
    nc.vector.memset(ones34[:], 1)

    # persistent per-core state
    idxall = gvp.tile([128, 512], F32, tag="idxall")   # final idx per qtile (f32)
    ceffall = gvp.tile([128, NQT], F32, tag="ceffall")
    outbuf = outp.tile([128, NQ], F32, tag="outbuf")

    def load_gvt(g):
        gvtg = gvtp.tile([128, 2 * GT * SUB], BF16, tag="gvtg")
        nc.sync.dma_start(out=gvtg[:], in_=ins[GVT_NAMES[g]])
        return gvtg

    # ---- phase A for a whole group of GT query tiles: per-tile d2 matmuls +
    # mask words, then batched selection post-processing ----
    GP = GT * PAIRS   # 384
    GS = GT * 544     # 2176

    def emit_group_A1(g):
        """d2 matmuls + in-radius mask words for group g (no MLP deps)."""
        rhsg2 = gvtp.tile([DROWS, GT * SUB], F16, tag="rhsg2")
        nc.sync.dma_start(out=rhsg2[:],
                          in_=ins['rhsd2t'][:, g * GT * SUB:(g + 1) * GT * SUB])
        w8g = selp1.tile([128, GT * WORDS], BF16, tag="w8g")
        for t in range(GT):
            i = g * GT + t
            for (off, csz) in CHUNKS:
                pd2 = ps_d2.tile([128, 512], F32, tag="ps_d2")
                nc.tensor.matmul(
                    pd2[:], ct['lhsq'][:, bass.ts(i, 128)],
                    rhsg2[:, t * SUB + off:t * SUB + off + csz],
                    start=True, stop=True)
                vw8c = selp.tile([128, 512], BF16, tag="vw8c")
                nc.vector.scalar_tensor_tensor(
                    vw8c[:], pd2[:], R2, ct['pow8'][:, 0:512],
                    op0=ALU.is_lt, op1=ALU.mult)
                nc.vector.tensor_reduce(
                    w8g[:, t * WORDS + off // 8:t * WORDS + (off + csz) // 8],
                    vw8c[:].rearrange("p (w t) -> p w t", t=8),
                    mybir.AxisListType.X, ALU.add)
        return w8g

    def emit_group_A2(g, w8g):
        """batched selection post-processing -> wrapped gather indices."""
        w8v = w8g[:].rearrange("p (c two) -> p c two", two=2)   # c = GP
        # scatter sources converted on the Pool engine (same queue as the
        # scatters that consume them -- no cross-engine hop, Pool is idle)
        w8e = selp2.tile([128, GP], I16, tag="w8e")
        w8o = selp2.tile([128, GP], I16, tag="w8o")
        nc.gpsimd.tensor_scalar(w8e[:], w8v[:, :, 0], 0.0, None, ALU.add)
        nc.gpsimd.tensor_scalar(w8o[:], w8v[:, :, 1], 0.0, None, ALU.add)

        nz = smallp.tile([128, GP], BF16, tag="nz")
        nc.vector.tensor_tensor(nz[:], w8v[:, :, 0], w8v[:, :, 1],
                                ALU.logical_or)
        crank = smallp.tile([128, GP], BF16, tag="crank")
        for t in range(GT):
            nc.vector.tensor_tensor_scan(
                crank[:, t * PAIRS:(t + 1) * PAIRS],
                nz[:, t * PAIRS:(t + 1) * PAIRS], c33[:, 0:PAIRS], 0.0,
                ALU.add, ALU.min)
        u = smallp.tile([128, GP], BF16, tag="u")
        nc.vector.tensor_tensor(u[:], crank[:], nz[:], ALU.mult)
        v = smallp.tile([128, GP], BF16, tag="nz")
        nc.vector.scalar_tensor_tensor(v[:], u[:], 32.5, u[:], op0=ALU.is_le,
                                       op1=ALU.mult)
        si16 = selp2.tile([128, GP], I16, tag="si16")
        nc.vector.tensor_scalar(si16[:], v[:], -1.0, None, ALU.add)

        dstID = selp2.tile([128, GT * 34], I16, tag="dstID")
        dstWe = selp2.tile([128, GT * 34], I16, tag="dstWe")
        dstWo = selp2.tile([128, GT * 34], I16, tag="dstWo")
        for t in range(GT):
            sl_ = slice(t * PAIRS, (t + 1) * PAIRS)
            ds_ = slice(t * 34, (t + 1) * 34)
            nc.gpsimd.local_scatter(dstID[:, ds_], ct['iotag'][:, sl_],
                                    si16[:, sl_], 128, 34, PAIRS)
            nc.gpsimd.local_scatter(dstWe[:, ds_], w8e[:, sl_], si16[:, sl_],
                                    128, 34, PAIRS)
            nc.gpsimd.local_scatter(dstWo[:, ds_], w8o[:, sl_], si16[:, sl_],
                                    128, 34, PAIRS)

        esel16 = selp2.tile([128, GS], I16, tag="esel16")
        # per tile: esel col s*16 + b*8 + u  <-  bit u of dstW(b) col s
        evb = esel16[:].rearrange("p (T s b u) -> p b u T s", s=34, b=2, u=8)
        onesT34 = ones34[:].unsqueeze(1).broadcast_to((128, GT, 34))
        for bidx, dstWx in ((0, dstWe), (1, dstWo)):
            dwv = dstWx[:].rearrange("p (T s) -> p T s", s=34)
            for t in range(8):
                nc.vector.scalar_tensor_tensor(evb[:, bidx, t], dwv,
                                               ct['shv'][:, t:t + 1], onesT34,
                                               op0=ALU.logical_shift_right,
                                               op1=ALU.bitwise_and)
        esel = smallp.tile([128, GS], BF16, tag="esel")
        nc.gpsimd.tensor_scalar(esel[:], esel16[:], 0.0, None, ALU.add)
        cjp1 = selp2.tile([128, GS], I16, tag="cjp1")
        nc.vector.tensor_tensor(
            cjp1[:].rearrange("p (T s u) -> p T s u", s=34, u=16),
            dstID[:].rearrange("p (T s) -> p T s", s=34)
                .unsqueeze(3).broadcast_to((128, GT, 34, 16)),
            ct['tpat16'][:].rearrange("p (s u) -> p s u", u=16)
                .unsqueeze(1).broadcast_to((128, GT, 34, 16)),
            ALU.add)

        crank2 = smallp.tile([128, GS], BF16, tag="crank2")
        for t in range(GT):
            nc.vector.tensor_tensor_scan(
                crank2[:, t * 544:(t + 1) * 544],
                esel[:, t * 544:(t + 1) * 544], c33[:], 0.0, ALU.add, ALU.min)
        # effective count (with query-mask fallback to 32)
        cnt0 = smallp.tile([128, GT], F32, tag="cnt0")
        nc.vector.tensor_scalar(
            cnt0[:], crank2[:].rearrange("p (T x) -> p T x", x=544)[:, :, 543],
            32.0, None, ALU.min)
        qfc = smallp.tile([128, GT], F32, tag="qfc")
        nc.vector.tensor_scalar(qfc[:], ct['qfm'][:, g * GT:(g + 1) * GT],
                                -32.0, 32.0, ALU.mult, ALU.add)
        nc.vector.tensor_tensor(ceffall[:, g * GT:(g + 1) * GT], cnt0[:],
                                qfc[:], ALU.max)
        u2 = smallp.tile([128, GS], BF16, tag="u2")
        nc.vector.tensor_tensor(u2[:], crank2[:], esel[:], ALU.mult)
        v2 = smallp.tile([128, GS], BF16, tag="esel")
        nc.vector.scalar_tensor_tensor(v2[:], u2[:], 32.5, u2[:], op0=ALU.is_le,
                                       op1=ALU.mult)
        si2 = selp2.tile([128, GS], I16, tag="si2")
        nc.vector.tensor_scalar(si2[:], v2[:], -1.0, None, ALU.add)
        idxp1 = selp2.tile([128, GT * 34], I16, tag="idxp1")
        for t in range(GT):
            nc.gpsimd.local_scatter(idxp1[:, t * 34:(t + 1) * 34],
                                    cjp1[:, t * 544:(t + 1) * 544],
                                    si2[:, t * 544:(t + 1) * 544], 128, 34, 544)

        # fill + final gather indices (kept in f32 for the phase-B matmul);
        # idx values are slot*SUB + local; local fallback 0 = global point 0
        ii = smallp.tile([128, GT * 32], F32, tag="ii")
        nc.gpsimd.tensor_scalar(
            ii[:].rearrange("p (T r) -> p T r", r=32),
            idxp1[:].rearrange("p (T s) -> p T s", s=34)[:, :, 0:32],
            0.0, None, ALU.add)
        iv = ii[:].rearrange("p (T r) -> p T r", r=32)
        flp1 = smallp.tile([128, GT], F32, tag="flp1")
        nc.vector.tensor_scalar(flp1[:], iv[:, :, 0], 1.0, None, ALU.max)
        flb = flp1[:].unsqueeze(2).broadcast_to((128, GT, 32))
        m = smallp.tile([128, GT * 32], F32, tag="m")
        nc.vector.tensor_scalar(m[:], ii[:], 0.0, None, ALU.is_gt)
        bb = smallp.tile([128, GT * 32], F32, tag="bb")
        nc.vector.tensor_tensor(bb[:], ii[:], m[:], ALU.mult)
        aa = smallp.tile([128, GT * 32], F32, tag="aa")
        nc.vector.tensor_tensor(aa[:].rearrange("p (T r) -> p T r", r=32),
                                m[:].rearrange("p (T r) -> p T r", r=32), flb,
                                ALU.mult)
        cc = smallp.tile([128, GT * 32], F32, tag="m")
        nc.vector.tensor_tensor(cc[:], bb[:], aa[:], ALU.subtract)
        dd0 = smallp.tile([128, GT * 32], F32, tag="bb")
        nc.vector.tensor_tensor(dd0[:].rearrange("p (T r) -> p T r", r=32),
                                cc[:].rearrange("p (T r) -> p T r", r=32), flb,
                                ALU.add)
        nc.vector.tensor_scalar(idxall[:, g * GT * 32:(g + 1) * GT * 32],
                                dd0[:], -1.0, None, ALU.add)

        # wrapped gather index slots via two group-wide permutation matmuls
        psWg = ps_d2.tile([128, 256], F32, tag="ps_d2")
        nc.tensor.matmul(psWg[:, 0:128], ct['mh0'][:],
                         idxall[:, g * 128:(g + 1) * 128], start=True, stop=True)
        nc.tensor.matmul(psWg[:, 128:256], ct['mh1'][:],
                         idxall[:, g * 128:(g + 1) * 128], start=True, stop=True)
        idxwg = idxwp.tile([128, GT * 64], I16, tag="idxwg")
        ivw = idxwg[:].rearrange("p (T k) -> p T k", k=64)
        nc.scalar.activation(ivw[:, :, 0::2],
                             psWg[:, 0:128].rearrange("p (T r) -> p T r", r=32),
                             ACTF.Copy)
        nc.scalar.activation(ivw[:, :, 1::2],
                             psWg[:, 128:256].rearrange("p (T r) -> p T r", r=32),
                             ACTF.Copy)
        return idxwg

    # ---- phase B: gathered-MLP + pooling for query tile i ----
    def emit_mlp(i, gout):
        gv_g = gout.rearrange("p (r q u) -> p r q u", r=32, u=2)[:, :, :, 0]
        gv_v0 = gout.rearrange("p (k u) -> p k u", u=2)[:, 0:32, 1]

        # d = V(center) + qdelta; h1 = relu(G + d)
        dd = mlpp.tile([128, 32], F32, tag="dd")
        nc.vector.tensor_tensor(dd[:], gv_v0, qdB[:, bass.ts(i, 32)], ALU.add)
        h1t = mlpp1.tile([128, 1024], F32, tag="h1t")
        nc.vector.tensor_tensor(
            h1t[:].rearrange("p (r q) -> p r q", q=32), gv_g,
            dd[:].unsqueeze(1).broadcast_to((128, 32, 32)), ALU.add)
        h1 = mlpp.tile([128, 1024], BF16, tag="h1")
        nc.scalar.activation(h1[:], h1t[:], ACTF.Relu)

        # layer 2: per unit uu (K=32 at partition 32*uu)
        h2 = mlpp1.tile([32, 4096], BF16, tag="h2")
        for uu in range(4):
            psL2 = ps_l2.tile([32, 1024], F32, tag="ps_a")
            for n in range(2):
                nc.tensor.matmul(
                    psL2[:, bass.ts(n, 512)],
                    ct['w1t4'][32 * uu:32 * uu + 32, :],
                    h1[32 * uu:32 * uu + 32, bass.ts(n, 512)],
                    start=True, stop=True,
                    tile_position=(32 * uu, 0))
            nc.scalar.activation(h2[:, bass.ts(uu, 1024)], psL2[:], ACTF.Relu,
                                 bias=ct['t1v'][:])

        # layer 3
        h3 = mlph3.tile([128, 4096], BF16, tag="h3")
        for n3 in range(8):
            psL3 = ps_l3.tile([128, 512], F32, tag="ps_b3")
            nc.tensor.matmul(psL3[:], ct['w2t'][:], h2[:, bass.ts(n3, 512)],
                             start=True, stop=True)
            nc.scalar.activation(h3[:, bass.ts(n3, 512)], psL3[:], ACTF.Relu,
                                 bias=ct['t2v'][:])

        # pooling: butterfly add-tree over the 32 neighbors; the first four
        # levels run in bf16 (2x DVE rate), the last writes f32
        h3v = h3[:].rearrange("p (a r q) -> p a r q", a=4, r=32)
        pt1 = smallp.tile([128, 2048], BF16, tag="pt1")
        p1v = pt1[:].rearrange("p (a r q) -> p a r q", a=4, r=16)
        nc.vector.tensor_tensor(p1v, h3v[:, :, 0:16, :], h3v[:, :, 16:32, :],
                                ALU.add)
        pt2 = smallp.tile([128, 1024], BF16, tag="pt2")
        p2v = pt2[:].rearrange("p (a r q) -> p a r q", a=4, r=8)
        nc.vector.tensor_tensor(p2v, p1v[:, :, 0:8, :], p1v[:, :, 8:16, :],
                                ALU.add)
        pt3 = smallp.tile([128, 512], BF16, tag="pt3")
        p3v = pt3[:].rearrange("p (a r q) -> p a r q", a=4, r=4)
        nc.vector.tensor_tensor(p3v, p2v[:, :, 0:4, :], p2v[:, :, 4:8, :],
                                ALU.add)
        pt4 = smallp.tile([128, 256], BF16, tag="pt4")
        p4v = pt4[:].rearrange("p (a r q) -> p a r q", a=4, r=2)
        nc.vector.tensor_tensor(p4v, p3v[:, :, 0:2, :], p3v[:, :, 2:4, :],
                                ALU.add)
        S = smallp.tile([128, 128], F32, tag="S")
        nc.vector.tensor_tensor(S[:].rearrange("p (a q) -> p a q", a=4),
                                p4v[:, :, 0, :], p4v[:, :, 1, :], ALU.add)

        # beta/gamma rows via PE transpose + broadcast
        ceff = ceffall[:, i:i + 1]
        beta = smallp.tile([128, 1], F32, tag="beta")
        nc.vector.reciprocal(beta[:], ceff)
        gm0 = smallp.tile([128, 1], F32, tag="gm0")
        nc.vector.tensor_scalar(gm0[:], ceff, -1.0, 32.0, ALU.mult, ALU.add)
        gamma = smallp.tile([128, 1], F32, tag="gamma")
        nc.vector.tensor_tensor(gamma[:], gm0[:], beta[:], ALU.mult)
        psBG = ps_d2.tile([1, 256], F32, tag="ps_d2")
        nc.tensor.matmul(psBG[:, 0:128], beta[:], ct['ident'][:],
                         start=True, stop=True)
        nc.tensor.matmul(psBG[:, 128:256], gamma[:], ct['ident'][:],
                         start=True, stop=True)
        bgrow = smallp.tile([1, 256], F32, tag="bgrow")
        nc.vector.tensor_copy(bgrow[:], psBG[:])
        psB = ps_d2.tile([128, 256], F32, tag="ps_d2")
        nc.tensor.matmul(psB[:], ct['onesk1'][:], bgrow[:], start=True, stop=True)

        e1 = smallp.tile([128, 128], F32, tag="e1")
        nc.vector.tensor_tensor(e1[:], S[:], psB[:, 0:128], ALU.mult)
        e2 = smallp.tile([128, 128], F32, tag="e2")
        nc.vector.tensor_tensor(
            e2[:].rearrange("p (a q) -> p a q", a=4), h3v[:, :, 0, :],
            psB[:, 128:256].rearrange("p (a q) -> p a q", a=4), ALU.mult)
        nc.vector.tensor_tensor(outbuf[:, bass.ts(i, 128)], e1[:], e2[:],
                                ALU.subtract)

    # ==== software-pipelined groups ====
    # engine-queue order per group g: gather(g) runs while DVE chews the
    # independent A1(g+1) mask work; mlp(g) then overlaps the A2(g+1)
    # selection chain.
    gvtg = load_gvt(0)
    idxwg = emit_group_A2(0, emit_group_A1(0))
    for g in range(NGRP):
        goutg = mlpp.tile([128, GT * 2048], BF16, tag="goutg")
        nc.gpsimd.ap_gather(goutg[:].rearrange("p (k u) -> p k u", u=2),
                            gvtg[:].rearrange("p (j u) -> p j u", u=2),
                            idxwg[:], 128, GT * SUB, 2, GT * 1024)
        if g + 1 < NGRP:
            gvtg = load_gvt(g + 1)
            w8gn = emit_group_A1(g + 1)
        for t in range(GT):
            emit_mlp(g * GT + t, goutg[:, t * 2048:(t + 1) * 2048])
        if g + 1 < NGRP:
            idxwg = emit_group_A2(g + 1, w8gn)
        nc.sync.dma_start(out=out_ap[:, g * GT * 128:(g + 1) * GT * 128],
                          in_=outbuf[:, g * GT * 128:(g + 1) * GT * 128])


# ==========================================================================
# harness entry point: kernel(**inputs) -> full output [2, 128, 8192]
# ==========================================================================

_CACHE = {}


def _build_nc():
    import concourse.bacc as bacc
    import concourse.tile as tile_mod
    nc = bacc.Bacc("TRN2", target_bir_lowering=False, debug=False, num_devices=8)
    in_tiles = {}
    for name, (shape, dt) in IN_SPECS.items():
        in_tiles[name] = nc.dram_tensor(
            name, list(shape), dt, kind="ExternalInput").ap()
    out_tile = nc.dram_tensor("out", (128, NQ), F32, kind="ExternalOutput").ap()
    with tile_mod.TileContext(nc) as t:
        build_kernel(t, out_tile, in_tiles)
    nc.compile()
    return nc


def kernel(**inputs):
    from concourse.bass_utils import run_bass_kernel_spmd
    in_maps = host_prep(inputs)
    if "nc" not in _CACHE:
        _CACHE["nc"] = _build_nc()
    res = run_bass_kernel_spmd(_CACHE["nc"], in_maps, list(range(8)))
    return host_finish(res.results)


# revision 56
# speedup vs baseline: 1.9302x; 1.0901x over previous
"""Trainium2 Bass kernel for nn_PointWiseMLP (ball query + gather + MLP + pool).

Self-contained: kernel(**inputs) shards across 8 NeuronCores (data-parallel
over batch x query-range), runs the Bass/Tile kernel via run_bass_kernel_spmd,
and gathers the full [2, 128, 8192] output.

v3: - support compacted by support_mask on host (order-preserving) and
      k-d-tree query reordering so each 128-query tile is spatially compact;
      each tile only tests the support points inside its radius-expanded bbox
      (<= SUB=1536 of 8192), cutting ball-query work ~5x.
    - per-tile (G,V) pair tables are built on host in tile-local index space
      and DMA'd per group, so the selection pipeline's local indices feed the
      gather directly (no index translation anywhere).
    - d2 decomposed into 15 fp16 rows (4x faster on PE than f32, ~1e-6 exact).
    - selection post-processing batched per 4-tile group, scan values in bf16,
      scatter-source conversions on the (idle) gpsimd engine.
"""
import sys
for _p in ("/opt/trn_rl_repo", "/root/.axon_site/_ro/trn_rl_repo"):
    if _p not in sys.path:
        sys.path.append(_p)


import numpy as np
from contextlib import ExitStack

import concourse.bass as bass
import concourse.tile as tile
from concourse import mybir
from concourse._compat import with_exitstack

F32 = mybir.dt.float32
F16 = mybir.dt.float16
BF16 = mybir.dt.bfloat16
I16 = mybir.dt.int16

RADIUS = 0.1
NSAMPLE = 32
EPS = 1e-5
N2C = 4608         # compacted+padded support count (host-side bound)
SUB = 1536         # per-tile support subset budget
WORDS = SUB // 8   # 192
PAIRS = SUB // 16  # 96
NQ = 2048          # queries per core
NQT = 16           # query tiles per core
GT = 4             # query tiles per gather group
NGRP = NQT // GT   # 4
BIG = 1024.0       # exactly representable in fp16
R2 = float(np.float32(0.01))  # threshold as f32
DROWS = 15         # fp16 d2 decomposition rows
CHUNKS = [(0, 512), (512, 512), (1024, 512)]

ALU = mybir.AluOpType
ACTF = mybir.ActivationFunctionType


# --------------------------------------------------------------------------
# host-side preparation
# --------------------------------------------------------------------------

def _split_hilo(x, grid=1024.0):
    """Grid split: x = hi + lo with hi on 1/grid grid (exact in fp16 for the
    value ranges used here)."""
    x = x.astype(np.float32)
    hi = np.floor(x.astype(np.float64) * grid) / grid
    hi = hi.astype(np.float32)
    lo = (x - hi).astype(np.float32)
    return hi, lo


def _kd_leaves(pts, idx, depth):
    if depth == 0:
        return [idx]
    ext = pts[idx].max(0) - pts[idx].min(0)
    ax = int(np.argmax(ext))
    order = idx[np.argsort(pts[idx, ax], kind="stable")]
    h = len(order) // 2
    return (_kd_leaves(pts, order[:h], depth - 1)
            + _kd_leaves(pts, order[h:], depth - 1))


_PERMS = {}


def host_prep(inputs):
    B = 2
    qx = np.asarray(inputs['query_xyz'], np.float32)
    sx = np.asarray(inputs['support_xyz'], np.float32)
    qm = np.asarray(inputs['query_mask'], np.int32)
    sm = np.asarray(inputs['support_mask'], np.int32)
    sf = np.asarray(inputs['support_features'], np.float32)

    W0 = np.asarray(inputs['W0'], np.float64)
    W1 = np.asarray(inputs['W1'], np.float64)
    W2 = np.asarray(inputs['W2'], np.float64)

    def fold(Wl, g, b, rm, rv):
        s = np.asarray(g, np.float64) / np.sqrt(np.asarray(rv, np.float64) + EPS)
        return Wl * s[:, None], np.asarray(b, np.float64) - np.asarray(rm, np.float64) * s

    W0p, t0 = fold(W0, inputs['g0'], inputs['b0'], inputs['rm0'], inputs['rv0'])
    W1p, t1 = fold(W1, inputs['g1'], inputs['b1'], inputs['rm1'], inputs['rv1'])
    W2p, t2 = fold(W2, inputs['g2'], inputs['b2'], inputs['rm2'], inputs['rv2'])

    P0 = W0p[:, 0:3] / RADIUS
    C0 = W0p[:, 3:67]
    D0 = W0p[:, 67:131]

    w1t4 = np.tile(W1p.T.astype(np.float32), (4, 1))       # [128, 32]
    w2t4 = np.tile(W2p.T.astype(np.float32), (4, 1))       # [128, 128]
    t1v4 = np.tile(t1.astype(np.float32), 4).reshape(128, 1)
    t2v = t2.astype(np.float32).reshape(128, 1)

    # permutation matmul weights for the wrapped gather index layout:
    # idxw[p, 2r+h] = idxg[32*(p//32) + 16h + p%16, r]
    Mh = np.zeros((2, 128, 128), np.float32)
    for h in range(2):
        for p in range(128):
            Mh[h, 32 * (p // 32) + 16 * h + p % 16, p] = 1.0
    ident = np.eye(128, dtype=np.float32)

    pow8 = np.tile((2.0 ** (np.arange(1024) % 8)).astype(np.float32)[None, :], (128, 1))
    # scatter id source, pre-scaled by 16 so cjp1 = dstID + tpat16 yields
    # (slot*SUB + local_idx + 1) directly
    iotag = np.tile(((np.arange(GT * PAIRS, dtype=np.int16) + 1) * 16)[None, :],
                    (128, 1))
    shv = np.tile(np.arange(8, dtype=np.int16)[None, :], (128, 1))
    tpat16 = np.tile((np.tile(np.arange(16, dtype=np.int16), 34) - 15)[None, :],
                     (128, 1))
    onesk1 = np.ones((1, 128), np.float32)

    batch_sup = []
    for b in range(B):
        # order-preserving compaction by support_mask; original point 0 is
        # always table entry 0 (selection-masked if its mask is 0) so the
        # zero-neighbor fill gathers the same point the reference does.
        valid = sm[b] > 0
        keep = np.nonzero(valid)[0]
        sel0 = True
        if not valid[0]:
            keep = np.concatenate([[0], keep])
            sel0 = False
        nv = len(keep)
        assert nv <= N2C, (nv, N2C)
        s = np.zeros((N2C, 3), np.float32)
        s[:nv] = sx[b][keep]
        fts = np.zeros((64, N2C), np.float32)
        fts[:, :nv] = sf[b][:, keep]
        selmask = np.zeros(N2C, np.float32)
        selmask[:nv] = 1.0
        if not sel0:
            selmask[0] = 0.0

        # fp16 d2 decomposition (support side), global-compacted columns
        sh, sl = _split_hilo(s)
        s64, sh64 = s.astype(np.float64), sh.astype(np.float64)
        Ls = (np.sum(s64 * s64, 1) - np.sum(sh64 * sh64, 1)).astype(np.float32)
        sh2 = np.sum(sh64 * sh64, 1).astype(np.float32)
        hi_s, lo_s = _split_hilo(sh2, 512.0)
        rhsg = np.zeros((DROWS, N2C), np.float32)
        rhsg[0:3] = sh.T
        rhsg[3:6] = -2.0 * sh.T
        rhsg[6:9] = -2.0 * sl.T
        rhsg[9] = 1.0
        rhsg[10] = 1.0
        rhsg[11] = 1.0
        rhsg[12] = hi_s
        rhsg[13] = lo_s + Ls
        rhsg[14] = BIG * (1.0 - selmask)

        # (G,V) pair table in global-compacted index space
        G = D0 @ fts.astype(np.float64) + (P0 @ s.T.astype(np.float64))
        V = (C0 - D0) @ fts.astype(np.float64)
        gvpair = np.empty((32, 2 * N2C), np.float32)
        gvpair[:, 0::2] = G.astype(np.float32)
        gvpair[:, 1::2] = V.astype(np.float32)
        batch_sup.append((rhsg, gvpair, s[:nv], nv))

    import ml_dtypes
    npdt = {F32: np.float32, F16: np.float16, BF16: ml_dtypes.bfloat16,
            I16: np.int16}
    in_maps = []
    for c in range(8):
        b = c // 4
        q0 = (c % 4) * NQ
        rhsg, gvpair, s_c, nv = batch_sup[b]

        # k-d reorder queries so each 128-tile is spatially compact
        qraw = qx[b, q0:q0 + NQ]
        perm = np.concatenate(_kd_leaves(qraw, np.arange(NQ), 4))
        _PERMS[c] = perm
        q = qraw[perm]
        qmk = qm[b, q0:q0 + NQ].astype(np.float32)[perm]

        qh, ql = _split_hilo(q)
        q64, qh64 = q.astype(np.float64), qh.astype(np.float64)
        Lq = (np.sum(q64 * q64, 1) - np.sum(qh64 * qh64, 1)).astype(np.float32)
        qh2 = np.sum(qh64 * qh64, 1).astype(np.float32)
        hi_q, lo_q = _split_hilo(qh2, 512.0)
        lhsq = np.zeros((DROWS, NQ), np.float32)
        lhsq[0:3] = -2.0 * qh.T
        lhsq[3:6] = ql.T
        lhsq[6:9] = q.T
        lhsq[9] = hi_q
        lhsq[10] = lo_q + Lq
        lhsq[11] = BIG * (1 - qmk)
        lhsq[12] = 1.0
        lhsq[13] = 1.0
        lhsq[14] = 1.0

        # per-tile support subsets (within bbox + RADIUS), tile-local tables
        rhsd2t = np.zeros((DROWS, NQT * SUB), np.float32)
        rhsd2t[14] = BIG
        gvt = np.zeros((4, 32, 2 * GT * SUB), np.float32)   # per group
        for i in range(NQT):
            qt = q[i * 128:(i + 1) * 128]
            lo = qt.min(0) - RADIUS
            hi = qt.max(0) + RADIUS
            subs = np.nonzero(np.all((s_c >= lo) & (s_c <= hi), axis=1))[0]
            if len(subs) == 0 or subs[0] != 0:
                subs = np.concatenate([[0], subs])   # fill fallback -> entry 0
            ns = len(subs)
            assert ns <= SUB, (ns, SUB)
            rhsd2t[:, i * SUB:i * SUB + ns] = rhsg[:, subs]
            g_, t_ = i // GT, i % GT
            pcols = (2 * subs[:, None] + np.arange(2)[None, :]).ravel()
            gvt[g_, :, 2 * t_ * SUB:2 * t_ * SUB + 2 * ns] = gvpair[:, pcols]

        # qdB[32g + u, i*32 + q'] = t0[u] - P0 @ q(i*128 + 32g + q')
        P0q = (P0 @ q.T.astype(np.float64)).reshape(32, NQT, 4, 32)
        qdB = np.zeros((128, 512), np.float64)
        for g in range(4):
            qdB[32 * g:32 * g + 32, :] = (
                t0[:, None] - P0q[:, :, g, :].reshape(32, NQT * 32))

        im = dict(
            lhsq=lhsq, rhsd2t=rhsd2t, qdB=qdB,
            gvt0=np.tile(gvt[0], (4, 1)), gvt1=np.tile(gvt[1], (4, 1)),
            gvt2=np.tile(gvt[2], (4, 1)), gvt3=np.tile(gvt[3], (4, 1)),
            t1v4=t1v4, t2v=t2v,
            w1t4=w1t4, w2t4=w2t4,
            mh0=Mh[0], mh1=Mh[1], ident=ident,
            pow8=pow8, iotag=iotag, shv=shv, tpat16=tpat16,
            qfm=qmk.reshape(NQT, 128).T.copy(),
            onesk1=onesk1,
        )
        for k in im:
            shape, dt = IN_SPECS[k]
            arr = np.ascontiguousarray(im[k]).astype(npdt[dt])
            assert arr.shape == shape, (k, arr.shape, shape)
            im[k] = arr
        in_maps.append(im)
    return in_maps


def host_finish(results):
    out = np.zeros((2, 128, 8192), np.float32)
    for c in range(8):
        b = c // 4
        q0 = (c % 4) * NQ
        out[b][:, q0 + _PERMS[c]] = results[c]['out']
    return out


IN_SPECS = dict(
    lhsq=((DROWS, NQ), F16), rhsd2t=((DROWS, NQT * SUB), F16),
    qdB=((128, 512), F32),
    gvt0=((128, 2 * GT * SUB), BF16), gvt1=((128, 2 * GT * SUB), BF16),
    gvt2=((128, 2 * GT * SUB), BF16), gvt3=((128, 2 * GT * SUB), BF16),
    t1v4=((128, 1), F32), t2v=((128, 1), F32),
    w1t4=((128, 32), BF16), w2t4=((128, 128), BF16),
    mh0=((128, 128), F32), mh1=((128, 128), F32), ident=((128, 128), F32),
    pow8=((128, 1024), BF16), iotag=((128, GT * PAIRS), I16), shv=((128, 8), I16),
    tpat16=((128, 544), I16), qfm=((128, NQT), F32), onesk1=((1, 128), F32),
)


# --------------------------------------------------------------------------
# device kernel
# --------------------------------------------------------------------------

@with_exitstack
def build_kernel(ctx: ExitStack, tc: tile.TileContext, out_ap: bass.AP, ins: dict):
    nc = tc.nc
    ctx.enter_context(nc.allow_low_precision("bf16 mlp + exact small-int sums"))

    consts = ctx.enter_context(tc.tile_pool(name="consts", bufs=1))
    gvp = ctx.enter_context(tc.tile_pool(name="gv", bufs=1))
    selp = ctx.enter_context(tc.tile_pool(name="sel", bufs=2))
    selp1 = ctx.enter_context(tc.tile_pool(name="sel1", bufs=1))
    selp2 = ctx.enter_context(tc.tile_pool(name="sel2", bufs=1))
    smallp = ctx.enter_context(tc.tile_pool(name="small", bufs=1))
    idxwp = ctx.enter_context(tc.tile_pool(name="idxw", bufs=2))
    gvtp = ctx.enter_context(tc.tile_pool(name="gvt", bufs=2))
    mlpp = ctx.enter_context(tc.tile_pool(name="mlp", bufs=2))
    mlpp1 = ctx.enter_context(tc.tile_pool(name="mlp1", bufs=1))
    mlph3 = ctx.enter_context(tc.tile_pool(name="mlph3", bufs=2))
    outp = ctx.enter_context(tc.tile_pool(name="outb", bufs=1))
    ps_d2 = ctx.enter_context(tc.tile_pool(name="psd2", bufs=2, space="PSUM"))
    ps_mlp = ctx.enter_context(tc.tile_pool(name="psmlp", bufs=3, space="PSUM"))

    GVT_NAMES = ("gvt0", "gvt1", "gvt2", "gvt3")
    ct = {}
    for name, (shape, dt) in IN_SPECS.items():
        if name in GVT_NAMES or name == "rhsd2t":
            continue
        t = consts.tile(list(shape), dt, tag=f"c_{name}")
        nc.sync.dma_start(out=t[:], in_=ins[name])
        ct[name] = t
    qdB = ct['qdB']

    c33 = consts.tile([128, 544], BF16, tag="c33")
    nc.vector.memset(c33[:], 33.0)
    ones34 = consts.tile([128, 34], I16, tag="ones34")
    nc.vector.memset(ones34[:], 1)

    # persistent per-core state
    idxall = gvp.tile([128, 512], F32, tag="idxall")   # final idx per qtile (f32)
    outbuf = outp.tile([128, NQ], F32, tag="outbuf")

    def load_gvt(g):
        gvtg = gvtp.tile([128, 2 * GT * SUB], BF16, tag="gvtg")
        nc.sync.dma_start(out=gvtg[:], in_=ins[GVT_NAMES[g]])
        return gvtg

    # ---- phase A for a whole group of GT query tiles: per-tile d2 matmuls +
    # mask words, then batched selection post-processing ----
    GP = GT * PAIRS   # 384
    GS = GT * 544     # 2176

    def emit_group_A1(g):
        """d2 matmuls + in-radius mask words for group g (no MLP deps)."""
        rhsg2 = gvtp.tile([DROWS, GT * SUB], F16, tag="rhsg2")
        nc.sync.dma_start(out=rhsg2[:],
                          in_=ins['rhsd2t'][:, g * GT * SUB:(g + 1) * GT * SUB])
        w8g = selp1.tile([128, GT * WORDS], BF16, tag="w8g")
        for t in range(GT):
            i = g * GT + t
            for (off, csz) in CHUNKS:
                pd2 = ps_d2.tile([128, 512], F32, tag="ps_d2")
                nc.tensor.matmul(
                    pd2[:], ct['lhsq'][:, bass.ts(i, 128)],
                    rhsg2[:, t * SUB + off:t * SUB + off + csz],
                    start=True, stop=True)
                vw8c = selp.tile([128, 512], BF16, tag="vw8c")
                nc.vector.scalar_tensor_tensor(
                    vw8c[:], pd2[:], R2, ct['pow8'][:, 0:512],
                    op0=ALU.is_lt, op1=ALU.mult)
                nc.vector.tensor_reduce(
                    w8g[:, t * WORDS + off // 8:t * WORDS + (off + csz) // 8],
                    vw8c[:].rearrange("p (w t) -> p w t", t=8),
                    mybir.AxisListType.X, ALU.add)
        return w8g

    def emit_group_A2(g, w8g):
        """batched selection post-processing -> wrapped gather indices."""
        w8v = w8g[:].rearrange("p (c two) -> p c two", two=2)   # c = GP
        # scatter sources converted on the Pool engine (same queue as the
        # scatters that consume them -- no cross-engine hop, Pool is idle)
        w8e = selp2.tile([128, GP], I16, tag="w8e")
        w8o = selp2.tile([128, GP], I16, tag="w8o")
        nc.gpsimd.tensor_scalar(w8e[:], w8v[:, :, 0], 0.0, None, ALU.add)
        nc.gpsimd.tensor_scalar(w8o[:], w8v[:, :, 1], 0.0, None, ALU.add)

        nz = smallp.tile([128, GP], BF16, tag="nz")
        nc.vector.tensor_tensor(nz[:], w8v[:, :, 0], w8v[:, :, 1],
                                ALU.logical_or)
        crank = smallp.tile([128, GP], BF16, tag="crank")
        for t in range(GT):
            nc.vector.tensor_tensor_scan(
                crank[:, t * PAIRS:(t + 1) * PAIRS],
                nz[:, t * PAIRS:(t + 1) * PAIRS], c33[:, 0:PAIRS], 0.0,
                ALU.add, ALU.min)
        u = smallp.tile([128, GP], BF16, tag="u")
        nc.vector.tensor_tensor(u[:], crank[:], nz[:], ALU.mult)
        v = smallp.tile([128, GP], BF16, tag="nz")
        nc.vector.scalar_tensor_tensor(v[:], u[:], 32.5, u[:], op0=ALU.is_le,
                                       op1=ALU.mult)
        si16 = selp2.tile([128, GP], I16, tag="si16")
        nc.vector.tensor_scalar(si16[:], v[:], -1.0, None, ALU.add)

        dstID = selp2.tile([128, GT * 34], I16, tag="dstID")
        dstWe = selp2.tile([128, GT * 34], I16, tag="dstWe")
        dstWo = selp2.tile([128, GT * 34], I16, tag="dstWo")
        for t in range(GT):
            sl_ = slice(t * PAIRS, (t + 1) * PAIRS)
            ds_ = slice(t * 34, (t + 1) * 34)
            nc.gpsimd.local_scatter(dstID[:, ds_], ct['iotag'][:, sl_],
                                    si16[:, sl_], 128, 34, PAIRS)
            nc.gpsimd.local_scatter(dstWe[:, ds_], w8e[:, sl_], si16[:, sl_],
                                    128, 34, PAIRS)
            nc.gpsimd.local_scatter(dstWo[:, ds_], w8o[:, sl_], si16[:, sl_],
                                    128, 34, PAIRS)

        esel16 = selp2.tile([128, GS], I16, tag="esel16")
        # per tile: esel col s*16 + b*8 + u  <-  bit u of dstW(b) col s
        evb = esel16[:].rearrange("p (T s b u) -> p b u T s", s=34, b=2, u=8)
        onesT34 = ones34[:].unsqueeze(1).broadcast_to((128, GT, 34))
        for bidx, dstWx in ((0, dstWe), (1, dstWo)):
            dwv = dstWx[:].rearrange("p (T s) -> p T s", s=34)
            for t in range(8):
                nc.vector.scalar_tensor_tensor(evb[:, bidx, t], dwv,
                                               ct['shv'][:, t:t + 1], onesT34,
                                               op0=ALU.logical_shift_right,
                                               op1=ALU.bitwise_and)
        esel = smallp.tile([128, GS], BF16, tag="esel")
        nc.gpsimd.tensor_scalar(esel[:], esel16[:], 0.0, None, ALU.add)
        cjp1 = selp2.tile([128, GS], I16, tag="cjp1")
        nc.vector.tensor_tensor(
            cjp1[:].rearrange("p (T s u) -> p T s u", s=34, u=16),
            dstID[:].rearrange("p (T s) -> p T s", s=34)
                .unsqueeze(3).broadcast_to((128, GT, 34, 16)),
            ct['tpat16'][:].rearrange("p (s u) -> p s u", u=16)
                .unsqueeze(1).broadcast_to((128, GT, 34, 16)),
            ALU.add)

        crank2 = smallp.tile([128, GS], BF16, tag="crank2")
        for t in range(GT):
            nc.vector.tensor_tensor_scan(
                crank2[:, t * 544:(t + 1) * 544],
                esel[:, t * 544:(t + 1) * 544], c33[:], 0.0, ALU.add, ALU.min)
        # effective count (with query-mask fallback to 32)
        cnt0 = smallp.tile([128, GT], F32, tag="cnt0")
        nc.vector.tensor_scalar(
            cnt0[:], crank2[:].rearrange("p (T x) -> p T x", x=544)[:, :, 543],
            32.0, None, ALU.min)
        qfc = smallp.tile([128, GT], F32, tag="qfc")
        nc.vector.tensor_scalar(qfc[:], ct['qfm'][:, g * GT:(g + 1) * GT],
                                -32.0, 32.0, ALU.mult, ALU.add)
        ceff4 = smallp.tile([128, GT], F32, tag="ceff4")
        nc.vector.tensor_tensor(ceff4[:], cnt0[:], qfc[:], ALU.max)
        # batched pooling scalars: bgrow4 row t = [beta_t(q) | gamma_t(q)]
        beta4 = smallp.tile([128, GT], F32, tag="beta4")
        nc.vector.reciprocal(beta4[:], ceff4[:])
        gm04 = smallp.tile([128, GT], F32, tag="gm04")
        nc.vector.tensor_scalar(gm04[:], ceff4[:], -1.0, 32.0, ALU.mult, ALU.add)
        gamma4 = smallp.tile([128, GT], F32, tag="gamma4")
        nc.vector.tensor_tensor(gamma4[:], gm04[:], beta4[:], ALU.mult)

        u2 = smallp.tile([128, GS], BF16, tag="u2")
        nc.vector.tensor_tensor(u2[:], crank2[:], esel[:], ALU.mult)
        v2 = smallp.tile([128, GS], BF16, tag="esel")
        nc.vector.scalar_tensor_tensor(v2[:], u2[:], 32.5, u2[:], op0=ALU.is_le,
                                       op1=ALU.mult)
        si2 = selp2.tile([128, GS], I16, tag="si2")
        nc.vector.tensor_scalar(si2[:], v2[:], -1.0, None, ALU.add)
        idxp1 = selp2.tile([128, GT * 34], I16, tag="idxp1")
        for t in range(GT):
            nc.gpsimd.local_scatter(idxp1[:, t * 34:(t + 1) * 34],
                                    cjp1[:, t * 544:(t + 1) * 544],
                                    si2[:, t * 544:(t + 1) * 544], 128, 34, 544)

        # fill + final gather indices (kept in f32 for the phase-B matmul);
        # idx values are slot*SUB + local; local fallback 0 = global point 0
        ii = smallp.tile([128, GT * 32], F32, tag="ii")
        nc.gpsimd.tensor_scalar(
            ii[:].rearrange("p (T r) -> p T r", r=32),
            idxp1[:].rearrange("p (T s) -> p T s", s=34)[:, :, 0:32],
            0.0, None, ALU.add)
        iv = ii[:].rearrange("p (T r) -> p T r", r=32)
        flp1 = smallp.tile([128, GT], F32, tag="flp1")
        nc.vector.tensor_scalar(flp1[:], iv[:, :, 0], 1.0, None, ALU.max)
        flb = flp1[:].unsqueeze(2).broadcast_to((128, GT, 32))
        m = smallp.tile([128, GT * 32], F32, tag="m")
        nc.vector.tensor_scalar(m[:], ii[:], 0.0, None, ALU.is_gt)
        bb = smallp.tile([128, GT * 32], F32, tag="bb")
        nc.vector.tensor_tensor(bb[:], ii[:], m[:], ALU.mult)
        aa = smallp.tile([128, GT * 32], F32, tag="aa")
        nc.vector.tensor_tensor(aa[:].rearrange("p (T r) -> p T r", r=32),
                                m[:].rearrange("p (T r) -> p T r", r=32), flb,
                                ALU.mult)
        cc = smallp.tile([128, GT * 32], F32, tag="m")
        nc.vector.tensor_tensor(cc[:], bb[:], aa[:], ALU.subtract)
        dd0 = smallp.tile([128, GT * 32], F32, tag="bb")
        nc.vector.tensor_tensor(dd0[:].rearrange("p (T r) -> p T r", r=32),
                                cc[:].rearrange("p (T r) -> p T r", r=32), flb,
                                ALU.add)
        nc.vector.tensor_scalar(idxall[:, g * GT * 32:(g + 1) * GT * 32],
                                dd0[:], -1.0, None, ALU.add)

        # wrapped gather index slots via two group-wide permutation matmuls
        psWg = ps_d2.tile([128, 256], F32, tag="ps_d2")
        nc.tensor.matmul(psWg[:, 0:128], ct['mh0'][:],
                         idxall[:, g * 128:(g + 1) * 128], start=True, stop=True)
        nc.tensor.matmul(psWg[:, 128:256], ct['mh1'][:],
                         idxall[:, g * 128:(g + 1) * 128], start=True, stop=True)
        idxwg = idxwp.tile([128, GT * 64], I16, tag="idxwg")
        ivw = idxwg[:].rearrange("p (T k) -> p T k", k=64)
        nc.scalar.activation(ivw[:, :, 0::2],
                             psWg[:, 0:128].rearrange("p (T r) -> p T r", r=32),
                             ACTF.Copy)
        nc.scalar.activation(ivw[:, :, 1::2],
                             psWg[:, 128:256].rearrange("p (T r) -> p T r", r=32),
                             ACTF.Copy)
        return idxwg, (beta4, gamma4)

    # ---- phase B: gathered-MLP + pooling for query tile i ----
    def emit_mlp(i, t, gout, bg4):
        gv_g = gout.rearrange("p (r q u) -> p r q u", r=32, u=2)[:, :, :, 0]
        gv_v0 = gout.rearrange("p (k u) -> p k u", u=2)[:, 0:32, 1]

        # d = V(center) + qdelta; h1 = relu(G + d)
        dd = mlpp.tile([128, 32], F32, tag="dd")
        nc.vector.tensor_tensor(dd[:], gv_v0, qdB[:, bass.ts(i, 32)], ALU.add)
        h1t = mlpp1.tile([128, 1024], F32, tag="h1t")
        nc.vector.tensor_tensor(
            h1t[:].rearrange("p (r q) -> p r q", q=32), gv_g,
            dd[:].unsqueeze(1).broadcast_to((128, 32, 32)), ALU.add)
        h1 = mlpp.tile([128, 1024], BF16, tag="h1")
        nc.scalar.activation(h1[:], h1t[:], ACTF.Relu)

        # layer 2: all 4 unit-replicas into one PSUM tile (partition-offset
        # writes) -> single fused activation
        psL2 = ps_mlp.tile([128, 1024], F32, tag="ps_mlp")
        for uu in range(4):
            for n in range(2):
                nc.tensor.matmul(
                    psL2[32 * uu:32 * uu + 32, bass.ts(n, 512)],
                    ct['w1t4'][32 * uu:32 * uu + 32, :],
                    h1[32 * uu:32 * uu + 32, bass.ts(n, 512)],
                    start=True, stop=True,
                    tile_position=(32 * uu, 32 * uu))
        h2 = mlpp1.tile([128, 1024], BF16, tag="h2")
        nc.scalar.activation(h2[:], psL2[:], ACTF.Relu, bias=ct['t1v4'][:])

        # layer 3: per unit-replica uu (K=32 at partition 32*uu), 1024 cols
        h3 = mlph3.tile([128, 4096], BF16, tag="h3")
        for uu in range(4):
            psL3 = ps_mlp.tile([128, 1024], F32, tag="ps_mlp")
            for n in range(2):
                nc.tensor.matmul(psL3[:, bass.ts(n, 512)],
                                 ct['w2t4'][32 * uu:32 * uu + 32, :],
                                 h2[32 * uu:32 * uu + 32, bass.ts(n, 512)],
                                 start=True, stop=True,
                                 tile_position=(32 * uu, 0))
            nc.scalar.activation(h3[:, bass.ts(uu, 1024)], psL3[:], ACTF.Relu,
                                 bias=ct['t2v'][:])

        # pooling: butterfly add-tree over the 32 neighbors; the first four
        # levels run in bf16 (2x DVE rate), the last writes f32
        h3v = h3[:].rearrange("p (a r q) -> p a r q", a=4, r=32)
        pt1 = smallp.tile([128, 2048], BF16, tag="pt1")
        p1v = pt1[:].rearrange("p (a r q) -> p a r q", a=4, r=16)
        nc.vector.tensor_tensor(p1v, h3v[:, :, 0:16, :], h3v[:, :, 16:32, :],
                                ALU.add)
        pt2 = smallp.tile([128, 1024], BF16, tag="pt2")
        p2v = pt2[:].rearrange("p (a r q) -> p a r q", a=4, r=8)
        nc.vector.tensor_tensor(p2v, p1v[:, :, 0:8, :], p1v[:, :, 8:16, :],
                                ALU.add)
        pt3 = smallp.tile([128, 512], BF16, tag="pt3")
        p3v = pt3[:].rearrange("p (a r q) -> p a r q", a=4, r=4)
        nc.vector.tensor_tensor(p3v, p2v[:, :, 0:4, :], p2v[:, :, 4:8, :],
                                ALU.add)
        pt4 = smallp.tile([128, 256], BF16, tag="pt4")
        p4v = pt4[:].rearrange("p (a r q) -> p a r q", a=4, r=2)
        nc.vector.tensor_tensor(p4v, p3v[:, :, 0:2, :], p3v[:, :, 2:4, :],
                                ALU.add)
        S = smallp.tile([128, 128], F32, tag="S")
        nc.vector.tensor_tensor(S[:].rearrange("p (a q) -> p a q", a=4),
                                p4v[:, :, 0, :], p4v[:, :, 1, :], ALU.add)

        # broadcast this tile's beta/gamma to a [128,256] row block
        beta4, gamma4 = bg4
        psBG = ps_d2.tile([1, 256], F32, tag="ps_d2")
        nc.tensor.matmul(psBG[:, 0:128], beta4[:, t:t + 1], ct['ident'][:],
                         start=True, stop=True)
        nc.tensor.matmul(psBG[:, 128:256], gamma4[:, t:t + 1], ct['ident'][:],
                         start=True, stop=True)
        bgrow = smallp.tile([1, 256], F32, tag="bgrow")
        nc.scalar.activation(bgrow[:], psBG[:], ACTF.Copy)
        psB = ps_d2.tile([128, 256], F32, tag="ps_d2")
        nc.tensor.matmul(psB[:], ct['onesk1'][:], bgrow[:], start=True, stop=True)

        e1 = smallp.tile([128, 128], F32, tag="e1")
        nc.vector.tensor_tensor(e1[:], S[:], psB[:, 0:128], ALU.mult)
        e2 = smallp.tile([128, 128], F32, tag="e2")
        nc.vector.tensor_tensor(
            e2[:].rearrange("p (a q) -> p a q", a=4), h3v[:, :, 0, :],
            psB[:, 128:256].rearrange("p (a q) -> p a q", a=4), ALU.mult)
        nc.vector.tensor_tensor(outbuf[:, bass.ts(i, 128)], e1[:], e2[:],
                                ALU.subtract)

    # ==== software-pipelined groups ====
    # engine-queue order per group g: gather(g) runs while DVE chews the
    # independent A1(g+1) mask work; mlp(g) then overlaps the A2(g+1)
    # selection chain.
    gvtg = load_gvt(0)
    idxwg, bg4 = emit_group_A2(0, emit_group_A1(0))
    for g in range(NGRP):
        goutg = mlpp.tile([128, GT * 2048], BF16, tag="goutg")
        nc.gpsimd.ap_gather(goutg[:].rearrange("p (k u) -> p k u", u=2),
                            gvtg[:].rearrange("p (j u) -> p j u", u=2),
                            idxwg[:], 128, GT * SUB, 2, GT * 1024)
        if g + 1 < NGRP:
            gvtg = load_gvt(g + 1)
            w8gn = emit_group_A1(g + 1)
        for t in range(GT):
            emit_mlp(g * GT + t, t, goutg[:, t * 2048:(t + 1) * 2048], bg4)
        if g + 1 < NGRP:
            idxwg, bg4 = emit_group_A2(g + 1, w8gn)
        nc.sync.dma_start(out=out_ap[:, g * GT * 128:(g + 1) * GT * 128],
                          in_=outbuf[:, g * GT * 128:(g + 1) * GT * 128])


# ==========================================================================
# harness entry point: kernel(**inputs) -> full output [2, 128, 8192]
# ==========================================================================

_CACHE = {}


def _build_nc():
    import concourse.bacc as bacc
    import concourse.tile as tile_mod
    nc = bacc.Bacc("TRN2", target_bir_lowering=False, debug=False, num_devices=8)
    in_tiles = {}
    for name, (shape, dt) in IN_SPECS.items():
        in_tiles[name] = nc.dram_tensor(
            name, list(shape), dt, kind="ExternalInput").ap()
    out_tile = nc.dram_tensor("out", (128, NQ), F32, kind="ExternalOutput").ap()
    with tile_mod.TileContext(nc) as t:
        build_kernel(t, out_tile, in_tiles)
    nc.compile()
    return nc


def kernel(**inputs):
    from concourse.bass_utils import run_bass_kernel_spmd
    in_maps = host_prep(inputs)
    if "nc" not in _CACHE:
        _CACHE["nc"] = _build_nc()
    res = run_bass_kernel_spmd(_CACHE["nc"], in_maps, list(range(8)))
    return host_finish(res.results)
